# revision 11
# baseline (speedup 1.0000x reference)
"""EnhancedRWKVBlock Trainium2 kernel (optimized).

Sharding: 8 cores = 4 batches x 2 sequence halves (pure data parallel).
The channel-mix token shift across the halves is seeded by one host-computed
row per odd shard.

Key optimizations over the f32r baseline:
  - All big matmuls run in bf16 (same PE rate as f32r, half the LDWEIGHTS
    time, half the weight DMA); the val/gate matmuls run in fp8e4 DoubleRow
    (2x PE rate) with host-quantized weights.
  - Weights are pre-arranged on host into [tile, 128, k, 128] layouts so
    every weight DMA is contiguous per partition (the baseline was DMA
    descriptor-bound with 2048 x 256B scatters per tile).
  - LN scale/bias are folded into the following projection weights on host;
    the level-mix 1/z is folded into e_t once instead of per (hout, sc).
  - No DRAM spills: xT/x1T/kk stay SBUF-resident.
  - sc-major phase ordering pipelines the vector-only LN2/token-shift block
    behind matmul phases, removing the PE bubble.
"""

import numpy as np
import ml_dtypes

B, T, H, D, FF = 4, 2048, 2048, 4, 8192
NCORES = 8
KH = H // 128            # 16 feature tiles of H
KF = FF // 128           # 64 feature tiles of FF
S = T // 2               # tokens per core
SC = 512                 # token chunk for bf16 matmuls
NSC = S // SC
DC = 256                 # token chunk for fp8 DoubleRow matmuls
NDC = SC // DC

VG_FP8 = True            # val/gate matmuls in fp8e4 DoubleRow
S_KK = 2.0               # fp8 scale for kk = relu(.)^2  (max kk ~27 << 120)

E4NP = ml_dtypes.float8_e4m3
BFNP = ml_dtypes.bfloat16


# ---------------------------------------------------------------------------
# device kernel
# ---------------------------------------------------------------------------

def build_bass():
    import concourse.bass as bass
    from concourse import bacc
    import concourse.mybir as mybir
    import concourse.tile as tile
    from concourse.masks import make_identity

    f32 = mybir.dt.float32
    f32r = mybir.dt.float32r
    bf16 = mybir.dt.bfloat16
    fp8 = mybir.dt.float8e4
    Alu = mybir.AluOpType
    Act = mybir.ActivationFunctionType
    DR = mybir.MatmulPerfMode.DoubleRow

    inv_h = 1.0 / H
    vg_dt = fp8 if VG_FP8 else bf16

    nc = bacc.Bacc()

    # --- external I/O (per core) ---
    x_d = nc.dram_tensor("xbf", [S, H], bf16, kind="ExternalInput")
    sh_d = nc.dram_tensor("shift_in", [128, KH], bf16, kind="ExternalInput")
    asd_d = nc.dram_tensor("asd", [D, H], f32, kind="ExternalInput")
    lvlw_d = nc.dram_tensor("lvl_w", [128, KH, D], bf16, kind="ExternalInput")
    lvlb_d = nc.dram_tensor("lvl_b", [D], f32, kind="ExternalInput")
    tmk_d = nc.dram_tensor("tmk", [128, KH], f32, kind="ExternalInput")
    bv_d = nc.dram_tensor("bv", [128, KH], f32, kind="ExternalInput")
    bk_d = nc.dram_tensor("bk", [128, KH], f32, kind="ExternalInput")
    br_d = nc.dram_tensor("br", [128, KH], f32, kind="ExternalInput")
    bkey_d = nc.dram_tensor("bkey", [128, KF], f32, kind="ExternalInput")
    wv_d = nc.dram_tensor("Wv", [KH, 128, KH, 128], bf16, kind="ExternalInput")
    wk_d = nc.dram_tensor("Wk", [KH, 128, KH, 128], bf16, kind="ExternalInput")
    wr_d = nc.dram_tensor("Wr", [KH, 128, KH, 128], bf16, kind="ExternalInput")
    wo_d = nc.dram_tensor("Wo", [KH, 128, KH, 128], bf16, kind="ExternalInput")
    wkey_d = nc.dram_tensor("Wkey", [KF, 128, KH, 128], bf16,
                            kind="ExternalInput")
    wval_d = nc.dram_tensor("Wval", [KH, 128, KF, 128], vg_dt,
                            kind="ExternalInput")
    wgate_d = nc.dram_tensor("Wgate", [KH, 128, KF, 128], vg_dt,
                             kind="ExternalInput")
    scl_d = nc.dram_tensor("scl", [128, 2], f32, kind="ExternalInput")
    out_d = nc.dram_tensor("out", [S, H], f32, kind="ExternalOutput")

    vec = nc.vector
    act = nc.scalar
    sy = nc.sync

    def sc_sl(sc):
        return slice(sc * SC, (sc + 1) * SC)

    with tile.TileContext(nc) as tc, \
            nc.allow_low_precision(reason="bf16/fp8 matmuls within rel-err budget"):
        # ---- persistent constants ----
        consts = tc.alloc_tile_pool(name="consts", bufs=1)
        ident = consts.tile([128, 128], f32)
        make_identity(nc, ident)
        ident_bf = consts.tile([128, 128], bf16)
        vec.tensor_copy(out=ident_bf[:, :], in_=ident[:, :])
        ones_f = consts.tile([128, 1], f32)
        vec.memset(ones_f[:, :], 1.0)
        ones = consts.tile([128, 1], f32r)
        vec.tensor_copy(out=ones[:, :], in_=ones_f[:, :])
        ones_bf = consts.tile([128, 1], bf16)
        vec.tensor_copy(out=ones_bf[:, :], in_=ones_f[:, :])
        ones_row_f = consts.tile([1, 128], f32)
        vec.memset(ones_row_f[:, :], 1.0)
        ones_row = consts.tile([1, 128], f32r)
        vec.tensor_copy(out=ones_row[:, :], in_=ones_row_f[:, :])
        eps_t = consts.tile([1, 1], f32)
        vec.memset(eps_t[:, :], 1e-5)
        tmk_t = consts.tile([128, KH], f32)
        sy.dma_start(out=tmk_t[:, :], in_=tmk_d[:, :])
        bv_t = consts.tile([128, KH], f32)
        sy.dma_start(out=bv_t[:, :], in_=bv_d[:, :])
        bk_t = consts.tile([128, KH], f32)
        sy.dma_start(out=bk_t[:, :], in_=bk_d[:, :])
        br_t = consts.tile([128, KH], f32)
        sy.dma_start(out=br_t[:, :], in_=br_d[:, :])
        bkey_t = consts.tile([128, KF], f32)
        sy.dma_start(out=bkey_t[:, :], in_=bkey_d[:, :])
        sh_t = consts.tile([128, KH], bf16)
        sy.dma_start(out=sh_t[:, :], in_=sh_d[:, :])
        scl_t = consts.tile([128, 2], f32)
        sy.dma_start(out=scl_t[:, :], in_=scl_d[:, :])

        # ---- attention-scoped constants ----
        attc = tc.alloc_tile_pool(name="attc", bufs=1, side="right")
        lvlw_t = attc.tile([128, KH, D], bf16)
        sy.dma_start(out=lvlw_t[:, :, :], in_=lvlw_d[:, :, :])
        lvlb_t = attc.tile([D, 1], f32)
        sy.dma_start(out=lvlb_t[:, :], in_=lvlb_d[:])
        asd_t = attc.tile([D, H], f32r)   # att_state * decay (host)
        sy.dma_start(out=asd_t[:, :], in_=asd_d[:, :].bitcast(f32r))
        e_t = attc.tile([D, S], f32r)     # softmax-normalized level weights
        zr_t = attc.tile([1, S], f32r)

        # ---- shared PSUM pool ----
        psum = tc.alloc_tile_pool(name="psum", bufs=1, space="PSUM")

        def mm_tile(p0=128, w=SC):
            return psum.tile([p0, w], f32, tag="mm", bufs=6, name="pt")

        def trp_tile():
            return psum.tile([128, 128], f32, tag="trp", bufs=2, name="tp")

        def bc_row(row_ap, dst_slice, w=SC):
            # broadcast a [1, w] f32r row across 128 partitions (K=1 matmul)
            pb = psum.tile([128, w], f32, tag="mm", bufs=6, name="pb")
            nc.tensor.matmul(pb[:, :], ones_row[:, :], row_ap,
                             start=True, stop=True)
            vec.tensor_copy(out=dst_slice, in_=pb[:, :])

        def ln_finish(s1p, s2p, tmp_pool):
            m_row = tmp_pool.tile([1, SC], f32r, name="mrow", bufs=1)
            vec.tensor_scalar_mul(out=m_row[:, :], in0=s1p[:, :],
                                  scalar1=inv_h)
            msq = tmp_pool.tile([1, SC], f32, name="msq", bufs=1)
            vec.tensor_mul(out=msq[:, :], in0=m_row[:, :], in1=m_row[:, :])
            var = tmp_pool.tile([1, SC], f32, name="var", bufs=1)
            vec.scalar_tensor_tensor(out=var[:, :], in0=s2p[:, :],
                                     scalar=inv_h, in1=msq[:, :],
                                     op0=Alu.mult, op1=Alu.subtract)
            act.activation(out=var[:, :], in_=var[:, :], func=Act.Sqrt,
                           bias=eps_t[:, 0:1])
            rs_row = tmp_pool.tile([1, SC], f32r, name="rsrow", bufs=1)
            vec.reciprocal(out=rs_row[:, :], in_=var[:, :])
            return m_row, rs_row

        # =================================================================
        # P0: load x, transpose to feature-major; LN1 stats+apply -> hT bf16
        # =================================================================
        vtmp_pool = tc.alloc_tile_pool(name="vtmp_pool", bufs=3)
        xT_pool = tc.alloc_tile_pool(name="xT_pool", bufs=1)
        xT = xT_pool.tile([128, KH, S], bf16)
        hT_pool = tc.alloc_tile_pool(name="hT_pool", bufs=1, side="right")
        hT = hT_pool.tile([128, KH, S], bf16)
        ln1_tmp = tc.alloc_tile_pool(name="ln1_tmp", bufs=3)
        tok_pool = tc.alloc_tile_pool(name="tok_pool", bufs=2)
        NTOK = S // 128
        for tt in range(NTOK):
            xtok = tok_pool.tile([128, H], bf16, name="xtok")
            sy.dma_start(out=xtok[:, :], in_=x_d[tt * 128:(tt + 1) * 128, :])
            for k in range(KH):
                tp = psum.tile([128, 128], bf16, tag="trp", bufs=2, name="tpb")
                nc.tensor.transpose(tp[:, :], xtok[:, k * 128:(k + 1) * 128],
                                    ident_bf[:, :])
                vec.tensor_copy(out=xT[:, k, tt * 128:(tt + 1) * 128],
                                in_=tp[:, :])
        tok_pool.release()

        for sc in range(NSC):
            ssl = sc_sl(sc)
            s1p = mm_tile(1)
            s2p = mm_tile(1)
            for k in range(KH):
                sq = ln1_tmp.tile([128, SC], bf16, tag="sq", name="sq")
                act.activation(out=sq[:, :], in_=xT[:, k, ssl],
                               func=Act.Square)
                nc.tensor.matmul(s1p[:, :], ones_bf[:, :], xT[:, k, ssl],
                                 start=(k == 0), stop=(k == KH - 1))
                nc.tensor.matmul(s2p[:, :], ones_bf[:, :], sq[:, :],
                                 start=(k == 0), stop=(k == KH - 1))
            m_row, rs_row = ln_finish(s1p, s2p, ln1_tmp)
            m1b = ln1_tmp.tile([128, SC], f32, name="m1b", bufs=1)
            rs1b = ln1_tmp.tile([128, SC], f32, name="rs1b", bufs=1)
            bc_row(m_row[0:1, :], m1b[:, :])
            bc_row(rs_row[0:1, :], rs1b[:, :])
            for k in range(KH):
                t1 = ln1_tmp.tile([128, SC], f32, tag="lt", name="t1")
                nc.gpsimd.tensor_sub(out=t1[:, :], in0=xT[:, k, ssl],
                                     in1=m1b[:, :])
                vec.tensor_mul(out=hT[:, k, ssl], in0=t1[:, :],
                               in1=rs1b[:, :])
            # level weights for this chunk: e = exp(h@lvl_w + lvl_b); then
            # fold 1/z so the per-(hout,sc) weighted term is just one matmul
            lp = mm_tile(D)
            for k in range(KH):
                nc.tensor.matmul(lp[:, :], lvlw_t[:, k, :], hT[:, k, ssl],
                                 start=(k == 0), stop=(k == KH - 1))
            act.activation(out=e_t[:, ssl], in_=lp[:, :], func=Act.Exp,
                           bias=lvlb_t[:, 0:1])
            zp = mm_tile(1)
            nc.tensor.matmul(zp[:, :], ones[0:D, :], e_t[:, ssl],
                             start=True, stop=True)
            vec.reciprocal(out=zr_t[:, ssl], in_=zp[:, :])
            z4 = psum.tile([D, SC], f32, tag="mm", bufs=6, name="z4")
            nc.tensor.matmul(z4[:, :], ones_row[0:1, 0:D], zr_t[0:1, ssl],
                             start=True, stop=True)
            vec.tensor_mul(out=e_t[:, ssl], in0=e_t[:, ssl], in1=z4[:, :])
        ln1_tmp.release()

        # =================================================================
        # P2: v/k/r projections, kv, weighted level term, r gate -> kvT bf16
        # =================================================================
        kvT_pool = tc.alloc_tile_pool(name="kvT_pool", bufs=1)
        kvT = kvT_pool.tile([128, KH, S], bf16)
        wvkr_pool = tc.alloc_tile_pool(name="wvkr_pool", bufs=2)
        for sc in range(NSC):
            ssl = sc_sl(sc)
            for hout in range(KH):
                wvc = wvkr_pool.tile([128, KH, 128], bf16, tag="wv", name="wvc")
                sy.dma_start(out=wvc[:, :, :], in_=wv_d[hout])
                wkc = wvkr_pool.tile([128, KH, 128], bf16, tag="wk", name="wkc")
                sy.dma_start(out=wkc[:, :, :], in_=wk_d[hout])
                wrc = wvkr_pool.tile([128, KH, 128], bf16, tag="wr", name="wrc")
                sy.dma_start(out=wrc[:, :, :], in_=wr_d[hout])
                pv = mm_tile()
                for k in range(KH):
                    nc.tensor.matmul(pv[:, :], wvc[:, k, :], hT[:, k, ssl],
                                     start=(k == 0), stop=(k == KH - 1))
                v_t = vtmp_pool.tile([128, SC], f32, name="v_t")
                vec.tensor_scalar_add(out=v_t[:, :], in0=pv[:, :],
                                      scalar1=bv_t[:, hout:hout + 1])
                pk = mm_tile()
                for k in range(KH):
                    nc.tensor.matmul(pk[:, :], wkc[:, k, :], hT[:, k, ssl],
                                     start=(k == 0), stop=(k == KH - 1))
                # kv = (k + bk) * v
                vec.scalar_tensor_tensor(out=kvT[:, hout, ssl], in0=pk[:, :],
                                         scalar=bk_t[:, hout:hout + 1],
                                         in1=v_t[:, :],
                                         op0=Alu.add, op1=Alu.mult)
                # + level-weighted state term
                hsl = slice(hout * 128, (hout + 1) * 128)
                pw1 = mm_tile()
                nc.tensor.matmul(pw1[:, :], asd_t[:, hsl], e_t[:, ssl],
                                 start=True, stop=True)
                vec.tensor_add(out=kvT[:, hout, ssl], in0=pw1[:, :],
                               in1=kvT[:, hout, ssl])
                # * sigmoid(r)
                pr = mm_tile()
                for k in range(KH):
                    nc.tensor.matmul(pr[:, :], wrc[:, k, :], hT[:, k, ssl],
                                     start=(k == 0), stop=(k == KH - 1))
                r_t = vtmp_pool.tile([128, SC], f32, name="r_t")
                act.activation(out=r_t[:, :], in_=pr[:, :], func=Act.Sigmoid,
                               bias=br_t[:, hout:hout + 1])
                vec.tensor_mul(out=kvT[:, hout, ssl], in0=r_t[:, :],
                               in1=kvT[:, hout, ssl])
        hT_pool.release()
        attc.release()
        wvkr_pool.release()

        # =================================================================
        # P3: att = rw @ Wo; x1 = x + att; LN2 stats (interleaved)
        # P4: LN2 apply + token shift + time-mix -> h2s bf16 (pipelined)
        # =================================================================
        # right stack: x1 (to end) under h2s (to P5 end) under ln2 (to P3 end)
        x1_pool = tc.alloc_tile_pool(name="x1_pool", bufs=1, side="right")
        x1T = x1_pool.tile([128, KH, S], f32r)
        h2_pool = tc.alloc_tile_pool(name="h2_pool", bufs=1, side="right")
        h2s = h2_pool.tile([128, KH, S + 1], bf16)
        ln2_tmp = tc.alloc_tile_pool(name="ln2_tmp", bufs=2, side="right")
        wo_pool = tc.alloc_tile_pool(name="wo_pool", bufs=2)
        # seed the token shift: h2s[:, k, 0] = shift row
        for k in range(KH):
            vec.tensor_copy(out=h2s[:, k, 0:1], in_=sh_t[:, k:k + 1])

        for sc in range(NSC):
            ssl = sc_sl(sc)
            s1p = mm_tile(1)
            s2p = mm_tile(1)
            for hout in range(KH):
                woc = wo_pool.tile([128, KH, 128], bf16, tag="wo", name="woc")
                sy.dma_start(out=woc[:, :, :], in_=wo_d[hout])
                pa = mm_tile()
                for k in range(KH):
                    nc.tensor.matmul(pa[:, :], woc[:, k, :], kvT[:, k, ssl],
                                     start=(k == 0), stop=(k == KH - 1))
                vec.tensor_add(out=x1T[:, hout, ssl], in0=pa[:, :],
                               in1=xT[:, hout, ssl])
                # LN2 stats accumulate as x1 tiles appear (k == hout)
                sq = ln2_tmp.tile([128, SC], bf16, tag="sq", name="sq")
                act.activation(out=sq[:, :], in_=x1T[:, hout, ssl],
                               func=Act.Square)
                nc.tensor.matmul(s1p[:, :], ones[:, :], x1T[:, hout, ssl],
                                 start=(hout == 0), stop=(hout == KH - 1))
                nc.tensor.matmul(s2p[:, :], ones_bf[:, :], sq[:, :],
                                 start=(hout == 0), stop=(hout == KH - 1))
            m_row, rs_row = ln_finish(s1p, s2p, ln2_tmp)
            m2b = ln2_tmp.tile([128, SC], f32, name="m2b", bufs=1)
            rs2b = ln2_tmp.tile([128, SC], f32, name="rs2b", bufs=1)
            bc_row(m_row[0:1, :], m2b[:, :])
            bc_row(rs_row[0:1, :], rs2b[:, :])
            # P4 for this chunk (vector-only; overlaps next chunk's matmuls)
            for k in range(KH):
                t1 = ln2_tmp.tile([128, SC], f32, tag="lt", name="t1")
                nc.gpsimd.tensor_sub(out=t1[:, :], in0=x1T[:, k, ssl],
                                     in1=m2b[:, :])
                vec.tensor_mul(out=h2s[:, k, 1 + sc * SC:1 + (sc + 1) * SC],
                               in0=t1[:, :], in1=rs2b[:, :])
                d_t = ln2_tmp.tile([128, SC], bf16, tag="dt", name="d_t")
                vec.tensor_sub(out=d_t[:, :],
                               in0=h2s[:, k, 1 + sc * SC:1 + (sc + 1) * SC],
                               in1=h2s[:, k, sc * SC:(sc + 1) * SC])
                vec.scalar_tensor_tensor(
                    out=h2s[:, k, sc * SC:(sc + 1) * SC], in0=d_t[:, :],
                    scalar=tmk_t[:, k:k + 1],
                    in1=h2s[:, k, sc * SC:(sc + 1) * SC],
                    op0=Alu.mult, op1=Alu.add)
        wo_pool.release()
        kvT_pool.release()
        xT_pool.release()
        ln2_tmp.release()

        # =================================================================
        # P5: kk = relu(sqrt(s_kk)*(km @ Wkey' + bkey))^2 -> fp8 (SBUF)
        # P6: out = x1 + (kk@Wval)*sigmoid(kk@Wgate); transpose; store
        # (interleaved per sc chunk so kk stays at [128, KF, SC])
        # =================================================================
        kk_pool = tc.alloc_tile_pool(name="kk_pool", bufs=1)
        kkw_pool = tc.alloc_tile_pool(name="kkw_pool", bufs=3)
        wvg_pool = tc.alloc_tile_pool(name="wvg_pool", bufs=2)
        fin_pool = tc.alloc_tile_pool(name="fin_pool", bufs=4)
        ot_pool = tc.alloc_tile_pool(name="ot_pool", bufs=4)
        sqrt_skk = float(np.sqrt(S_KK)) if VG_FP8 else 1.0
        for sc in range(NSC):
            ssl = sc_sl(sc)
            kk = kk_pool.tile([128, KF, SC], vg_dt, tag="kk", name="kk")
            for ff in range(KF):
                wyc = kkw_pool.tile([128, KH, 128], bf16, name="wyc")
                sy.dma_start(out=wyc[:, :, :], in_=wkey_d[ff])
                pkk = mm_tile()
                for k in range(KH):
                    nc.tensor.matmul(pkk[:, :], wyc[:, k, :],
                                     h2s[:, k, sc * SC:(sc + 1) * SC],
                                     start=(k == 0), stop=(k == KH - 1))
                u_t = vtmp_pool.tile([128, SC], bf16, name="u_t")
                act.activation(out=u_t[:, :], in_=pkk[:, :], func=Act.Relu,
                               bias=bkey_t[:, ff:ff + 1], scale=sqrt_skk)
                vec.tensor_mul(out=kk[:, ff, :], in0=u_t[:, :],
                               in1=u_t[:, :])
            for hout in range(KH):
                wv8 = wvg_pool.tile([128, KF, 128], vg_dt, tag="wv8",
                                    name="wv8")
                sy.dma_start(out=wv8[:, :, :], in_=wval_d[hout])
                wg8 = wvg_pool.tile([128, KF, 128], vg_dt, tag="wg8",
                                    name="wg8")
                sy.dma_start(out=wg8[:, :, :], in_=wgate_d[hout])
                for dc in range(NDC):
                    dsl = slice(dc * DC, (dc + 1) * DC)
                    xsl = slice(sc * SC + dc * DC, sc * SC + (dc + 1) * DC)
                    psv = psum.tile([128, DC], f32, tag="mm", bufs=6,
                                    name="psv")
                    psg = psum.tile([128, DC], f32, tag="mm", bufs=6,
                                    name="psg")
                    if VG_FP8:
                        for f in range(KF // 2):
                            nc.tensor.matmul(psv[:, :],
                                             wv8[:, 2 * f:2 * f + 2, :],
                                             kk[:, 2 * f:2 * f + 2, dsl],
                                             start=(f == 0),
                                             stop=(f == KF // 2 - 1),
                                             perf_mode=DR)
                        for f in range(KF // 2):
                            nc.tensor.matmul(psg[:, :],
                                             wg8[:, 2 * f:2 * f + 2, :],
                                             kk[:, 2 * f:2 * f + 2, dsl],
                                             start=(f == 0),
                                             stop=(f == KF // 2 - 1),
                                             perf_mode=DR)
                    else:
                        for f in range(KF):
                            nc.tensor.matmul(psv[:, :], wv8[:, f, :],
                                             kk[:, f, dsl],
                                             start=(f == 0),
                                             stop=(f == KF - 1))
                        for f in range(KF):
                            nc.tensor.matmul(psg[:, :], wg8[:, f, :],
                                             kk[:, f, dsl],
                                             start=(f == 0),
                                             stop=(f == KF - 1))
                    sig_t = fin_pool.tile([128, DC], f32, name="sig_t")
                    act.activation(out=sig_t[:, :], in_=psg[:, :],
                                   func=Act.Sigmoid, scale=scl_t[:, 1:2])
                    glu_t = fin_pool.tile([128, DC], f32, name="glu_t")
                    vec.tensor_scalar_mul(out=glu_t[:, :], in0=psv[:, :],
                                          scalar1=scl_t[:, 0:1])
                    vec.tensor_mul(out=glu_t[:, :], in0=glu_t[:, :],
                                   in1=sig_t[:, :])
                    vec.tensor_add(out=glu_t[:, :], in0=glu_t[:, :],
                                   in1=x1T[:, hout, xsl])
                    for j in range(DC // 128):
                        tp = trp_tile()
                        nc.tensor.transpose(tp[:, :],
                                            glu_t[:, j * 128:(j + 1) * 128],
                                            ident[:, :])
                        ot = ot_pool.tile([128, 128], f32, name="ot")
                        vec.tensor_copy(out=ot[:, :], in_=tp[:, :])
                        tt = (sc * SC + dc * DC) // 128 + j
                        sy.dma_start(
                            out=out_d[tt * 128:(tt + 1) * 128,
                                      hout * 128:(hout + 1) * 128],
                            in_=ot[:, :])
        ot_pool.release()
        fin_pool.release()
        wvg_pool.release()
        kkw_pool.release()
        kk_pool.release()
        h2_pool.release()
        x1_pool.release()
        vtmp_pool.release()
        consts.release()
        psum.release()
    nc.finalize()
    return nc


# ---------------------------------------------------------------------------
# host side
# ---------------------------------------------------------------------------

def _ln_np(x, s, b):
    m = x.mean(-1, keepdims=True)
    vv = ((x - m) ** 2).mean(-1, keepdims=True)
    return (x - m) / np.sqrt(vv + 1e-5) * s + b


def _h2hat_row(xrow, att_state_b, ln1_s, ln1_b, ln2_s, ln2_b, td, lvl_w,
               lvl_b, Wv, Wk, Wr, Wo):
    """(x1 - m)/std for a single token row (LN2 without scale/bias)."""
    h = _ln_np(xrow[None, :], ln1_s, ln1_b)[0]
    vv = h @ Wv
    kk = h @ Wk
    rr = 1.0 / (1.0 + np.exp(-(h @ Wr)))
    lg = h @ lvl_w + lvl_b
    e = np.exp(lg - lg.max())
    lw = e / e.sum()
    decay = np.exp(-np.exp(td))
    weighted = (lw[None, :] @ (att_state_b * decay))[0] + kk * vv
    att = (rr * weighted) @ Wo
    x1 = xrow + att
    m = x1.mean()
    sd = np.sqrt(((x1 - m) ** 2).mean() + 1e-5)
    return ((x1 - m) / sd).astype(np.float32)


def _arrange_hkh(W):
    """[H, H] -> [KH, 128, KH, 128]: arr[ho, p, k, m] = W[k*128+p, ho*128+m]"""
    Wr = W.reshape(KH, 128, -1, 128)            # [k, p, ho, m]
    return np.ascontiguousarray(Wr.transpose(2, 1, 0, 3))


def _arrange_cols(v):
    """[H] -> [128, KH]: arr[p, k] = v[k*128+p]"""
    return np.ascontiguousarray(v.reshape(-1, 128).T)


_BUILT = None


def _get_built():
    global _BUILT
    if _BUILT is None:
        _BUILT = build_bass()
    return _BUILT


def make_in_maps(x, att_state, cm_state, ln1_s, ln1_b, ln2_s, ln2_b,
                 td_multi, lvl_w, lvl_b, Wv, Wk, Wr, Wo, tmk,
                 Wkey, Wval, Wgate):
    f = np.float32
    x = np.asarray(x, f)
    att_state = np.asarray(att_state, f)
    cm_state = np.asarray(cm_state, f)
    ln1_s, ln1_b = np.asarray(ln1_s, f), np.asarray(ln1_b, f)
    ln2_s, ln2_b = np.asarray(ln2_s, f), np.asarray(ln2_b, f)
    td = np.asarray(td_multi, f)
    lvl_w, lvl_b = np.asarray(lvl_w, f), np.asarray(lvl_b, f)
    Wv, Wk, Wr, Wo = (np.asarray(a, f) for a in (Wv, Wk, Wr, Wo))
    tmk = np.asarray(tmk, f)
    Wkey, Wval, Wgate = (np.asarray(a, f) for a in (Wkey, Wval, Wgate))

    # fold LN1 scale into Wv/Wk/Wr/lvl_w; LN1 bias becomes output biases
    decay = np.exp(-np.exp(td))
    sqrt_skk = np.sqrt(S_KK) if VG_FP8 else 1.0
    if VG_FP8:
        s_wv = 224.0 / max(np.abs(Wval).max(), 1e-9)
        s_wg = 224.0 / max(np.abs(Wgate).max(), 1e-9)
        wval_a = np.ascontiguousarray(
            _arrange_khf(np.clip(Wval * s_wv, -240, 240)).astype(E4NP))
        wgate_a = np.ascontiguousarray(
            _arrange_khf(np.clip(Wgate * s_wg, -240, 240)).astype(E4NP))
        scl = np.tile(np.array([1.0 / (S_KK * s_wv),
                                1.0 / (S_KK * s_wg)], f), (128, 1))
    else:
        wval_a = np.ascontiguousarray(_arrange_khf(Wval).astype(BFNP))
        wgate_a = np.ascontiguousarray(_arrange_khf(Wgate).astype(BFNP))
        scl = np.tile(np.array([1.0, 1.0], f), (128, 1))

    shared = {
        "lvl_w": np.ascontiguousarray(
            (ln1_s[:, None] * lvl_w).reshape(KH, 128, D)
            .transpose(1, 0, 2)).astype(BFNP),
        "lvl_b": lvl_b + ln1_b @ lvl_w,
        "tmk": _arrange_cols(tmk),
        "bv": _arrange_cols(ln1_b @ Wv),
        "bk": _arrange_cols(ln1_b @ Wk),
        "br": _arrange_cols(ln1_b @ Wr),
        "bkey": np.ascontiguousarray(
            ((ln2_b @ Wkey) * sqrt_skk).reshape(KF, 128).T),
        "Wv": _arrange_hkh(ln1_s[:, None] * Wv).astype(BFNP),
        "Wk": _arrange_hkh(ln1_s[:, None] * Wk).astype(BFNP),
        "Wr": _arrange_hkh(ln1_s[:, None] * Wr).astype(BFNP),
        "Wo": _arrange_hkh(Wo).astype(BFNP),
        "Wkey": _arrange_khf_key(ln2_s[:, None] * Wkey).astype(BFNP),
        "Wval": wval_a,
        "Wgate": wgate_a,
        "scl": scl,
    }
    shared = {k: np.ascontiguousarray(v) for k, v in shared.items()}

    in_maps = []
    for c in range(NCORES):
        b, piece = c // 2, c % 2
        t0 = piece * S
        if piece == 0:
            shift = (cm_state[b] - ln2_b) / ln2_s
        else:
            shift = _h2hat_row(x[b, t0 - 1], att_state[b], ln1_s, ln1_b,
                               ln2_s, ln2_b, td, lvl_w, lvl_b, Wv, Wk, Wr, Wo)
        in_maps.append({
            "xbf": np.ascontiguousarray(x[b, t0:t0 + S].astype(BFNP)),
            "shift_in": np.ascontiguousarray(
                shift.reshape(KH, 128).T.astype(BFNP)),
            "asd": np.ascontiguousarray(att_state[b] * decay, f),
            **shared,
        })
    return in_maps


def _arrange_khf(W):
    """[FF, H] -> [KH, 128, KF, 128]: arr[ho, p, f, m] = W[f*128+p, ho*128+m]"""
    Wr = W.reshape(KF, 128, KH, 128)            # [f, p, ho, m]
    return np.ascontiguousarray(Wr.transpose(2, 1, 0, 3))


def _arrange_khf_key(W):
    """[H, FF] -> [KF, 128, KH, 128]: arr[fo, p, k, m] = W[k*128+p, fo*128+m]"""
    Wr = W.reshape(KH, 128, KF, 128)            # [k, p, fo, m]
    return np.ascontiguousarray(Wr.transpose(2, 1, 0, 3))


def kernel(x, att_state, cm_state, ln1_s, ln1_b, ln2_s, ln2_b,
           td_multi, lvl_w, lvl_b, Wv, Wk, Wr, Wo, tmk,
           Wkey, Wval, Wgate):
    from concourse.bass_utils import run_bass_kernel_spmd

    in_maps = make_in_maps(x, att_state, cm_state, ln1_s, ln1_b, ln2_s,
                           ln2_b, td_multi, lvl_w, lvl_b, Wv, Wk, Wr, Wo,
                           tmk, Wkey, Wval, Wgate)
    nc = _get_built()
    res = run_bass_kernel_spmd(nc, in_maps, list(range(NCORES)))
    out = np.empty((B, T, H), np.float32)
    for c in range(NCORES):
        b, piece = c // 2, c % 2
        out[b, piece * S:(piece + 1) * S] = res.results[c]["out"]
    return out


# revision 14
# speedup vs baseline: 1.0308x; 1.0308x over previous
"""EnhancedRWKVBlock Trainium2 kernel (optimized).

Sharding: 8 cores = 4 batches x 2 sequence halves (pure data parallel).
The channel-mix token shift across the halves is seeded by one host-computed
row per odd shard.

Key optimizations over the f32r baseline:
  - All big matmuls run in bf16 (same PE rate as f32r, half the LDWEIGHTS
    time, half the weight DMA); the val/gate matmuls run in fp8e4 DoubleRow
    (2x PE rate) with host-quantized weights.
  - Weights are pre-arranged on host into [tile, 128, k, 128] layouts so
    every weight DMA is contiguous per partition (the baseline was DMA
    descriptor-bound with 2048 x 256B scatters per tile).
  - LN scale/bias are folded into the following projection weights on host;
    the level-mix 1/z is folded into e_t once instead of per (hout, sc).
  - No DRAM spills: xT/x1T/kk stay SBUF-resident.
  - sc-major phase ordering pipelines the vector-only LN2/token-shift block
    behind matmul phases, removing the PE bubble.
"""

import numpy as np
import ml_dtypes

B, T, H, D, FF = 4, 2048, 2048, 4, 8192
NCORES = 8
KH = H // 128            # 16 feature tiles of H
KF = FF // 128           # 64 feature tiles of FF
S = T // 2               # tokens per core
SC = 512                 # token chunk for bf16 matmuls
NSC = S // SC
DC = 256                 # token chunk for fp8 DoubleRow matmuls
NDC = SC // DC

VG_FP8 = True            # val/gate matmuls in fp8e4 DoubleRow
S_KK = 2.0               # fp8 scale for kk = relu(.)^2  (max kk ~27 << 120)

E4NP = ml_dtypes.float8_e4m3
BFNP = ml_dtypes.bfloat16


# ---------------------------------------------------------------------------
# device kernel
# ---------------------------------------------------------------------------

def build_bass():
    import concourse.bass as bass
    from concourse import bacc
    import concourse.mybir as mybir
    import concourse.tile as tile
    from concourse.masks import make_identity

    f32 = mybir.dt.float32
    f32r = mybir.dt.float32r
    bf16 = mybir.dt.bfloat16
    fp8 = mybir.dt.float8e4
    Alu = mybir.AluOpType
    Act = mybir.ActivationFunctionType
    DR = mybir.MatmulPerfMode.DoubleRow

    inv_h = 1.0 / H
    vg_dt = fp8 if VG_FP8 else bf16

    nc = bacc.Bacc()

    # --- external I/O (per core) ---
    x_d = nc.dram_tensor("xbf", [S, H], bf16, kind="ExternalInput")
    sh_d = nc.dram_tensor("shift_in", [128, KH], bf16, kind="ExternalInput")
    asd_d = nc.dram_tensor("asd", [D, H], f32, kind="ExternalInput")
    lvlw_d = nc.dram_tensor("lvl_w", [128, KH, D], bf16, kind="ExternalInput")
    lvlb_d = nc.dram_tensor("lvl_b", [D], f32, kind="ExternalInput")
    tmk_d = nc.dram_tensor("tmk", [128, KH], f32, kind="ExternalInput")
    bv_d = nc.dram_tensor("bv", [128, KH], f32, kind="ExternalInput")
    bk_d = nc.dram_tensor("bk", [128, KH], f32, kind="ExternalInput")
    br_d = nc.dram_tensor("br", [128, KH], f32, kind="ExternalInput")
    bkey_d = nc.dram_tensor("bkey", [128, KF], f32, kind="ExternalInput")
    wv_d = nc.dram_tensor("Wv", [KH, 128, KH, 128], bf16, kind="ExternalInput")
    wk_d = nc.dram_tensor("Wk", [KH, 128, KH, 128], bf16, kind="ExternalInput")
    wr_d = nc.dram_tensor("Wr", [KH, 128, KH, 128], bf16, kind="ExternalInput")
    wo_d = nc.dram_tensor("Wo", [KH, 128, KH, 128], bf16, kind="ExternalInput")
    wkey_d = nc.dram_tensor("Wkey", [KF, 128, KH, 128], bf16,
                            kind="ExternalInput")
    wval_d = nc.dram_tensor("Wval", [KH, 128, KF, 128], vg_dt,
                            kind="ExternalInput")
    wgate_d = nc.dram_tensor("Wgate", [KH, 128, KF, 128], vg_dt,
                             kind="ExternalInput")
    scl_d = nc.dram_tensor("scl", [128, 2], f32, kind="ExternalInput")
    out_d = nc.dram_tensor("out", [S, H], f32, kind="ExternalOutput")

    vec = nc.vector
    act = nc.scalar
    sy = nc.sync

    def sc_sl(sc):
        return slice(sc * SC, (sc + 1) * SC)

    with tile.TileContext(nc) as tc, \
            nc.allow_low_precision(reason="bf16/fp8 matmuls within rel-err budget"):
        # ---- persistent constants ----
        consts = tc.alloc_tile_pool(name="consts", bufs=1)
        ident = consts.tile([128, 128], f32)
        make_identity(nc, ident)
        ident_bf = consts.tile([128, 128], bf16)
        vec.tensor_copy(out=ident_bf[:, :], in_=ident[:, :])
        ones_f = consts.tile([128, 1], f32)
        vec.memset(ones_f[:, :], 1.0)
        ones = consts.tile([128, 1], f32r)
        vec.tensor_copy(out=ones[:, :], in_=ones_f[:, :])
        ones_bf = consts.tile([128, 1], bf16)
        vec.tensor_copy(out=ones_bf[:, :], in_=ones_f[:, :])
        ones_row_f = consts.tile([1, 128], f32)
        vec.memset(ones_row_f[:, :], 1.0)
        ones_row = consts.tile([1, 128], f32r)
        vec.tensor_copy(out=ones_row[:, :], in_=ones_row_f[:, :])
        eps_t = consts.tile([1, 1], f32)
        vec.memset(eps_t[:, :], 1e-5)
        tmk_t = consts.tile([128, KH], f32)
        sy.dma_start(out=tmk_t[:, :], in_=tmk_d[:, :])
        bv_t = consts.tile([128, KH], f32)
        sy.dma_start(out=bv_t[:, :], in_=bv_d[:, :])
        bk_t = consts.tile([128, KH], f32)
        sy.dma_start(out=bk_t[:, :], in_=bk_d[:, :])
        br_t = consts.tile([128, KH], f32)
        sy.dma_start(out=br_t[:, :], in_=br_d[:, :])
        bkey_t = consts.tile([128, KF], f32)
        sy.dma_start(out=bkey_t[:, :], in_=bkey_d[:, :])
        sh_t = consts.tile([128, KH], bf16)
        sy.dma_start(out=sh_t[:, :], in_=sh_d[:, :])
        scl_t = consts.tile([128, 2], f32)
        sy.dma_start(out=scl_t[:, :], in_=scl_d[:, :])

        # ---- attention-scoped constants ----
        attc = tc.alloc_tile_pool(name="attc", bufs=1, side="right")
        lvlw_t = attc.tile([128, KH, D], bf16)
        sy.dma_start(out=lvlw_t[:, :, :], in_=lvlw_d[:, :, :])
        lvlb_t = attc.tile([D, 1], f32)
        sy.dma_start(out=lvlb_t[:, :], in_=lvlb_d[:])
        asd_t = attc.tile([D, H], f32r)   # att_state * decay (host)
        sy.dma_start(out=asd_t[:, :], in_=asd_d[:, :].bitcast(f32r))
        e_t = attc.tile([D, S], f32r)     # softmax-normalized level weights
        zr_t = attc.tile([1, S], f32r)

        # ---- shared PSUM pool ----
        psum = tc.alloc_tile_pool(name="psum", bufs=1, space="PSUM")

        def mm_tile(p0=128, w=SC):
            return psum.tile([p0, w], f32, tag="mm", bufs=6, name="pt")

        def trp_tile():
            return psum.tile([128, 128], f32, tag="trp", bufs=2, name="tp")

        def bc_row(row_ap, dst_slice, w=SC):
            # broadcast a [1, w] f32r row across 128 partitions (K=1 matmul)
            pb = psum.tile([128, w], f32, tag="mm", bufs=6, name="pb")
            nc.tensor.matmul(pb[:, :], ones_row[:, :], row_ap,
                             start=True, stop=True)
            vec.tensor_copy(out=dst_slice, in_=pb[:, :])

        def ln_finish(s1p, s2p, tmp_pool):
            m_row = tmp_pool.tile([1, SC], f32r, name="mrow", bufs=1)
            vec.tensor_scalar_mul(out=m_row[:, :], in0=s1p[:, :],
                                  scalar1=inv_h)
            msq = tmp_pool.tile([1, SC], f32, name="msq", bufs=1)
            vec.tensor_mul(out=msq[:, :], in0=m_row[:, :], in1=m_row[:, :])
            var = tmp_pool.tile([1, SC], f32, name="var", bufs=1)
            vec.scalar_tensor_tensor(out=var[:, :], in0=s2p[:, :],
                                     scalar=inv_h, in1=msq[:, :],
                                     op0=Alu.mult, op1=Alu.subtract)
            act.activation(out=var[:, :], in_=var[:, :], func=Act.Sqrt,
                           bias=eps_t[:, 0:1])
            rs_row = tmp_pool.tile([1, SC], f32r, name="rsrow", bufs=1)
            vec.reciprocal(out=rs_row[:, :], in_=var[:, :])
            return m_row, rs_row

        # =================================================================
        # P0: load x, transpose to feature-major; LN1 stats+apply -> hT bf16
        # =================================================================
        vtmp_pool = tc.alloc_tile_pool(name="vtmp_pool", bufs=3)
        xT_pool = tc.alloc_tile_pool(name="xT_pool", bufs=1)
        xT = xT_pool.tile([128, KH, S], bf16)
        hT_pool = tc.alloc_tile_pool(name="hT_pool", bufs=1, side="right")
        hT = hT_pool.tile([128, KH, S], bf16)
        ln1_tmp = tc.alloc_tile_pool(name="ln1_tmp", bufs=3)
        tok_pool = tc.alloc_tile_pool(name="tok_pool", bufs=2)
        NTOK = S // 128
        for tt in range(NTOK):
            xtok = tok_pool.tile([128, H], bf16, name="xtok")
            sy.dma_start(out=xtok[:, :], in_=x_d[tt * 128:(tt + 1) * 128, :])
            for k in range(KH):
                tp = psum.tile([128, 128], bf16, tag="trp", bufs=2, name="tpb")
                nc.tensor.transpose(tp[:, :], xtok[:, k * 128:(k + 1) * 128],
                                    ident_bf[:, :])
                vec.tensor_copy(out=xT[:, k, tt * 128:(tt + 1) * 128],
                                in_=tp[:, :])
        tok_pool.release()

        fins = []
        for sc in range(NSC):
            ssl = sc_sl(sc)
            s1p = mm_tile(1)
            s2p = mm_tile(1)
            for k in range(KH):
                sq = ln1_tmp.tile([128, SC], bf16, tag="sq", name="sq")
                act.activation(out=sq[:, :], in_=xT[:, k, ssl],
                               func=Act.Square)
                nc.tensor.matmul(s1p[:, :], ones_bf[:, :], xT[:, k, ssl],
                                 start=(k == 0), stop=(k == KH - 1))
                nc.tensor.matmul(s2p[:, :], ones_bf[:, :], sq[:, :],
                                 start=(k == 0), stop=(k == KH - 1))
            fins.append((s1p, s2p))
        rows = [ln_finish(*fins[0], ln1_tmp)]
        for sc in range(NSC):
            ssl = sc_sl(sc)
            m_row, rs_row = rows[sc]
            m1b = ln1_tmp.tile([128, SC], f32, name="m1b", bufs=1)
            rs1b = ln1_tmp.tile([128, SC], f32, name="rs1b", bufs=1)
            bc_row(m_row[0:1, :], m1b[:, :])
            bc_row(rs_row[0:1, :], rs1b[:, :])
            if sc + 1 < NSC:
                rows.append(ln_finish(*fins[sc + 1], ln1_tmp))
            for k in range(KH):
                t1 = ln1_tmp.tile([128, SC], f32, tag="lt", name="t1")
                vec.tensor_sub(out=t1[:, :], in0=xT[:, k, ssl],
                               in1=m1b[:, :])
                vec.tensor_mul(out=hT[:, k, ssl], in0=t1[:, :],
                               in1=rs1b[:, :])
            # level weights: e = exp(h@lvl_w + lvl_b) with 1/z folded in
            lp = mm_tile(D)
            for k in range(KH):
                nc.tensor.matmul(lp[:, :], lvlw_t[:, k, :], hT[:, k, ssl],
                                 start=(k == 0), stop=(k == KH - 1))
            act.activation(out=e_t[:, ssl], in_=lp[:, :], func=Act.Exp,
                           bias=lvlb_t[:, 0:1])
            zp = mm_tile(1)
            nc.tensor.matmul(zp[:, :], ones[0:D, :], e_t[:, ssl],
                             start=True, stop=True)
            vec.reciprocal(out=zr_t[:, ssl], in_=zp[:, :])
            z4 = psum.tile([D, SC], f32, tag="mm", bufs=6, name="z4")
            nc.tensor.matmul(z4[:, :], ones_row[0:1, 0:D], zr_t[0:1, ssl],
                             start=True, stop=True)
            vec.tensor_mul(out=e_t[:, ssl], in0=e_t[:, ssl], in1=z4[:, :])
        ln1_tmp.release()

        # =================================================================
        # P2: v/k/r projections, kv, weighted level term, r gate -> kvT bf16
        # =================================================================
        kvT_pool = tc.alloc_tile_pool(name="kvT_pool", bufs=1)
        kvT = kvT_pool.tile([128, KH, S], bf16)
        wvkr_pool = tc.alloc_tile_pool(name="wvkr_pool", bufs=2)
        for sc in range(NSC):
            ssl = sc_sl(sc)
            for hout in range(KH):
                wvc = wvkr_pool.tile([128, KH, 128], bf16, tag="wv", name="wvc")
                sy.dma_start(out=wvc[:, :, :], in_=wv_d[hout])
                wkc = wvkr_pool.tile([128, KH, 128], bf16, tag="wk", name="wkc")
                sy.dma_start(out=wkc[:, :, :], in_=wk_d[hout])
                wrc = wvkr_pool.tile([128, KH, 128], bf16, tag="wr", name="wrc")
                sy.dma_start(out=wrc[:, :, :], in_=wr_d[hout])
                pv = mm_tile()
                for k in range(KH):
                    nc.tensor.matmul(pv[:, :], wvc[:, k, :], hT[:, k, ssl],
                                     start=(k == 0), stop=(k == KH - 1))
                v_t = vtmp_pool.tile([128, SC], f32, name="v_t")
                vec.tensor_scalar_add(out=v_t[:, :], in0=pv[:, :],
                                      scalar1=bv_t[:, hout:hout + 1])
                pk = mm_tile()
                for k in range(KH):
                    nc.tensor.matmul(pk[:, :], wkc[:, k, :], hT[:, k, ssl],
                                     start=(k == 0), stop=(k == KH - 1))
                # kv = (k + bk) * v
                vec.scalar_tensor_tensor(out=kvT[:, hout, ssl], in0=pk[:, :],
                                         scalar=bk_t[:, hout:hout + 1],
                                         in1=v_t[:, :],
                                         op0=Alu.add, op1=Alu.mult)
                # + level-weighted state term
                hsl = slice(hout * 128, (hout + 1) * 128)
                pw1 = mm_tile()
                nc.tensor.matmul(pw1[:, :], asd_t[:, hsl], e_t[:, ssl],
                                 start=True, stop=True)
                vec.tensor_add(out=kvT[:, hout, ssl], in0=pw1[:, :],
                               in1=kvT[:, hout, ssl])
                # * sigmoid(r)
                pr = mm_tile()
                for k in range(KH):
                    nc.tensor.matmul(pr[:, :], wrc[:, k, :], hT[:, k, ssl],
                                     start=(k == 0), stop=(k == KH - 1))
                r_t = vtmp_pool.tile([128, SC], f32, name="r_t")
                act.activation(out=r_t[:, :], in_=pr[:, :], func=Act.Sigmoid,
                               bias=br_t[:, hout:hout + 1])
                vec.tensor_mul(out=kvT[:, hout, ssl], in0=r_t[:, :],
                               in1=kvT[:, hout, ssl])
        hT_pool.release()
        attc.release()
        wvkr_pool.release()

        # =================================================================
        # P3: att = rw @ Wo; x1 = x + att; LN2 stats (interleaved)
        # P4: LN2 apply + token shift + time-mix -> h2s bf16 (pipelined)
        # =================================================================
        # right stack: x1 (to end) under h2s (to P5 end) under ln2 (to P3 end)
        x1_pool = tc.alloc_tile_pool(name="x1_pool", bufs=1, side="right")
        x1T = x1_pool.tile([128, KH, S], f32r)
        h2_pool = tc.alloc_tile_pool(name="h2_pool", bufs=1, side="right")
        h2s = h2_pool.tile([128, KH, S + 1], bf16)
        ln2_tmp = tc.alloc_tile_pool(name="ln2_tmp", bufs=2, side="right")
        wo_pool = tc.alloc_tile_pool(name="wo_pool", bufs=2)
        # seed the token shift: h2s[:, k, 0] = shift row
        for k in range(KH):
            vec.tensor_copy(out=h2s[:, k, 0:1], in_=sh_t[:, k:k + 1])

        def p3_hout(sc, hout):
            ssl = sc_sl(sc)
            woc = wo_pool.tile([128, KH, 128], bf16, tag="wo", name="woc")
            sy.dma_start(out=woc[:, :, :], in_=wo_d[hout])
            pa = mm_tile()
            for k in range(KH):
                nc.tensor.matmul(pa[:, :], woc[:, k, :], kvT[:, k, ssl],
                                 start=(k == 0), stop=(k == KH - 1))
            vec.tensor_add(out=x1T[:, hout, ssl], in0=pa[:, :],
                           in1=xT[:, hout, ssl])

        def p3_stats(sc):
            ssl = sc_sl(sc)
            s1p = mm_tile(1)
            s2p = mm_tile(1)
            for k in range(KH):
                sq = ln2_tmp.tile([128, SC], bf16, tag="sq", name="sq")
                act.activation(out=sq[:, :], in_=x1T[:, k, ssl],
                               func=Act.Square)
                nc.tensor.matmul(s1p[:, :], ones[:, :], x1T[:, k, ssl],
                                 start=(k == 0), stop=(k == KH - 1))
                nc.tensor.matmul(s2p[:, :], ones_bf[:, :], sq[:, :],
                                 start=(k == 0), stop=(k == KH - 1))
            return ln_finish(s1p, s2p, ln2_tmp)

        def p3_bc(fin):
            m_row, rs_row = fin
            m2b = ln2_tmp.tile([128, SC], f32, name="m2b", bufs=1)
            rs2b = ln2_tmp.tile([128, SC], f32, name="rs2b", bufs=1)
            bc_row(m_row[0:1, :], m2b[:, :])
            bc_row(rs_row[0:1, :], rs2b[:, :])
            return m2b, rs2b

        def p4(sc, m2b, rs2b):
            for k in range(KH):
                t1 = ln2_tmp.tile([128, SC], f32, tag="lt", name="t1")
                nc.gpsimd.tensor_sub(out=t1[:, :], in0=x1T[:, k, ssl2(sc)],
                                     in1=m2b[:, :])
                vec.tensor_mul(out=h2s[:, k, 1 + sc * SC:1 + (sc + 1) * SC],
                               in0=t1[:, :], in1=rs2b[:, :])
                d_t = ln2_tmp.tile([128, SC], bf16, tag="dt", name="d_t")
                vec.tensor_sub(out=d_t[:, :],
                               in0=h2s[:, k, 1 + sc * SC:1 + (sc + 1) * SC],
                               in1=h2s[:, k, sc * SC:(sc + 1) * SC])
                vec.scalar_tensor_tensor(
                    out=h2s[:, k, sc * SC:(sc + 1) * SC], in0=d_t[:, :],
                    scalar=tmk_t[:, k:k + 1],
                    in1=h2s[:, k, sc * SC:(sc + 1) * SC],
                    op0=Alu.mult, op1=Alu.add)

        def ssl2(sc):
            return sc_sl(sc)

        # staged emission: stats de-interleaved; broadcasts tucked behind the
        # next chunk's matmuls; P4 vector work shadowed by pa/P5 matmuls
        for hout in range(KH):
            p3_hout(0, hout)
        fin0 = p3_stats(0)
        p3_hout(1, 0)
        p3_hout(1, 1)
        mb0 = p3_bc(fin0)
        for hout in range(2, KH):
            p3_hout(1, hout)
        p4(0, *mb0)
        fin1 = p3_stats(1)
        wo_pool.release()
        kvT_pool.release()
        xT_pool.release()

        # =================================================================
        # P5: kk = relu(sqrt(s_kk)*(km @ Wkey' + bkey))^2 -> fp8 (SBUF)
        # P6: out = x1 + (kk@Wval)*sigmoid(kk@Wgate); transpose; store
        # =================================================================
        kk_pool = tc.alloc_tile_pool(name="kk_pool", bufs=1)
        kkw_pool = tc.alloc_tile_pool(name="kkw_pool", bufs=3)
        wvg_pool = fin_pool = ot_pool = None
        sqrt_skk = float(np.sqrt(S_KK)) if VG_FP8 else 1.0

        def p5_group(sc, ff, kk):
            wyc = kkw_pool.tile([128, KH, 128], bf16, name="wyc")
            sy.dma_start(out=wyc[:, :, :], in_=wkey_d[ff])
            pkk = mm_tile()
            for k in range(KH):
                nc.tensor.matmul(pkk[:, :], wyc[:, k, :],
                                 h2s[:, k, sc * SC:(sc + 1) * SC],
                                 start=(k == 0), stop=(k == KH - 1))
            u_t = vtmp_pool.tile([128, SC], bf16, name="u_t")
            act.activation(out=u_t[:, :], in_=pkk[:, :], func=Act.Relu,
                           bias=bkey_t[:, ff:ff + 1], scale=sqrt_skk)
            vec.tensor_mul(out=kk[:, ff, :], in0=u_t[:, :], in1=u_t[:, :])

        def p6(sc, kk):
            for hout in range(KH):
                wv8 = wvg_pool.tile([128, KF, 128], vg_dt, tag="wv8",
                                    name="wv8")
                sy.dma_start(out=wv8[:, :, :], in_=wval_d[hout])
                wg8 = wvg_pool.tile([128, KF, 128], vg_dt, tag="wg8",
                                    name="wg8")
                sy.dma_start(out=wg8[:, :, :], in_=wgate_d[hout])
                for dc in range(NDC):
                    dsl = slice(dc * DC, (dc + 1) * DC)
                    xsl = slice(sc * SC + dc * DC, sc * SC + (dc + 1) * DC)
                    psv = psum.tile([128, DC], f32, tag="mm", bufs=6,
                                    name="psv")
                    psg = psum.tile([128, DC], f32, tag="mm", bufs=6,
                                    name="psg")
                    if VG_FP8:
                        for f in range(KF // 2):
                            nc.tensor.matmul(psv[:, :],
                                             wv8[:, 2 * f:2 * f + 2, :],
                                             kk[:, 2 * f:2 * f + 2, dsl],
                                             start=(f == 0),
                                             stop=(f == KF // 2 - 1),
                                             perf_mode=DR)
                        for f in range(KF // 2):
                            nc.tensor.matmul(psg[:, :],
                                             wg8[:, 2 * f:2 * f + 2, :],
                                             kk[:, 2 * f:2 * f + 2, dsl],
                                             start=(f == 0),
                                             stop=(f == KF // 2 - 1),
                                             perf_mode=DR)
                    else:
                        for f in range(KF):
                            nc.tensor.matmul(psv[:, :], wv8[:, f, :],
                                             kk[:, f, dsl],
                                             start=(f == 0),
                                             stop=(f == KF - 1))
                        for f in range(KF):
                            nc.tensor.matmul(psg[:, :], wg8[:, f, :],
                                             kk[:, f, dsl],
                                             start=(f == 0),
                                             stop=(f == KF - 1))
                    sig_t = fin_pool.tile([128, DC], f32, name="sig_t")
                    act.activation(out=sig_t[:, :], in_=psg[:, :],
                                   func=Act.Sigmoid, scale=scl_t[:, 1:2])
                    glu_t = fin_pool.tile([128, DC], f32, name="glu_t")
                    vec.tensor_scalar_mul(out=glu_t[:, :], in0=psv[:, :],
                                          scalar1=scl_t[:, 0:1])
                    vec.tensor_mul(out=glu_t[:, :], in0=glu_t[:, :],
                                   in1=sig_t[:, :])
                    vec.tensor_add(out=glu_t[:, :], in0=glu_t[:, :],
                                   in1=x1T[:, hout, xsl])
                    for j in range(DC // 128):
                        tp = trp_tile()
                        nc.tensor.transpose(tp[:, :],
                                            glu_t[:, j * 128:(j + 1) * 128],
                                            ident[:, :])
                        ot = ot_pool.tile([128, 128], f32, name="ot")
                        vec.tensor_copy(out=ot[:, :], in_=tp[:, :])
                        tt = (sc * SC + dc * DC) // 128 + j
                        sy.dma_start(
                            out=out_d[tt * 128:(tt + 1) * 128,
                                      hout * 128:(hout + 1) * 128],
                            in_=ot[:, :])

        kk0 = kk_pool.tile([128, KF, SC], vg_dt, tag="kk", name="kk")
        for ff in range(4):
            p5_group(0, ff, kk0)
        mb1 = p3_bc(fin1)
        p4(1, *mb1)
        for ff in range(4, KF):
            p5_group(0, ff, kk0)
        ln2_tmp.release()
        wvg_pool = tc.alloc_tile_pool(name="wvg_pool", bufs=2)
        fin_pool = tc.alloc_tile_pool(name="fin_pool", bufs=4)
        ot_pool = tc.alloc_tile_pool(name="ot_pool", bufs=4)
        p6(0, kk0)
        kk1 = kk_pool.tile([128, KF, SC], vg_dt, tag="kk", name="kk")
        for ff in range(KF):
            p5_group(1, ff, kk1)
        p6(1, kk1)
        ot_pool.release()
        fin_pool.release()
        wvg_pool.release()
        kkw_pool.release()
        kk_pool.release()
        h2_pool.release()
        x1_pool.release()
        vtmp_pool.release()
        consts.release()
        psum.release()
    nc.finalize()
    return nc


# ---------------------------------------------------------------------------
# host side
# ---------------------------------------------------------------------------

def _ln_np(x, s, b):
    m = x.mean(-1, keepdims=True)
    vv = ((x - m) ** 2).mean(-1, keepdims=True)
    return (x - m) / np.sqrt(vv + 1e-5) * s + b


def _h2hat_row(xrow, att_state_b, ln1_s, ln1_b, ln2_s, ln2_b, td, lvl_w,
               lvl_b, Wv, Wk, Wr, Wo):
    """(x1 - m)/std for a single token row (LN2 without scale/bias)."""
    h = _ln_np(xrow[None, :], ln1_s, ln1_b)[0]
    vv = h @ Wv
    kk = h @ Wk
    rr = 1.0 / (1.0 + np.exp(-(h @ Wr)))
    lg = h @ lvl_w + lvl_b
    e = np.exp(lg - lg.max())
    lw = e / e.sum()
    decay = np.exp(-np.exp(td))
    weighted = (lw[None, :] @ (att_state_b * decay))[0] + kk * vv
    att = (rr * weighted) @ Wo
    x1 = xrow + att
    m = x1.mean()
    sd = np.sqrt(((x1 - m) ** 2).mean() + 1e-5)
    return ((x1 - m) / sd).astype(np.float32)


def _arrange_hkh(W):
    """[H, H] -> [KH, 128, KH, 128]: arr[ho, p, k, m] = W[k*128+p, ho*128+m]"""
    Wr = W.reshape(KH, 128, -1, 128)            # [k, p, ho, m]
    return np.ascontiguousarray(Wr.transpose(2, 1, 0, 3))


def _arrange_cols(v):
    """[H] -> [128, KH]: arr[p, k] = v[k*128+p]"""
    return np.ascontiguousarray(v.reshape(-1, 128).T)


_BUILT = None


def _get_built():
    global _BUILT
    if _BUILT is None:
        _BUILT = build_bass()
    return _BUILT


def make_in_maps(x, att_state, cm_state, ln1_s, ln1_b, ln2_s, ln2_b,
                 td_multi, lvl_w, lvl_b, Wv, Wk, Wr, Wo, tmk,
                 Wkey, Wval, Wgate):
    f = np.float32
    x = np.asarray(x, f)
    att_state = np.asarray(att_state, f)
    cm_state = np.asarray(cm_state, f)
    ln1_s, ln1_b = np.asarray(ln1_s, f), np.asarray(ln1_b, f)
    ln2_s, ln2_b = np.asarray(ln2_s, f), np.asarray(ln2_b, f)
    td = np.asarray(td_multi, f)
    lvl_w, lvl_b = np.asarray(lvl_w, f), np.asarray(lvl_b, f)
    Wv, Wk, Wr, Wo = (np.asarray(a, f) for a in (Wv, Wk, Wr, Wo))
    tmk = np.asarray(tmk, f)
    Wkey, Wval, Wgate = (np.asarray(a, f) for a in (Wkey, Wval, Wgate))

    # fold LN1 scale into Wv/Wk/Wr/lvl_w; LN1 bias becomes output biases
    decay = np.exp(-np.exp(td))
    sqrt_skk = np.sqrt(S_KK) if VG_FP8 else 1.0
    if VG_FP8:
        s_wv = 224.0 / max(np.abs(Wval).max(), 1e-9)
        s_wg = 224.0 / max(np.abs(Wgate).max(), 1e-9)
        wval_a = np.ascontiguousarray(
            _arrange_khf(np.clip(Wval * s_wv, -240, 240)).astype(E4NP))
        wgate_a = np.ascontiguousarray(
            _arrange_khf(np.clip(Wgate * s_wg, -240, 240)).astype(E4NP))
        scl = np.tile(np.array([1.0 / (S_KK * s_wv),
                                1.0 / (S_KK * s_wg)], f), (128, 1))
    else:
        wval_a = np.ascontiguousarray(_arrange_khf(Wval).astype(BFNP))
        wgate_a = np.ascontiguousarray(_arrange_khf(Wgate).astype(BFNP))
        scl = np.tile(np.array([1.0, 1.0], f), (128, 1))

    shared = {
        "lvl_w": np.ascontiguousarray(
            (ln1_s[:, None] * lvl_w).reshape(KH, 128, D)
            .transpose(1, 0, 2)).astype(BFNP),
        "lvl_b": lvl_b + ln1_b @ lvl_w,
        "tmk": _arrange_cols(tmk),
        "bv": _arrange_cols(ln1_b @ Wv),
        "bk": _arrange_cols(ln1_b @ Wk),
        "br": _arrange_cols(ln1_b @ Wr),
        "bkey": np.ascontiguousarray(
            ((ln2_b @ Wkey) * sqrt_skk).reshape(KF, 128).T),
        "Wv": _arrange_hkh(ln1_s[:, None] * Wv).astype(BFNP),
        "Wk": _arrange_hkh(ln1_s[:, None] * Wk).astype(BFNP),
        "Wr": _arrange_hkh(ln1_s[:, None] * Wr).astype(BFNP),
        "Wo": _arrange_hkh(Wo).astype(BFNP),
        "Wkey": _arrange_khf_key(ln2_s[:, None] * Wkey).astype(BFNP),
        "Wval": wval_a,
        "Wgate": wgate_a,
        "scl": scl,
    }
    shared = {k: np.ascontiguousarray(v) for k, v in shared.items()}

    in_maps = []
    for c in range(NCORES):
        b, piece = c // 2, c % 2
        t0 = piece * S
        if piece == 0:
            shift = (cm_state[b] - ln2_b) / ln2_s
        else:
            shift = _h2hat_row(x[b, t0 - 1], att_state[b], ln1_s, ln1_b,
                               ln2_s, ln2_b, td, lvl_w, lvl_b, Wv, Wk, Wr, Wo)
        in_maps.append({
            "xbf": np.ascontiguousarray(x[b, t0:t0 + S].astype(BFNP)),
            "shift_in": np.ascontiguousarray(
                shift.reshape(KH, 128).T.astype(BFNP)),
            "asd": np.ascontiguousarray(att_state[b] * decay, f),
            **shared,
        })
    return in_maps


def _arrange_khf(W):
    """[FF, H] -> [KH, 128, KF, 128]: arr[ho, p, f, m] = W[f*128+p, ho*128+m]"""
    Wr = W.reshape(KF, 128, KH, 128)            # [f, p, ho, m]
    return np.ascontiguousarray(Wr.transpose(2, 1, 0, 3))


def _arrange_khf_key(W):
    """[H, FF] -> [KF, 128, KH, 128]: arr[fo, p, k, m] = W[k*128+p, fo*128+m]"""
    Wr = W.reshape(KH, 128, KF, 128)            # [k, p, fo, m]
    return np.ascontiguousarray(Wr.transpose(2, 1, 0, 3))


def kernel(x, att_state, cm_state, ln1_s, ln1_b, ln2_s, ln2_b,
           td_multi, lvl_w, lvl_b, Wv, Wk, Wr, Wo, tmk,
           Wkey, Wval, Wgate):
    from concourse.bass_utils import run_bass_kernel_spmd

    in_maps = make_in_maps(x, att_state, cm_state, ln1_s, ln1_b, ln2_s,
                           ln2_b, td_multi, lvl_w, lvl_b, Wv, Wk, Wr, Wo,
                           tmk, Wkey, Wval, Wgate)
    nc = _get_built()
    res = run_bass_kernel_spmd(nc, in_maps, list(range(NCORES)))
    out = np.empty((B, T, H), np.float32)
    for c in range(NCORES):
        b, piece = c // 2, c % 2
        out[b, piece * S:(piece + 1) * S] = res.results[c]["out"]
    return out


# revision 15
# speedup vs baseline: 1.0359x; 1.0050x over previous
"""EnhancedRWKVBlock Trainium2 kernel (optimized).

Sharding: 8 cores = 4 batches x 2 sequence halves (pure data parallel).
The channel-mix token shift across the halves is seeded by one host-computed
row per odd shard.

Key optimizations over the f32r baseline:
  - All big matmuls run in bf16 (same PE rate as f32r, half the LDWEIGHTS
    time, half the weight DMA); the val/gate matmuls run in fp8e4 DoubleRow
    (2x PE rate) with host-quantized weights.
  - Weights are pre-arranged on host into [tile, 128, k, 128] layouts so
    every weight DMA is contiguous per partition (the baseline was DMA
    descriptor-bound with 2048 x 256B scatters per tile).
  - LN scale/bias are folded into the following projection weights on host;
    the level-mix 1/z is folded into e_t once instead of per (hout, sc).
  - No DRAM spills: xT/x1T/kk stay SBUF-resident.
  - sc-major phase ordering pipelines the vector-only LN2/token-shift block
    behind matmul phases, removing the PE bubble.
"""

import numpy as np
import ml_dtypes

B, T, H, D, FF = 4, 2048, 2048, 4, 8192
NCORES = 8
KH = H // 128            # 16 feature tiles of H
KF = FF // 128           # 64 feature tiles of FF
S = T // 2               # tokens per core
SC = 512                 # token chunk for bf16 matmuls
NSC = S // SC
DC = 256                 # token chunk for fp8 DoubleRow matmuls
NDC = SC // DC

VG_FP8 = True            # val/gate matmuls in fp8e4 DoubleRow
S_KK = 2.0               # fp8 scale for kk = relu(.)^2  (max kk ~27 << 120)

E4NP = ml_dtypes.float8_e4m3
BFNP = ml_dtypes.bfloat16


# ---------------------------------------------------------------------------
# device kernel
# ---------------------------------------------------------------------------

def build_bass():
    import concourse.bass as bass
    from concourse import bacc
    import concourse.mybir as mybir
    import concourse.tile as tile
    from concourse.masks import make_identity

    f32 = mybir.dt.float32
    f32r = mybir.dt.float32r
    bf16 = mybir.dt.bfloat16
    fp8 = mybir.dt.float8e4
    Alu = mybir.AluOpType
    Act = mybir.ActivationFunctionType
    DR = mybir.MatmulPerfMode.DoubleRow

    inv_h = 1.0 / H
    vg_dt = fp8 if VG_FP8 else bf16

    nc = bacc.Bacc()

    # --- external I/O (per core) ---
    x_d = nc.dram_tensor("xbf", [S, H], bf16, kind="ExternalInput")
    sh_d = nc.dram_tensor("shift_in", [128, KH], bf16, kind="ExternalInput")
    asd_d = nc.dram_tensor("asd", [D, H], f32, kind="ExternalInput")
    lvlw_d = nc.dram_tensor("lvl_w", [128, KH, D], bf16, kind="ExternalInput")
    lvlb_d = nc.dram_tensor("lvl_b", [D], f32, kind="ExternalInput")
    tmk_d = nc.dram_tensor("tmk", [128, KH], f32, kind="ExternalInput")
    bv_d = nc.dram_tensor("bv", [128, KH], f32, kind="ExternalInput")
    bk_d = nc.dram_tensor("bk", [128, KH], f32, kind="ExternalInput")
    br_d = nc.dram_tensor("br", [128, KH], f32, kind="ExternalInput")
    bkey_d = nc.dram_tensor("bkey", [128, KF], f32, kind="ExternalInput")
    wv_d = nc.dram_tensor("Wv", [KH, 128, KH, 128], bf16, kind="ExternalInput")
    wk_d = nc.dram_tensor("Wk", [KH, 128, KH, 128], bf16, kind="ExternalInput")
    wr_d = nc.dram_tensor("Wr", [KH, 128, KH, 128], bf16, kind="ExternalInput")
    wo_d = nc.dram_tensor("Wo", [KH, 128, KH, 128], bf16, kind="ExternalInput")
    wkey_d = nc.dram_tensor("Wkey", [KF, 128, KH, 128], bf16,
                            kind="ExternalInput")
    wval_d = nc.dram_tensor("Wval", [KH, 128, KF, 128], vg_dt,
                            kind="ExternalInput")
    wgate_d = nc.dram_tensor("Wgate", [KH, 128, KF, 128], vg_dt,
                             kind="ExternalInput")
    scl_d = nc.dram_tensor("scl", [128, 2], f32, kind="ExternalInput")
    out_d = nc.dram_tensor("out", [S, H], f32, kind="ExternalOutput")

    vec = nc.vector
    act = nc.scalar
    sy = nc.sync

    def sc_sl(sc):
        return slice(sc * SC, (sc + 1) * SC)

    with tile.TileContext(nc) as tc, \
            nc.allow_low_precision(reason="bf16/fp8 matmuls within rel-err budget"):
        # ---- persistent constants ----
        consts = tc.alloc_tile_pool(name="consts", bufs=1)
        ident = consts.tile([128, 128], f32)
        make_identity(nc, ident)
        ident_bf = consts.tile([128, 128], bf16)
        vec.tensor_copy(out=ident_bf[:, :], in_=ident[:, :])
        ones_f = consts.tile([128, 1], f32)
        vec.memset(ones_f[:, :], 1.0)
        ones = consts.tile([128, 1], f32r)
        vec.tensor_copy(out=ones[:, :], in_=ones_f[:, :])
        ones_bf = consts.tile([128, 1], bf16)
        vec.tensor_copy(out=ones_bf[:, :], in_=ones_f[:, :])
        ones_row_f = consts.tile([1, 128], f32)
        vec.memset(ones_row_f[:, :], 1.0)
        ones_row = consts.tile([1, 128], f32r)
        vec.tensor_copy(out=ones_row[:, :], in_=ones_row_f[:, :])
        eps_t = consts.tile([1, 1], f32)
        vec.memset(eps_t[:, :], 1e-5)
        tmk_t = consts.tile([128, KH], f32)
        sy.dma_start(out=tmk_t[:, :], in_=tmk_d[:, :])
        bv_t = consts.tile([128, KH], f32)
        sy.dma_start(out=bv_t[:, :], in_=bv_d[:, :])
        bk_t = consts.tile([128, KH], f32)
        sy.dma_start(out=bk_t[:, :], in_=bk_d[:, :])
        br_t = consts.tile([128, KH], f32)
        sy.dma_start(out=br_t[:, :], in_=br_d[:, :])
        bkey_t = consts.tile([128, KF], f32)
        sy.dma_start(out=bkey_t[:, :], in_=bkey_d[:, :])
        sh_t = consts.tile([128, KH], bf16)
        sy.dma_start(out=sh_t[:, :], in_=sh_d[:, :])
        scl_t = consts.tile([128, 2], f32)
        sy.dma_start(out=scl_t[:, :], in_=scl_d[:, :])

        # ---- attention-scoped constants ----
        attc = tc.alloc_tile_pool(name="attc", bufs=1, side="right")
        lvlw_t = attc.tile([128, KH, D], bf16)
        sy.dma_start(out=lvlw_t[:, :, :], in_=lvlw_d[:, :, :])
        lvlb_t = attc.tile([D, 1], f32)
        sy.dma_start(out=lvlb_t[:, :], in_=lvlb_d[:])
        asd_t = attc.tile([D, H], f32r)   # att_state * decay (host)
        sy.dma_start(out=asd_t[:, :], in_=asd_d[:, :].bitcast(f32r))
        e_t = attc.tile([D, S], f32r)     # softmax-normalized level weights
        zr_t = attc.tile([1, S], f32r)

        # ---- shared PSUM pool ----
        psum = tc.alloc_tile_pool(name="psum", bufs=1, space="PSUM")

        def mm_tile(p0=128, w=SC):
            return psum.tile([p0, w], f32, tag="mm", bufs=6, name="pt")

        def trp_tile():
            return psum.tile([128, 128], f32, tag="trp", bufs=2, name="tp")

        def bc_row(row_ap, dst_slice, w=SC):
            # broadcast a [1, w] f32r row across 128 partitions (K=1 matmul)
            pb = psum.tile([128, w], f32, tag="mm", bufs=6, name="pb")
            nc.tensor.matmul(pb[:, :], ones_row[:, :], row_ap,
                             start=True, stop=True)
            vec.tensor_copy(out=dst_slice, in_=pb[:, :])

        def ln_finish(s1p, s2p, tmp_pool):
            m_row = tmp_pool.tile([1, SC], f32r, name="mrow", bufs=1)
            vec.tensor_scalar_mul(out=m_row[:, :], in0=s1p[:, :],
                                  scalar1=inv_h)
            msq = tmp_pool.tile([1, SC], f32, name="msq", bufs=1)
            vec.tensor_mul(out=msq[:, :], in0=m_row[:, :], in1=m_row[:, :])
            var = tmp_pool.tile([1, SC], f32, name="var", bufs=1)
            vec.scalar_tensor_tensor(out=var[:, :], in0=s2p[:, :],
                                     scalar=inv_h, in1=msq[:, :],
                                     op0=Alu.mult, op1=Alu.subtract)
            act.activation(out=var[:, :], in_=var[:, :], func=Act.Sqrt,
                           bias=eps_t[:, 0:1])
            rs_row = tmp_pool.tile([1, SC], f32r, name="rsrow", bufs=1)
            vec.reciprocal(out=rs_row[:, :], in_=var[:, :])
            return m_row, rs_row

        # =================================================================
        # P0: load x, transpose to feature-major; LN1 stats+apply -> hT bf16
        # =================================================================
        vtmp_pool = tc.alloc_tile_pool(name="vtmp_pool", bufs=3)
        xT_pool = tc.alloc_tile_pool(name="xT_pool", bufs=1)
        xT = xT_pool.tile([128, KH, S], bf16)
        hT_pool = tc.alloc_tile_pool(name="hT_pool", bufs=1, side="right")
        hT = hT_pool.tile([128, KH, S], bf16)
        ln1_tmp = tc.alloc_tile_pool(name="ln1_tmp", bufs=3)
        tok_pool = tc.alloc_tile_pool(name="tok_pool", bufs=2)
        NTOK = S // 128
        for tt in range(NTOK):
            xtok = tok_pool.tile([128, H], bf16, name="xtok")
            sy.dma_start(out=xtok[:, :], in_=x_d[tt * 128:(tt + 1) * 128, :])
            for k in range(KH):
                tp = psum.tile([128, 128], bf16, tag="trp", bufs=2, name="tpb")
                nc.tensor.transpose(tp[:, :], xtok[:, k * 128:(k + 1) * 128],
                                    ident_bf[:, :])
                vec.tensor_copy(out=xT[:, k, tt * 128:(tt + 1) * 128],
                                in_=tp[:, :])
        tok_pool.release()

        fins = []
        for sc in range(NSC):
            ssl = sc_sl(sc)
            s1p = mm_tile(1)
            s2p = mm_tile(1)
            for k in range(KH):
                sq = ln1_tmp.tile([128, SC], bf16, tag="sq", name="sq")
                act.activation(out=sq[:, :], in_=xT[:, k, ssl],
                               func=Act.Square)
                nc.tensor.matmul(s1p[:, :], ones_bf[:, :], xT[:, k, ssl],
                                 start=(k == 0), stop=(k == KH - 1))
                nc.tensor.matmul(s2p[:, :], ones_bf[:, :], sq[:, :],
                                 start=(k == 0), stop=(k == KH - 1))
            fins.append((s1p, s2p))
        rows = [ln_finish(*fins[0], ln1_tmp)]
        for sc in range(NSC):
            ssl = sc_sl(sc)
            m_row, rs_row = rows[sc]
            m1b = ln1_tmp.tile([128, SC], bf16, name="m1b", bufs=1)
            rs1b = ln1_tmp.tile([128, SC], bf16, name="rs1b", bufs=1)
            bc_row(m_row[0:1, :], m1b[:, :])
            bc_row(rs_row[0:1, :], rs1b[:, :])
            if sc + 1 < NSC:
                rows.append(ln_finish(*fins[sc + 1], ln1_tmp))
            for k in range(KH):
                t1 = ln1_tmp.tile([128, SC], bf16, tag="lt", name="t1")
                vec.tensor_sub(out=t1[:, :], in0=xT[:, k, ssl],
                               in1=m1b[:, :])
                vec.tensor_mul(out=hT[:, k, ssl], in0=t1[:, :],
                               in1=rs1b[:, :])
            # level weights: e = exp(h@lvl_w + lvl_b) with 1/z folded in
            lp = mm_tile(D)
            for k in range(KH):
                nc.tensor.matmul(lp[:, :], lvlw_t[:, k, :], hT[:, k, ssl],
                                 start=(k == 0), stop=(k == KH - 1))
            act.activation(out=e_t[:, ssl], in_=lp[:, :], func=Act.Exp,
                           bias=lvlb_t[:, 0:1])
            zp = mm_tile(1)
            nc.tensor.matmul(zp[:, :], ones[0:D, :], e_t[:, ssl],
                             start=True, stop=True)
            vec.reciprocal(out=zr_t[:, ssl], in_=zp[:, :])
            z4 = psum.tile([D, SC], f32, tag="mm", bufs=6, name="z4")
            nc.tensor.matmul(z4[:, :], ones_row[0:1, 0:D], zr_t[0:1, ssl],
                             start=True, stop=True)
            vec.tensor_mul(out=e_t[:, ssl], in0=e_t[:, ssl], in1=z4[:, :])
        ln1_tmp.release()

        # =================================================================
        # P2: v/k/r projections, kv, weighted level term, r gate -> kvT bf16
        # =================================================================
        kvT_pool = tc.alloc_tile_pool(name="kvT_pool", bufs=1)
        kvT = kvT_pool.tile([128, KH, S], bf16)
        wvkr_pool = tc.alloc_tile_pool(name="wvkr_pool", bufs=2)
        for sc in range(NSC):
            ssl = sc_sl(sc)
            for hout in range(KH):
                wvc = wvkr_pool.tile([128, KH, 128], bf16, tag="wv", name="wvc")
                sy.dma_start(out=wvc[:, :, :], in_=wv_d[hout])
                wkc = wvkr_pool.tile([128, KH, 128], bf16, tag="wk", name="wkc")
                sy.dma_start(out=wkc[:, :, :], in_=wk_d[hout])
                wrc = wvkr_pool.tile([128, KH, 128], bf16, tag="wr", name="wrc")
                sy.dma_start(out=wrc[:, :, :], in_=wr_d[hout])
                pv = mm_tile()
                for k in range(KH):
                    nc.tensor.matmul(pv[:, :], wvc[:, k, :], hT[:, k, ssl],
                                     start=(k == 0), stop=(k == KH - 1))
                v_t = vtmp_pool.tile([128, SC], f32, name="v_t")
                vec.tensor_scalar_add(out=v_t[:, :], in0=pv[:, :],
                                      scalar1=bv_t[:, hout:hout + 1])
                pk = mm_tile()
                for k in range(KH):
                    nc.tensor.matmul(pk[:, :], wkc[:, k, :], hT[:, k, ssl],
                                     start=(k == 0), stop=(k == KH - 1))
                # kv = (k + bk) * v
                vec.scalar_tensor_tensor(out=kvT[:, hout, ssl], in0=pk[:, :],
                                         scalar=bk_t[:, hout:hout + 1],
                                         in1=v_t[:, :],
                                         op0=Alu.add, op1=Alu.mult)
                # + level-weighted state term
                hsl = slice(hout * 128, (hout + 1) * 128)
                pw1 = mm_tile()
                nc.tensor.matmul(pw1[:, :], asd_t[:, hsl], e_t[:, ssl],
                                 start=True, stop=True)
                vec.tensor_add(out=kvT[:, hout, ssl], in0=pw1[:, :],
                               in1=kvT[:, hout, ssl])
                # * sigmoid(r)
                pr = mm_tile()
                for k in range(KH):
                    nc.tensor.matmul(pr[:, :], wrc[:, k, :], hT[:, k, ssl],
                                     start=(k == 0), stop=(k == KH - 1))
                r_t = vtmp_pool.tile([128, SC], f32, name="r_t")
                act.activation(out=r_t[:, :], in_=pr[:, :], func=Act.Sigmoid,
                               bias=br_t[:, hout:hout + 1])
                vec.tensor_mul(out=kvT[:, hout, ssl], in0=r_t[:, :],
                               in1=kvT[:, hout, ssl])
        hT_pool.release()
        attc.release()
        wvkr_pool.release()

        # =================================================================
        # P3: att = rw @ Wo; x1 = x + att; LN2 stats (interleaved)
        # P4: LN2 apply + token shift + time-mix -> h2s bf16 (pipelined)
        # =================================================================
        # right stack: x1 (to end) under h2s (to P5 end) under ln2 (to P3 end)
        x1_pool = tc.alloc_tile_pool(name="x1_pool", bufs=1, side="right")
        x1T = x1_pool.tile([128, KH, S], f32r)
        h2_pool = tc.alloc_tile_pool(name="h2_pool", bufs=1, side="right")
        h2s = h2_pool.tile([128, KH, S + 1], bf16)
        ln2_tmp = tc.alloc_tile_pool(name="ln2_tmp", bufs=2, side="right")
        wo_pool = tc.alloc_tile_pool(name="wo_pool", bufs=2)
        # seed the token shift: h2s[:, k, 0] = shift row
        for k in range(KH):
            vec.tensor_copy(out=h2s[:, k, 0:1], in_=sh_t[:, k:k + 1])

        def p3_hout(sc, hout):
            ssl = sc_sl(sc)
            woc = wo_pool.tile([128, KH, 128], bf16, tag="wo", name="woc")
            sy.dma_start(out=woc[:, :, :], in_=wo_d[hout])
            pa = mm_tile()
            for k in range(KH):
                nc.tensor.matmul(pa[:, :], woc[:, k, :], kvT[:, k, ssl],
                                 start=(k == 0), stop=(k == KH - 1))
            vec.tensor_add(out=x1T[:, hout, ssl], in0=pa[:, :],
                           in1=xT[:, hout, ssl])

        def p3_stats(sc):
            ssl = sc_sl(sc)
            s1p = mm_tile(1)
            s2p = mm_tile(1)
            for k in range(KH):
                sq = ln2_tmp.tile([128, SC], bf16, tag="sq", name="sq")
                act.activation(out=sq[:, :], in_=x1T[:, k, ssl],
                               func=Act.Square)
                nc.tensor.matmul(s1p[:, :], ones[:, :], x1T[:, k, ssl],
                                 start=(k == 0), stop=(k == KH - 1))
                nc.tensor.matmul(s2p[:, :], ones_bf[:, :], sq[:, :],
                                 start=(k == 0), stop=(k == KH - 1))
            return ln_finish(s1p, s2p, ln2_tmp)

        def p3_bc(fin):
            m_row, rs_row = fin
            m2b = ln2_tmp.tile([128, SC], bf16, name="m2b", bufs=1)
            rs2b = ln2_tmp.tile([128, SC], bf16, name="rs2b", bufs=1)
            bc_row(m_row[0:1, :], m2b[:, :])
            bc_row(rs_row[0:1, :], rs2b[:, :])
            return m2b, rs2b

        def p4(sc, m2b, rs2b):
            for k in range(KH):
                t1 = ln2_tmp.tile([128, SC], bf16, tag="lt", name="t1")
                vec.tensor_sub(out=t1[:, :], in0=x1T[:, k, ssl2(sc)],
                               in1=m2b[:, :])
                vec.tensor_mul(out=h2s[:, k, 1 + sc * SC:1 + (sc + 1) * SC],
                               in0=t1[:, :], in1=rs2b[:, :])
                d_t = ln2_tmp.tile([128, SC], bf16, tag="dt", name="d_t")
                vec.tensor_sub(out=d_t[:, :],
                               in0=h2s[:, k, 1 + sc * SC:1 + (sc + 1) * SC],
                               in1=h2s[:, k, sc * SC:(sc + 1) * SC])
                vec.scalar_tensor_tensor(
                    out=h2s[:, k, sc * SC:(sc + 1) * SC], in0=d_t[:, :],
                    scalar=tmk_t[:, k:k + 1],
                    in1=h2s[:, k, sc * SC:(sc + 1) * SC],
                    op0=Alu.mult, op1=Alu.add)

        def ssl2(sc):
            return sc_sl(sc)

        # staged emission: stats de-interleaved; broadcasts tucked behind the
        # next chunk's matmuls; P4 vector work shadowed by pa/P5 matmuls
        for hout in range(KH):
            p3_hout(0, hout)
        fin0 = p3_stats(0)
        p3_hout(1, 0)
        p3_hout(1, 1)
        mb0 = p3_bc(fin0)
        p4(0, *mb0)
        for hout in range(2, KH):
            p3_hout(1, hout)
        fin1 = p3_stats(1)
        wo_pool.release()
        kvT_pool.release()
        xT_pool.release()

        # =================================================================
        # P5: kk = relu(sqrt(s_kk)*(km @ Wkey' + bkey))^2 -> fp8 (SBUF)
        # P6: out = x1 + (kk@Wval)*sigmoid(kk@Wgate); transpose; store
        # =================================================================
        kk_pool = tc.alloc_tile_pool(name="kk_pool", bufs=1)
        kkw_pool = tc.alloc_tile_pool(name="kkw_pool", bufs=3)
        wvg_pool = fin_pool = ot_pool = None
        sqrt_skk = float(np.sqrt(S_KK)) if VG_FP8 else 1.0

        def p5_group(sc, ff, kk):
            wyc = kkw_pool.tile([128, KH, 128], bf16, name="wyc")
            sy.dma_start(out=wyc[:, :, :], in_=wkey_d[ff])
            pkk = mm_tile()
            for k in range(KH):
                nc.tensor.matmul(pkk[:, :], wyc[:, k, :],
                                 h2s[:, k, sc * SC:(sc + 1) * SC],
                                 start=(k == 0), stop=(k == KH - 1))
            u_t = vtmp_pool.tile([128, SC], bf16, name="u_t")
            act.activation(out=u_t[:, :], in_=pkk[:, :], func=Act.Relu,
                           bias=bkey_t[:, ff:ff + 1], scale=sqrt_skk)
            vec.tensor_mul(out=kk[:, ff, :], in0=u_t[:, :], in1=u_t[:, :])

        def p6(sc, kk):
            for hout in range(KH):
                wv8 = wvg_pool.tile([128, KF, 128], vg_dt, tag="wv8",
                                    name="wv8")
                sy.dma_start(out=wv8[:, :, :], in_=wval_d[hout])
                wg8 = wvg_pool.tile([128, KF, 128], vg_dt, tag="wg8",
                                    name="wg8")
                sy.dma_start(out=wg8[:, :, :], in_=wgate_d[hout])
                for dc in range(NDC):
                    dsl = slice(dc * DC, (dc + 1) * DC)
                    xsl = slice(sc * SC + dc * DC, sc * SC + (dc + 1) * DC)
                    psv = psum.tile([128, DC], f32, tag="mm", bufs=6,
                                    name="psv")
                    psg = psum.tile([128, DC], f32, tag="mm", bufs=6,
                                    name="psg")
                    if VG_FP8:
                        for f in range(KF // 2):
                            nc.tensor.matmul(psv[:, :],
                                             wv8[:, 2 * f:2 * f + 2, :],
                                             kk[:, 2 * f:2 * f + 2, dsl],
                                             start=(f == 0),
                                             stop=(f == KF // 2 - 1),
                                             perf_mode=DR)
                        for f in range(KF // 2):
                            nc.tensor.matmul(psg[:, :],
                                             wg8[:, 2 * f:2 * f + 2, :],
                                             kk[:, 2 * f:2 * f + 2, dsl],
                                             start=(f == 0),
                                             stop=(f == KF // 2 - 1),
                                             perf_mode=DR)
                    else:
                        for f in range(KF):
                            nc.tensor.matmul(psv[:, :], wv8[:, f, :],
                                             kk[:, f, dsl],
                                             start=(f == 0),
                                             stop=(f == KF - 1))
                        for f in range(KF):
                            nc.tensor.matmul(psg[:, :], wg8[:, f, :],
                                             kk[:, f, dsl],
                                             start=(f == 0),
                                             stop=(f == KF - 1))
                    sig_t = fin_pool.tile([128, DC], f32, name="sig_t")
                    act.activation(out=sig_t[:, :], in_=psg[:, :],
                                   func=Act.Sigmoid, scale=scl_t[:, 1:2])
                    glu_t = fin_pool.tile([128, DC], f32, name="glu_t")
                    vec.tensor_scalar_mul(out=glu_t[:, :], in0=psv[:, :],
                                          scalar1=scl_t[:, 0:1])
                    vec.tensor_mul(out=glu_t[:, :], in0=glu_t[:, :],
                                   in1=sig_t[:, :])
                    vec.tensor_add(out=glu_t[:, :], in0=glu_t[:, :],
                                   in1=x1T[:, hout, xsl])
                    for j in range(DC // 128):
                        tp = trp_tile()
                        nc.tensor.transpose(tp[:, :],
                                            glu_t[:, j * 128:(j + 1) * 128],
                                            ident[:, :])
                        ot = ot_pool.tile([128, 128], f32, name="ot")
                        vec.tensor_copy(out=ot[:, :], in_=tp[:, :])
                        tt = (sc * SC + dc * DC) // 128 + j
                        sy.dma_start(
                            out=out_d[tt * 128:(tt + 1) * 128,
                                      hout * 128:(hout + 1) * 128],
                            in_=ot[:, :])

        kk0 = kk_pool.tile([128, KF, SC], vg_dt, tag="kk", name="kk")
        for ff in range(4):
            p5_group(0, ff, kk0)
        mb1 = p3_bc(fin1)
        p4(1, *mb1)
        for ff in range(4, KF):
            p5_group(0, ff, kk0)
        ln2_tmp.release()
        wvg_pool = tc.alloc_tile_pool(name="wvg_pool", bufs=2)
        fin_pool = tc.alloc_tile_pool(name="fin_pool", bufs=4)
        ot_pool = tc.alloc_tile_pool(name="ot_pool", bufs=4)
        p6(0, kk0)
        kk1 = kk_pool.tile([128, KF, SC], vg_dt, tag="kk", name="kk")
        for ff in range(KF):
            p5_group(1, ff, kk1)
        p6(1, kk1)
        ot_pool.release()
        fin_pool.release()
        wvg_pool.release()
        kkw_pool.release()
        kk_pool.release()
        h2_pool.release()
        x1_pool.release()
        vtmp_pool.release()
        consts.release()
        psum.release()
    nc.finalize()
    return nc


# ---------------------------------------------------------------------------
# host side
# ---------------------------------------------------------------------------

def _ln_np(x, s, b):
    m = x.mean(-1, keepdims=True)
    vv = ((x - m) ** 2).mean(-1, keepdims=True)
    return (x - m) / np.sqrt(vv + 1e-5) * s + b


def _h2hat_row(xrow, att_state_b, ln1_s, ln1_b, ln2_s, ln2_b, td, lvl_w,
               lvl_b, Wv, Wk, Wr, Wo):
    """(x1 - m)/std for a single token row (LN2 without scale/bias)."""
    h = _ln_np(xrow[None, :], ln1_s, ln1_b)[0]
    vv = h @ Wv
    kk = h @ Wk
    rr = 1.0 / (1.0 + np.exp(-(h @ Wr)))
    lg = h @ lvl_w + lvl_b
    e = np.exp(lg - lg.max())
    lw = e / e.sum()
    decay = np.exp(-np.exp(td))
    weighted = (lw[None, :] @ (att_state_b * decay))[0] + kk * vv
    att = (rr * weighted) @ Wo
    x1 = xrow + att
    m = x1.mean()
    sd = np.sqrt(((x1 - m) ** 2).mean() + 1e-5)
    return ((x1 - m) / sd).astype(np.float32)


def _arrange_hkh(W):
    """[H, H] -> [KH, 128, KH, 128]: arr[ho, p, k, m] = W[k*128+p, ho*128+m]"""
    Wr = W.reshape(KH, 128, -1, 128)            # [k, p, ho, m]
    return np.ascontiguousarray(Wr.transpose(2, 1, 0, 3))


def _arrange_cols(v):
    """[H] -> [128, KH]: arr[p, k] = v[k*128+p]"""
    return np.ascontiguousarray(v.reshape(-1, 128).T)


_BUILT = None


def _get_built():
    global _BUILT
    if _BUILT is None:
        _BUILT = build_bass()
    return _BUILT


def make_in_maps(x, att_state, cm_state, ln1_s, ln1_b, ln2_s, ln2_b,
                 td_multi, lvl_w, lvl_b, Wv, Wk, Wr, Wo, tmk,
                 Wkey, Wval, Wgate):
    f = np.float32
    x = np.asarray(x, f)
    att_state = np.asarray(att_state, f)
    cm_state = np.asarray(cm_state, f)
    ln1_s, ln1_b = np.asarray(ln1_s, f), np.asarray(ln1_b, f)
    ln2_s, ln2_b = np.asarray(ln2_s, f), np.asarray(ln2_b, f)
    td = np.asarray(td_multi, f)
    lvl_w, lvl_b = np.asarray(lvl_w, f), np.asarray(lvl_b, f)
    Wv, Wk, Wr, Wo = (np.asarray(a, f) for a in (Wv, Wk, Wr, Wo))
    tmk = np.asarray(tmk, f)
    Wkey, Wval, Wgate = (np.asarray(a, f) for a in (Wkey, Wval, Wgate))

    # fold LN1 scale into Wv/Wk/Wr/lvl_w; LN1 bias becomes output biases
    decay = np.exp(-np.exp(td))
    sqrt_skk = np.sqrt(S_KK) if VG_FP8 else 1.0
    if VG_FP8:
        s_wv = 224.0 / max(np.abs(Wval).max(), 1e-9)
        s_wg = 224.0 / max(np.abs(Wgate).max(), 1e-9)
        wval_a = np.ascontiguousarray(
            _arrange_khf(np.clip(Wval * s_wv, -240, 240)).astype(E4NP))
        wgate_a = np.ascontiguousarray(
            _arrange_khf(np.clip(Wgate * s_wg, -240, 240)).astype(E4NP))
        scl = np.tile(np.array([1.0 / (S_KK * s_wv),
                                1.0 / (S_KK * s_wg)], f), (128, 1))
    else:
        wval_a = np.ascontiguousarray(_arrange_khf(Wval).astype(BFNP))
        wgate_a = np.ascontiguousarray(_arrange_khf(Wgate).astype(BFNP))
        scl = np.tile(np.array([1.0, 1.0], f), (128, 1))

    shared = {
        "lvl_w": np.ascontiguousarray(
            (ln1_s[:, None] * lvl_w).reshape(KH, 128, D)
            .transpose(1, 0, 2)).astype(BFNP),
        "lvl_b": lvl_b + ln1_b @ lvl_w,
        "tmk": _arrange_cols(tmk),
        "bv": _arrange_cols(ln1_b @ Wv),
        "bk": _arrange_cols(ln1_b @ Wk),
        "br": _arrange_cols(ln1_b @ Wr),
        "bkey": np.ascontiguousarray(
            ((ln2_b @ Wkey) * sqrt_skk).reshape(KF, 128).T),
        "Wv": _arrange_hkh(ln1_s[:, None] * Wv).astype(BFNP),
        "Wk": _arrange_hkh(ln1_s[:, None] * Wk).astype(BFNP),
        "Wr": _arrange_hkh(ln1_s[:, None] * Wr).astype(BFNP),
        "Wo": _arrange_hkh(Wo).astype(BFNP),
        "Wkey": _arrange_khf_key(ln2_s[:, None] * Wkey).astype(BFNP),
        "Wval": wval_a,
        "Wgate": wgate_a,
        "scl": scl,
    }
    shared = {k: np.ascontiguousarray(v) for k, v in shared.items()}

    in_maps = []
    for c in range(NCORES):
        b, piece = c // 2, c % 2
        t0 = piece * S
        if piece == 0:
            shift = (cm_state[b] - ln2_b) / ln2_s
        else:
            shift = _h2hat_row(x[b, t0 - 1], att_state[b], ln1_s, ln1_b,
                               ln2_s, ln2_b, td, lvl_w, lvl_b, Wv, Wk, Wr, Wo)
        in_maps.append({
            "xbf": np.ascontiguousarray(x[b, t0:t0 + S].astype(BFNP)),
            "shift_in": np.ascontiguousarray(
                shift.reshape(KH, 128).T.astype(BFNP)),
            "asd": np.ascontiguousarray(att_state[b] * decay, f),
            **shared,
        })
    return in_maps


def _arrange_khf(W):
    """[FF, H] -> [KH, 128, KF, 128]: arr[ho, p, f, m] = W[f*128+p, ho*128+m]"""
    Wr = W.reshape(KF, 128, KH, 128)            # [f, p, ho, m]
    return np.ascontiguousarray(Wr.transpose(2, 1, 0, 3))


def _arrange_khf_key(W):
    """[H, FF] -> [KF, 128, KH, 128]: arr[fo, p, k, m] = W[k*128+p, fo*128+m]"""
    Wr = W.reshape(KH, 128, KF, 128)            # [k, p, fo, m]
    return np.ascontiguousarray(Wr.transpose(2, 1, 0, 3))


def kernel(x, att_state, cm_state, ln1_s, ln1_b, ln2_s, ln2_b,
           td_multi, lvl_w, lvl_b, Wv, Wk, Wr, Wo, tmk,
           Wkey, Wval, Wgate):
    from concourse.bass_utils import run_bass_kernel_spmd

    in_maps = make_in_maps(x, att_state, cm_state, ln1_s, ln1_b, ln2_s,
                           ln2_b, td_multi, lvl_w, lvl_b, Wv, Wk, Wr, Wo,
                           tmk, Wkey, Wval, Wgate)
    nc = _get_built()
    res = run_bass_kernel_spmd(nc, in_maps, list(range(NCORES)))
    out = np.empty((B, T, H), np.float32)
    for c in range(NCORES):
        b, piece = c // 2, c % 2
        out[b, piece * S:(piece + 1) * S] = res.results[c]["out"]
    return out


# revision 17
# speedup vs baseline: 1.0487x; 1.0123x over previous
"""EnhancedRWKVBlock Trainium2 kernel (optimized).

Sharding: 8 cores = 4 batches x 2 sequence halves (pure data parallel).
The channel-mix token shift across the halves is seeded by one host-computed
row per odd shard.

Key optimizations over the f32r baseline:
  - All big matmuls run in bf16 (same PE rate as f32r, half the LDWEIGHTS
    time, half the weight DMA); the val/gate matmuls run in fp8e4 DoubleRow
    (2x PE rate) with host-quantized weights.
  - Weights are pre-arranged on host into [tile, 128, k, 128] layouts so
    every weight DMA is contiguous per partition (the baseline was DMA
    descriptor-bound with 2048 x 256B scatters per tile).
  - LN scale/bias are folded into the following projection weights on host;
    the level-mix 1/z is folded into e_t once instead of per (hout, sc).
  - No DRAM spills: xT/x1T/kk stay SBUF-resident.
  - sc-major phase ordering pipelines the vector-only LN2/token-shift block
    behind matmul phases, removing the PE bubble.
"""

import numpy as np
import ml_dtypes

B, T, H, D, FF = 4, 2048, 2048, 4, 8192
NCORES = 8
KH = H // 128            # 16 feature tiles of H
KF = FF // 128           # 64 feature tiles of FF
S = T // 2               # tokens per core
SC = 512                 # token chunk for bf16 matmuls
NSC = S // SC
DC = 256                 # token chunk for fp8 DoubleRow matmuls
NDC = SC // DC

VG_FP8 = True            # val/gate matmuls in fp8e4 DoubleRow
S_KK = 2.0               # fp8 scale for kk = relu(.)^2  (max kk ~27 << 120)

E4NP = ml_dtypes.float8_e4m3
BFNP = ml_dtypes.bfloat16


# ---------------------------------------------------------------------------
# device kernel
# ---------------------------------------------------------------------------

def build_bass():
    import concourse.bass as bass
    from concourse import bacc
    import concourse.mybir as mybir
    import concourse.tile as tile
    from concourse.masks import make_identity

    f32 = mybir.dt.float32
    f32r = mybir.dt.float32r
    bf16 = mybir.dt.bfloat16
    fp8 = mybir.dt.float8e4
    Alu = mybir.AluOpType
    Act = mybir.ActivationFunctionType
    DR = mybir.MatmulPerfMode.DoubleRow

    inv_h = 1.0 / H
    vg_dt = fp8 if VG_FP8 else bf16

    nc = bacc.Bacc()

    # --- external I/O (per core) ---
    x_d = nc.dram_tensor("xbf", [S, H], bf16, kind="ExternalInput")
    sh_d = nc.dram_tensor("shift_in", [128, KH], bf16, kind="ExternalInput")
    asd_d = nc.dram_tensor("asd", [D, H], bf16, kind="ExternalInput")
    lvlw_d = nc.dram_tensor("lvl_w", [128, KH, D], bf16, kind="ExternalInput")
    lvlb_d = nc.dram_tensor("lvl_b", [D], f32, kind="ExternalInput")
    tmk_d = nc.dram_tensor("tmk", [128, KH], f32, kind="ExternalInput")
    bv_d = nc.dram_tensor("bv", [128, KH], f32, kind="ExternalInput")
    bk_d = nc.dram_tensor("bk", [128, KH], f32, kind="ExternalInput")
    br_d = nc.dram_tensor("br", [128, KH], f32, kind="ExternalInput")
    bkey_d = nc.dram_tensor("bkey", [128, KF], f32, kind="ExternalInput")
    wv_d = nc.dram_tensor("Wv", [KH, 128, KH, 128], bf16, kind="ExternalInput")
    wk_d = nc.dram_tensor("Wk", [KH, 128, KH, 128], bf16, kind="ExternalInput")
    wr_d = nc.dram_tensor("Wr", [KH, 128, KH, 128], bf16, kind="ExternalInput")
    wo_d = nc.dram_tensor("Wo", [KH, 128, KH, 128], bf16, kind="ExternalInput")
    wkey_d = nc.dram_tensor("Wkey", [KF, 128, KH, 128], bf16,
                            kind="ExternalInput")
    wval_d = nc.dram_tensor("Wval", [KH, 128, KF, 128], vg_dt,
                            kind="ExternalInput")
    wgate_d = nc.dram_tensor("Wgate", [KH, 128, KF, 128], vg_dt,
                             kind="ExternalInput")
    scl_d = nc.dram_tensor("scl", [128, 2], f32, kind="ExternalInput")
    out_d = nc.dram_tensor("out", [S, H], f32, kind="ExternalOutput")

    vec = nc.vector
    act = nc.scalar
    sy = nc.sync

    def sc_sl(sc):
        return slice(sc * SC, (sc + 1) * SC)

    with tile.TileContext(nc) as tc, \
            nc.allow_low_precision(reason="bf16/fp8 matmuls within rel-err budget"):
        # ---- persistent constants ----
        consts = tc.alloc_tile_pool(name="consts", bufs=1)
        ident = consts.tile([128, 128], f32)
        make_identity(nc, ident)
        ident_bf = consts.tile([128, 128], bf16)
        vec.tensor_copy(out=ident_bf[:, :], in_=ident[:, :])
        ones_f = consts.tile([128, 1], f32)
        vec.memset(ones_f[:, :], 1.0)
        ones = consts.tile([128, 1], f32r)
        vec.tensor_copy(out=ones[:, :], in_=ones_f[:, :])
        ones_bf = consts.tile([128, 1], bf16)
        vec.tensor_copy(out=ones_bf[:, :], in_=ones_f[:, :])
        ones_row_f = consts.tile([1, 128], f32)
        vec.memset(ones_row_f[:, :], 1.0)
        ones_row = consts.tile([1, 128], f32r)
        vec.tensor_copy(out=ones_row[:, :], in_=ones_row_f[:, :])
        eps_t = consts.tile([1, 1], f32)
        vec.memset(eps_t[:, :], 1e-5)
        tmk_t = consts.tile([128, KH], f32)
        sy.dma_start(out=tmk_t[:, :], in_=tmk_d[:, :])
        bv_t = consts.tile([128, KH], f32)
        sy.dma_start(out=bv_t[:, :], in_=bv_d[:, :])
        bk_t = consts.tile([128, KH], f32)
        sy.dma_start(out=bk_t[:, :], in_=bk_d[:, :])
        br_t = consts.tile([128, KH], f32)
        sy.dma_start(out=br_t[:, :], in_=br_d[:, :])
        bkey_t = consts.tile([128, KF], f32)
        sy.dma_start(out=bkey_t[:, :], in_=bkey_d[:, :])
        sh_t = consts.tile([128, KH], bf16)
        sy.dma_start(out=sh_t[:, :], in_=sh_d[:, :])
        scl_t = consts.tile([128, 2], f32)
        sy.dma_start(out=scl_t[:, :], in_=scl_d[:, :])

        # ---- attention-scoped constants ----
        attc = tc.alloc_tile_pool(name="attc", bufs=1, side="right")
        lvlw_t = attc.tile([128, KH, D], bf16)
        sy.dma_start(out=lvlw_t[:, :, :], in_=lvlw_d[:, :, :])
        lvlb_t = attc.tile([D, 1], f32)
        sy.dma_start(out=lvlb_t[:, :], in_=lvlb_d[:])
        asd_t = attc.tile([D, H], bf16)   # att_state * decay (host)
        sy.dma_start(out=asd_t[:, :], in_=asd_d[:, :])
        e_t = attc.tile([D, S], bf16)     # softmax-normalized level weights
        zr_t = attc.tile([1, S], f32r)

        # ---- shared PSUM pool ----
        psum = tc.alloc_tile_pool(name="psum", bufs=1, space="PSUM")

        def mm_tile(p0=128, w=SC):
            return psum.tile([p0, w], f32, tag="mm", bufs=6, name="pt")

        def trp_tile():
            return psum.tile([128, 128], f32, tag="trp", bufs=2, name="tp")

        def bc_row(row_ap, dst_slice, w=SC):
            # broadcast a [1, w] f32r row across 128 partitions (K=1 matmul)
            pb = psum.tile([128, w], f32, tag="mm", bufs=6, name="pb")
            nc.tensor.matmul(pb[:, :], ones_row[:, :], row_ap,
                             start=True, stop=True)
            vec.tensor_copy(out=dst_slice, in_=pb[:, :])

        def ln_finish(s1p, s2p, tmp_pool):
            m_row = tmp_pool.tile([1, SC], f32r, name="mrow", bufs=1)
            vec.tensor_scalar_mul(out=m_row[:, :], in0=s1p[:, :],
                                  scalar1=inv_h)
            msq = tmp_pool.tile([1, SC], f32, name="msq", bufs=1)
            vec.tensor_mul(out=msq[:, :], in0=m_row[:, :], in1=m_row[:, :])
            var = tmp_pool.tile([1, SC], f32, name="var", bufs=1)
            vec.scalar_tensor_tensor(out=var[:, :], in0=s2p[:, :],
                                     scalar=inv_h, in1=msq[:, :],
                                     op0=Alu.mult, op1=Alu.subtract)
            act.activation(out=var[:, :], in_=var[:, :], func=Act.Sqrt,
                           bias=eps_t[:, 0:1])
            rs_row = tmp_pool.tile([1, SC], f32r, name="rsrow", bufs=1)
            vec.reciprocal(out=rs_row[:, :], in_=var[:, :])
            return m_row, rs_row

        # =================================================================
        # P0: load x, transpose to feature-major; LN1 stats+apply -> hT bf16
        # =================================================================
        vtmp_pool = tc.alloc_tile_pool(name="vtmp_pool", bufs=3)
        xT_pool = tc.alloc_tile_pool(name="xT_pool", bufs=1)
        xT = xT_pool.tile([128, KH, S], bf16)
        hT_pool = tc.alloc_tile_pool(name="hT_pool", bufs=1, side="right")
        hT = hT_pool.tile([128, KH, S], bf16)
        ln1_tmp = tc.alloc_tile_pool(name="ln1_tmp", bufs=3)
        tok_pool = tc.alloc_tile_pool(name="tok_pool", bufs=3)
        NTOK = S // 128
        for tt in range(NTOK):
            xtok = tok_pool.tile([128, H], bf16, name="xtok")
            sy.dma_start(out=xtok[:, :], in_=x_d[tt * 128:(tt + 1) * 128, :])
            for k in range(KH):
                tp = psum.tile([128, 128], bf16, tag="trp", bufs=2, name="tpb")
                nc.tensor.transpose(tp[:, :], xtok[:, k * 128:(k + 1) * 128],
                                    ident_bf[:, :])
                if k % 2 == 0:
                    vec.tensor_copy(out=xT[:, k, tt * 128:(tt + 1) * 128],
                                    in_=tp[:, :])
                else:
                    act.activation(out=xT[:, k, tt * 128:(tt + 1) * 128],
                                   in_=tp[:, :], func=Act.Copy)
        tok_pool.release()

        fins = []
        for sc in range(NSC):
            ssl = sc_sl(sc)
            s1p = mm_tile(1)
            s2p = mm_tile(1)
            for k in range(KH):
                sq = ln1_tmp.tile([128, SC], bf16, tag="sq", name="sq")
                act.activation(out=sq[:, :], in_=xT[:, k, ssl],
                               func=Act.Square)
                nc.tensor.matmul(s1p[:, :], ones_bf[:, :], xT[:, k, ssl],
                                 start=(k == 0), stop=(k == KH - 1))
                nc.tensor.matmul(s2p[:, :], ones_bf[:, :], sq[:, :],
                                 start=(k == 0), stop=(k == KH - 1))
            fins.append((s1p, s2p))
        rows = [ln_finish(*fins[0], ln1_tmp)]
        for sc in range(NSC):
            ssl = sc_sl(sc)
            m_row, rs_row = rows[sc]
            m1b = ln1_tmp.tile([128, SC], bf16, name="m1b", bufs=1)
            rs1b = ln1_tmp.tile([128, SC], bf16, name="rs1b", bufs=1)
            bc_row(m_row[0:1, :], m1b[:, :])
            bc_row(rs_row[0:1, :], rs1b[:, :])
            if sc + 1 < NSC:
                rows.append(ln_finish(*fins[sc + 1], ln1_tmp))
            for k in range(KH):
                t1 = ln1_tmp.tile([128, SC], bf16, tag="lt", name="t1")
                vec.tensor_sub(out=t1[:, :], in0=xT[:, k, ssl],
                               in1=m1b[:, :])
                vec.tensor_mul(out=hT[:, k, ssl], in0=t1[:, :],
                               in1=rs1b[:, :])
            # level weights: e = exp(h@lvl_w + lvl_b) with 1/z folded in
            lp = mm_tile(D)
            for k in range(KH):
                nc.tensor.matmul(lp[:, :], lvlw_t[:, k, :], hT[:, k, ssl],
                                 start=(k == 0), stop=(k == KH - 1))
            act.activation(out=e_t[:, ssl], in_=lp[:, :], func=Act.Exp,
                           bias=lvlb_t[:, 0:1])
            zp = mm_tile(1)
            nc.tensor.matmul(zp[:, :], ones_bf[0:D, :], e_t[:, ssl],
                             start=True, stop=True)
            vec.reciprocal(out=zr_t[:, ssl], in_=zp[:, :])
            z4 = psum.tile([D, SC], f32, tag="mm", bufs=6, name="z4")
            nc.tensor.matmul(z4[:, :], ones_row[0:1, 0:D], zr_t[0:1, ssl],
                             start=True, stop=True)
            vec.tensor_mul(out=e_t[:, ssl], in0=e_t[:, ssl], in1=z4[:, :])
        ln1_tmp.release()

        # =================================================================
        # P2: v/k/r projections, kv, weighted level term, r gate -> kvT bf16
        # =================================================================
        kvT_pool = tc.alloc_tile_pool(name="kvT_pool", bufs=1)
        kvT = kvT_pool.tile([128, KH, S], bf16)
        wvkr_pool = tc.alloc_tile_pool(name="wvkr_pool", bufs=2)
        for sc in range(NSC):
            ssl = sc_sl(sc)
            for hout in range(KH):
                wvc = wvkr_pool.tile([128, KH, 128], bf16, tag="wv", name="wvc")
                sy.dma_start(out=wvc[:, :, :], in_=wv_d[hout])
                wkc = wvkr_pool.tile([128, KH, 128], bf16, tag="wk", name="wkc")
                sy.dma_start(out=wkc[:, :, :], in_=wk_d[hout])
                wrc = wvkr_pool.tile([128, KH, 128], bf16, tag="wr", name="wrc")
                sy.dma_start(out=wrc[:, :, :], in_=wr_d[hout])
                pv = mm_tile()
                for k in range(KH):
                    nc.tensor.matmul(pv[:, :], wvc[:, k, :], hT[:, k, ssl],
                                     start=(k == 0), stop=(k == KH - 1))
                v_t = vtmp_pool.tile([128, SC], f32, name="v_t")
                vec.tensor_scalar_add(out=v_t[:, :], in0=pv[:, :],
                                      scalar1=bv_t[:, hout:hout + 1])
                pk = mm_tile()
                for k in range(KH):
                    nc.tensor.matmul(pk[:, :], wkc[:, k, :], hT[:, k, ssl],
                                     start=(k == 0), stop=(k == KH - 1))
                # kv = (k + bk) * v
                vec.scalar_tensor_tensor(out=kvT[:, hout, ssl], in0=pk[:, :],
                                         scalar=bk_t[:, hout:hout + 1],
                                         in1=v_t[:, :],
                                         op0=Alu.add, op1=Alu.mult)
                # + level-weighted state term
                hsl = slice(hout * 128, (hout + 1) * 128)
                pw1 = mm_tile()
                nc.tensor.matmul(pw1[:, :], asd_t[:, hsl], e_t[:, ssl],
                                 start=True, stop=True)
                vec.tensor_add(out=kvT[:, hout, ssl], in0=pw1[:, :],
                               in1=kvT[:, hout, ssl])
                # * sigmoid(r)
                pr = mm_tile()
                for k in range(KH):
                    nc.tensor.matmul(pr[:, :], wrc[:, k, :], hT[:, k, ssl],
                                     start=(k == 0), stop=(k == KH - 1))
                r_t = vtmp_pool.tile([128, SC], f32, name="r_t")
                act.activation(out=r_t[:, :], in_=pr[:, :], func=Act.Sigmoid,
                               bias=br_t[:, hout:hout + 1])
                vec.tensor_mul(out=kvT[:, hout, ssl], in0=r_t[:, :],
                               in1=kvT[:, hout, ssl])
        hT_pool.release()
        attc.release()
        wvkr_pool.release()

        # =================================================================
        # P3: att = rw @ Wo; x1 = x + att; LN2 stats (interleaved)
        # P4: LN2 apply + token shift + time-mix -> h2s bf16 (pipelined)
        # =================================================================
        # right stack: x1 (to end) under h2s (to P5 end) under ln2 (to P3 end)
        x1_pool = tc.alloc_tile_pool(name="x1_pool", bufs=1, side="right")
        x1T = x1_pool.tile([128, KH, S], f32r)
        h2_pool = tc.alloc_tile_pool(name="h2_pool", bufs=1, side="right")
        h2s = h2_pool.tile([128, KH, S + 1], bf16)
        ln2_tmp = tc.alloc_tile_pool(name="ln2_tmp", bufs=2, side="right")
        wo_pool = tc.alloc_tile_pool(name="wo_pool", bufs=3)
        # seed the token shift: h2s[:, k, 0] = shift row
        for k in range(KH):
            vec.tensor_copy(out=h2s[:, k, 0:1], in_=sh_t[:, k:k + 1])

        def p3_hout(sc, hout):
            ssl = sc_sl(sc)
            woc = wo_pool.tile([128, KH, 128], bf16, tag="wo", name="woc")
            sy.dma_start(out=woc[:, :, :], in_=wo_d[hout])
            pa = mm_tile()
            for k in range(KH):
                nc.tensor.matmul(pa[:, :], woc[:, k, :], kvT[:, k, ssl],
                                 start=(k == 0), stop=(k == KH - 1))
            vec.tensor_add(out=x1T[:, hout, ssl], in0=pa[:, :],
                           in1=xT[:, hout, ssl])

        def p3_stats(sc):
            ssl = sc_sl(sc)
            s1p = mm_tile(1)
            s2p = mm_tile(1)
            for k in range(KH):
                sq = ln2_tmp.tile([128, SC], bf16, tag="sq", name="sq")
                act.activation(out=sq[:, :], in_=x1T[:, k, ssl],
                               func=Act.Square)
                nc.tensor.matmul(s1p[:, :], ones[:, :], x1T[:, k, ssl],
                                 start=(k == 0), stop=(k == KH - 1))
                nc.tensor.matmul(s2p[:, :], ones_bf[:, :], sq[:, :],
                                 start=(k == 0), stop=(k == KH - 1))
            return ln_finish(s1p, s2p, ln2_tmp)

        def p3_bc(fin):
            m_row, rs_row = fin
            m2b = ln2_tmp.tile([128, SC], bf16, name="m2b", bufs=1)
            rs2b = ln2_tmp.tile([128, SC], bf16, name="rs2b", bufs=1)
            bc_row(m_row[0:1, :], m2b[:, :])
            bc_row(rs_row[0:1, :], rs2b[:, :])
            return m2b, rs2b

        def p4(sc, m2b, rs2b):
            for k in range(KH):
                t1 = ln2_tmp.tile([128, SC], bf16, tag="lt", name="t1")
                vec.tensor_sub(out=t1[:, :], in0=x1T[:, k, ssl2(sc)],
                               in1=m2b[:, :])
                vec.tensor_mul(out=h2s[:, k, 1 + sc * SC:1 + (sc + 1) * SC],
                               in0=t1[:, :], in1=rs2b[:, :])
                d_t = ln2_tmp.tile([128, SC], bf16, tag="dt", name="d_t")
                vec.tensor_sub(out=d_t[:, :],
                               in0=h2s[:, k, 1 + sc * SC:1 + (sc + 1) * SC],
                               in1=h2s[:, k, sc * SC:(sc + 1) * SC])
                vec.scalar_tensor_tensor(
                    out=h2s[:, k, sc * SC:(sc + 1) * SC], in0=d_t[:, :],
                    scalar=tmk_t[:, k:k + 1],
                    in1=h2s[:, k, sc * SC:(sc + 1) * SC],
                    op0=Alu.mult, op1=Alu.add)

        def ssl2(sc):
            return sc_sl(sc)

        # staged emission: stats de-interleaved; broadcasts tucked behind the
        # next chunk's matmuls; P4 vector work shadowed by pa/P5 matmuls
        for hout in range(KH):
            p3_hout(0, hout)
        fin0 = p3_stats(0)
        p3_hout(1, 0)
        p3_hout(1, 1)
        mb0 = p3_bc(fin0)
        p4(0, *mb0)
        for hout in range(2, KH):
            p3_hout(1, hout)
        fin1 = p3_stats(1)
        wo_pool.release()
        kvT_pool.release()
        xT_pool.release()

        # =================================================================
        # P5: kk = relu(sqrt(s_kk)*(km @ Wkey' + bkey))^2 -> fp8 (SBUF)
        # P6: out = x1 + (kk@Wval)*sigmoid(kk@Wgate); transpose; store
        # =================================================================
        kk_pool = tc.alloc_tile_pool(name="kk_pool", bufs=1)
        kkw_pool = tc.alloc_tile_pool(name="kkw_pool", bufs=3)
        wvg_pool = fin_pool = ot_pool = None
        sqrt_skk = float(np.sqrt(S_KK)) if VG_FP8 else 1.0

        def p5_group(sc, ff, kk):
            wyc = kkw_pool.tile([128, KH, 128], bf16, name="wyc")
            sy.dma_start(out=wyc[:, :, :], in_=wkey_d[ff])
            pkk = mm_tile()
            for k in range(KH):
                nc.tensor.matmul(pkk[:, :], wyc[:, k, :],
                                 h2s[:, k, sc * SC:(sc + 1) * SC],
                                 start=(k == 0), stop=(k == KH - 1))
            u_t = vtmp_pool.tile([128, SC], bf16, name="u_t")
            act.activation(out=u_t[:, :], in_=pkk[:, :], func=Act.Relu,
                           bias=bkey_t[:, ff:ff + 1], scale=sqrt_skk)
            vec.tensor_mul(out=kk[:, ff, :], in0=u_t[:, :], in1=u_t[:, :])

        def p6(sc, kk):
            for hout in range(KH):
                wv8 = wvg_pool.tile([128, KF, 128], vg_dt, tag="wv8",
                                    name="wv8")
                sy.dma_start(out=wv8[:, :, :], in_=wval_d[hout])
                wg8 = wvg_pool.tile([128, KF, 128], vg_dt, tag="wg8",
                                    name="wg8")
                sy.dma_start(out=wg8[:, :, :], in_=wgate_d[hout])
                for dc in range(NDC):
                    dsl = slice(dc * DC, (dc + 1) * DC)
                    xsl = slice(sc * SC + dc * DC, sc * SC + (dc + 1) * DC)
                    psv = psum.tile([128, DC], f32, tag="mm", bufs=6,
                                    name="psv")
                    psg = psum.tile([128, DC], f32, tag="mm", bufs=6,
                                    name="psg")
                    if VG_FP8:
                        for f in range(KF // 2):
                            nc.tensor.matmul(psv[:, :],
                                             wv8[:, 2 * f:2 * f + 2, :],
                                             kk[:, 2 * f:2 * f + 2, dsl],
                                             start=(f == 0),
                                             stop=(f == KF // 2 - 1),
                                             perf_mode=DR)
                        for f in range(KF // 2):
                            nc.tensor.matmul(psg[:, :],
                                             wg8[:, 2 * f:2 * f + 2, :],
                                             kk[:, 2 * f:2 * f + 2, dsl],
                                             start=(f == 0),
                                             stop=(f == KF // 2 - 1),
                                             perf_mode=DR)
                    else:
                        for f in range(KF):
                            nc.tensor.matmul(psv[:, :], wv8[:, f, :],
                                             kk[:, f, dsl],
                                             start=(f == 0),
                                             stop=(f == KF - 1))
                        for f in range(KF):
                            nc.tensor.matmul(psg[:, :], wg8[:, f, :],
                                             kk[:, f, dsl],
                                             start=(f == 0),
                                             stop=(f == KF - 1))
                    sig_t = fin_pool.tile([128, DC], f32, name="sig_t")
                    act.activation(out=sig_t[:, :], in_=psg[:, :],
                                   func=Act.Sigmoid, scale=scl_t[:, 1:2])
                    glu_t = fin_pool.tile([128, DC], f32, name="glu_t")
                    vec.tensor_scalar_mul(out=glu_t[:, :], in0=psv[:, :],
                                          scalar1=scl_t[:, 0:1])
                    vec.tensor_mul(out=glu_t[:, :], in0=glu_t[:, :],
                                   in1=sig_t[:, :])
                    vec.tensor_add(out=glu_t[:, :], in0=glu_t[:, :],
                                   in1=x1T[:, hout, xsl])
                    for j in range(DC // 128):
                        tp = trp_tile()
                        nc.tensor.transpose(tp[:, :],
                                            glu_t[:, j * 128:(j + 1) * 128],
                                            ident[:, :])
                        ot = ot_pool.tile([128, 128], f32, name="ot")
                        vec.tensor_copy(out=ot[:, :], in_=tp[:, :])
                        tt = (sc * SC + dc * DC) // 128 + j
                        sy.dma_start(
                            out=out_d[tt * 128:(tt + 1) * 128,
                                      hout * 128:(hout + 1) * 128],
                            in_=ot[:, :])

        kk0 = kk_pool.tile([128, KF, SC], vg_dt, tag="kk", name="kk")
        for ff in range(4):
            p5_group(0, ff, kk0)
        mb1 = p3_bc(fin1)
        p4(1, *mb1)
        for ff in range(4, KF):
            p5_group(0, ff, kk0)
        ln2_tmp.release()
        wvg_pool = tc.alloc_tile_pool(name="wvg_pool", bufs=2)
        fin_pool = tc.alloc_tile_pool(name="fin_pool", bufs=4)
        ot_pool = tc.alloc_tile_pool(name="ot_pool", bufs=4)
        p6(0, kk0)
        kk1 = kk_pool.tile([128, KF, SC], vg_dt, tag="kk", name="kk")
        for ff in range(KF):
            p5_group(1, ff, kk1)
        p6(1, kk1)
        ot_pool.release()
        fin_pool.release()
        wvg_pool.release()
        kkw_pool.release()
        kk_pool.release()
        h2_pool.release()
        x1_pool.release()
        vtmp_pool.release()
        consts.release()
        psum.release()
    nc.finalize()
    return nc


# ---------------------------------------------------------------------------
# host side
# ---------------------------------------------------------------------------

def _ln_np(x, s, b):
    m = x.mean(-1, keepdims=True)
    vv = ((x - m) ** 2).mean(-1, keepdims=True)
    return (x - m) / np.sqrt(vv + 1e-5) * s + b


def _h2hat_row(xrow, att_state_b, ln1_s, ln1_b, ln2_s, ln2_b, td, lvl_w,
               lvl_b, Wv, Wk, Wr, Wo):
    """(x1 - m)/std for a single token row (LN2 without scale/bias)."""
    h = _ln_np(xrow[None, :], ln1_s, ln1_b)[0]
    vv = h @ Wv
    kk = h @ Wk
    rr = 1.0 / (1.0 + np.exp(-(h @ Wr)))
    lg = h @ lvl_w + lvl_b
    e = np.exp(lg - lg.max())
    lw = e / e.sum()
    decay = np.exp(-np.exp(td))
    weighted = (lw[None, :] @ (att_state_b * decay))[0] + kk * vv
    att = (rr * weighted) @ Wo
    x1 = xrow + att
    m = x1.mean()
    sd = np.sqrt(((x1 - m) ** 2).mean() + 1e-5)
    return ((x1 - m) / sd).astype(np.float32)


def _arrange_hkh(W):
    """[H, H] -> [KH, 128, KH, 128]: arr[ho, p, k, m] = W[k*128+p, ho*128+m]"""
    Wr = W.reshape(KH, 128, -1, 128)            # [k, p, ho, m]
    return np.ascontiguousarray(Wr.transpose(2, 1, 0, 3))


def _arrange_cols(v):
    """[H] -> [128, KH]: arr[p, k] = v[k*128+p]"""
    return np.ascontiguousarray(v.reshape(-1, 128).T)


_BUILT = None


def _get_built():
    global _BUILT
    if _BUILT is None:
        _BUILT = build_bass()
    return _BUILT


def make_in_maps(x, att_state, cm_state, ln1_s, ln1_b, ln2_s, ln2_b,
                 td_multi, lvl_w, lvl_b, Wv, Wk, Wr, Wo, tmk,
                 Wkey, Wval, Wgate):
    f = np.float32
    x = np.asarray(x, f)
    att_state = np.asarray(att_state, f)
    cm_state = np.asarray(cm_state, f)
    ln1_s, ln1_b = np.asarray(ln1_s, f), np.asarray(ln1_b, f)
    ln2_s, ln2_b = np.asarray(ln2_s, f), np.asarray(ln2_b, f)
    td = np.asarray(td_multi, f)
    lvl_w, lvl_b = np.asarray(lvl_w, f), np.asarray(lvl_b, f)
    Wv, Wk, Wr, Wo = (np.asarray(a, f) for a in (Wv, Wk, Wr, Wo))
    tmk = np.asarray(tmk, f)
    Wkey, Wval, Wgate = (np.asarray(a, f) for a in (Wkey, Wval, Wgate))

    # fold LN1 scale into Wv/Wk/Wr/lvl_w; LN1 bias becomes output biases
    decay = np.exp(-np.exp(td))
    sqrt_skk = np.sqrt(S_KK) if VG_FP8 else 1.0
    if VG_FP8:
        s_wv = 224.0 / max(np.abs(Wval).max(), 1e-9)
        s_wg = 224.0 / max(np.abs(Wgate).max(), 1e-9)
        wval_a = np.ascontiguousarray(
            _arrange_khf(np.clip(Wval * s_wv, -240, 240)).astype(E4NP))
        wgate_a = np.ascontiguousarray(
            _arrange_khf(np.clip(Wgate * s_wg, -240, 240)).astype(E4NP))
        scl = np.tile(np.array([1.0 / (S_KK * s_wv),
                                1.0 / (S_KK * s_wg)], f), (128, 1))
    else:
        wval_a = np.ascontiguousarray(_arrange_khf(Wval).astype(BFNP))
        wgate_a = np.ascontiguousarray(_arrange_khf(Wgate).astype(BFNP))
        scl = np.tile(np.array([1.0, 1.0], f), (128, 1))

    shared = {
        "lvl_w": np.ascontiguousarray(
            (ln1_s[:, None] * lvl_w).reshape(KH, 128, D)
            .transpose(1, 0, 2)).astype(BFNP),
        "lvl_b": lvl_b + ln1_b @ lvl_w,
        "tmk": _arrange_cols(tmk),
        "bv": _arrange_cols(ln1_b @ Wv),
        "bk": _arrange_cols(ln1_b @ Wk),
        "br": _arrange_cols(ln1_b @ Wr),
        "bkey": np.ascontiguousarray(
            ((ln2_b @ Wkey) * sqrt_skk).reshape(KF, 128).T),
        "Wv": _arrange_hkh(ln1_s[:, None] * Wv).astype(BFNP),
        "Wk": _arrange_hkh(ln1_s[:, None] * Wk).astype(BFNP),
        "Wr": _arrange_hkh(ln1_s[:, None] * Wr).astype(BFNP),
        "Wo": _arrange_hkh(Wo).astype(BFNP),
        "Wkey": _arrange_khf_key(ln2_s[:, None] * Wkey).astype(BFNP),
        "Wval": wval_a,
        "Wgate": wgate_a,
        "scl": scl,
    }
    shared = {k: np.ascontiguousarray(v) for k, v in shared.items()}

    in_maps = []
    for c in range(NCORES):
        b, piece = c // 2, c % 2
        t0 = piece * S
        if piece == 0:
            shift = (cm_state[b] - ln2_b) / ln2_s
        else:
            shift = _h2hat_row(x[b, t0 - 1], att_state[b], ln1_s, ln1_b,
                               ln2_s, ln2_b, td, lvl_w, lvl_b, Wv, Wk, Wr, Wo)
        in_maps.append({
            "xbf": np.ascontiguousarray(x[b, t0:t0 + S].astype(BFNP)),
            "shift_in": np.ascontiguousarray(
                shift.reshape(KH, 128).T.astype(BFNP)),
            "asd": np.ascontiguousarray((att_state[b] * decay).astype(BFNP)),
            **shared,
        })
    return in_maps


def _arrange_khf(W):
    """[FF, H] -> [KH, 128, KF, 128]: arr[ho, p, f, m] = W[f*128+p, ho*128+m]"""
    Wr = W.reshape(KF, 128, KH, 128)            # [f, p, ho, m]
    return np.ascontiguousarray(Wr.transpose(2, 1, 0, 3))


def _arrange_khf_key(W):
    """[H, FF] -> [KF, 128, KH, 128]: arr[fo, p, k, m] = W[k*128+p, fo*128+m]"""
    Wr = W.reshape(KH, 128, KF, 128)            # [k, p, fo, m]
    return np.ascontiguousarray(Wr.transpose(2, 1, 0, 3))


def kernel(x, att_state, cm_state, ln1_s, ln1_b, ln2_s, ln2_b,
           td_multi, lvl_w, lvl_b, Wv, Wk, Wr, Wo, tmk,
           Wkey, Wval, Wgate):
    from concourse.bass_utils import run_bass_kernel_spmd

    in_maps = make_in_maps(x, att_state, cm_state, ln1_s, ln1_b, ln2_s,
                           ln2_b, td_multi, lvl_w, lvl_b, Wv, Wk, Wr, Wo,
                           tmk, Wkey, Wval, Wgate)
    nc = _get_built()
    res = run_bass_kernel_spmd(nc, in_maps, list(range(NCORES)))
    out = np.empty((B, T, H), np.float32)
    for c in range(NCORES):
        b, piece = c // 2, c % 2
        out[b, piece * S:(piece + 1) * S] = res.results[c]["out"]
    return out


# revision 18
# speedup vs baseline: 1.0569x; 1.0079x over previous
"""EnhancedRWKVBlock Trainium2 kernel (optimized).

Sharding: 8 cores = 4 batches x 2 sequence halves (pure data parallel).
The channel-mix token shift across the halves is seeded by one host-computed
row per odd shard.

Key optimizations over the f32r baseline:
  - All big matmuls run in bf16 (same PE rate as f32r, half the LDWEIGHTS
    time, half the weight DMA); the val/gate matmuls run in fp8e4 DoubleRow
    (2x PE rate) with host-quantized weights.
  - Weights are pre-arranged on host into [tile, 128, k, 128] layouts so
    every weight DMA is contiguous per partition (the baseline was DMA
    descriptor-bound with 2048 x 256B scatters per tile).
  - LN scale/bias are folded into the following projection weights on host;
    the level-mix 1/z is folded into e_t once instead of per (hout, sc).
  - No DRAM spills: xT/x1T/kk stay SBUF-resident.
  - sc-major phase ordering pipelines the vector-only LN2/token-shift block
    behind matmul phases, removing the PE bubble.
"""

import numpy as np
import ml_dtypes

B, T, H, D, FF = 4, 2048, 2048, 4, 8192
NCORES = 8
KH = H // 128            # 16 feature tiles of H
KF = FF // 128           # 64 feature tiles of FF
S = T // 2               # tokens per core
SC = 512                 # token chunk for bf16 matmuls
NSC = S // SC
DC = 256                 # token chunk for fp8 DoubleRow matmuls
NDC = SC // DC

VG_FP8 = True            # val/gate matmuls in fp8e4 DoubleRow
S_KK = 2.0               # fp8 scale for kk = relu(.)^2  (max kk ~27 << 120)

E4NP = ml_dtypes.float8_e4m3
BFNP = ml_dtypes.bfloat16


# ---------------------------------------------------------------------------
# device kernel
# ---------------------------------------------------------------------------

def build_bass():
    import concourse.bass as bass
    from concourse import bacc
    import concourse.mybir as mybir
    import concourse.tile as tile
    from concourse.masks import make_identity

    f32 = mybir.dt.float32
    f32r = mybir.dt.float32r
    bf16 = mybir.dt.bfloat16
    fp8 = mybir.dt.float8e4
    Alu = mybir.AluOpType
    Act = mybir.ActivationFunctionType
    DR = mybir.MatmulPerfMode.DoubleRow

    inv_h = 1.0 / H
    vg_dt = fp8 if VG_FP8 else bf16

    nc = bacc.Bacc()

    # --- external I/O (per core) ---
    x_d = nc.dram_tensor("xbf", [S, H], bf16, kind="ExternalInput")
    sh_d = nc.dram_tensor("shift_in", [128, KH], bf16, kind="ExternalInput")
    asd_d = nc.dram_tensor("asd", [D, H], bf16, kind="ExternalInput")
    lvlw_d = nc.dram_tensor("lvl_w", [128, KH, D], bf16, kind="ExternalInput")
    lvlb_d = nc.dram_tensor("lvl_b", [D], f32, kind="ExternalInput")
    tmk_d = nc.dram_tensor("tmk", [128, KH], f32, kind="ExternalInput")
    bv_d = nc.dram_tensor("bv", [128, KH], f32, kind="ExternalInput")
    bk_d = nc.dram_tensor("bk", [128, KH], f32, kind="ExternalInput")
    br_d = nc.dram_tensor("br", [128, KH], f32, kind="ExternalInput")
    bkey_d = nc.dram_tensor("bkey", [128, KF], f32, kind="ExternalInput")
    wv_d = nc.dram_tensor("Wv", [KH, 128, KH, 128], bf16, kind="ExternalInput")
    wk_d = nc.dram_tensor("Wk", [KH, 128, KH, 128], bf16, kind="ExternalInput")
    wr_d = nc.dram_tensor("Wr", [KH, 128, KH, 128], bf16, kind="ExternalInput")
    wo_d = nc.dram_tensor("Wo", [KH, 128, KH, 128], bf16, kind="ExternalInput")
    wkey_d = nc.dram_tensor("Wkey", [KF, 128, KH, 128], bf16,
                            kind="ExternalInput")
    wval_d = nc.dram_tensor("Wval", [KH, 128, KF, 128], vg_dt,
                            kind="ExternalInput")
    wgate_d = nc.dram_tensor("Wgate", [KH, 128, KF, 128], vg_dt,
                             kind="ExternalInput")
    scl_d = nc.dram_tensor("scl", [128, 2], f32, kind="ExternalInput")
    out_d = nc.dram_tensor("out", [S, H], f32, kind="ExternalOutput")

    vec = nc.vector
    act = nc.scalar
    sy = nc.sync

    def sc_sl(sc):
        return slice(sc * SC, (sc + 1) * SC)

    with tile.TileContext(nc) as tc, \
            nc.allow_low_precision(reason="bf16/fp8 matmuls within rel-err budget"):
        # ---- persistent constants ----
        consts = tc.alloc_tile_pool(name="consts", bufs=1)
        ident = consts.tile([128, 128], f32)
        make_identity(nc, ident)
        ident_bf = consts.tile([128, 128], bf16)
        vec.tensor_copy(out=ident_bf[:, :], in_=ident[:, :])
        ones_f = consts.tile([128, 1], f32)
        vec.memset(ones_f[:, :], 1.0)
        ones = consts.tile([128, 1], f32r)
        vec.tensor_copy(out=ones[:, :], in_=ones_f[:, :])
        ones_bf = consts.tile([128, 1], bf16)
        vec.tensor_copy(out=ones_bf[:, :], in_=ones_f[:, :])
        ones_row_f = consts.tile([1, 128], f32)
        vec.memset(ones_row_f[:, :], 1.0)
        ones_row = consts.tile([1, 128], f32r)
        vec.tensor_copy(out=ones_row[:, :], in_=ones_row_f[:, :])
        eps_t = consts.tile([1, 1], f32)
        vec.memset(eps_t[:, :], 1e-5)
        tmk_t = consts.tile([128, KH], f32)
        sy.dma_start(out=tmk_t[:, :], in_=tmk_d[:, :])
        bv_t = consts.tile([128, KH], f32)
        sy.dma_start(out=bv_t[:, :], in_=bv_d[:, :])
        bk_t = consts.tile([128, KH], f32)
        sy.dma_start(out=bk_t[:, :], in_=bk_d[:, :])
        br_t = consts.tile([128, KH], f32)
        sy.dma_start(out=br_t[:, :], in_=br_d[:, :])
        bkey_t = consts.tile([128, KF], f32)
        sy.dma_start(out=bkey_t[:, :], in_=bkey_d[:, :])
        sh_t = consts.tile([128, KH], bf16)
        sy.dma_start(out=sh_t[:, :], in_=sh_d[:, :])
        scl_t = consts.tile([128, 2], f32)
        sy.dma_start(out=scl_t[:, :], in_=scl_d[:, :])

        # ---- attention-scoped constants ----
        attc = tc.alloc_tile_pool(name="attc", bufs=1, side="right")
        lvlw_t = attc.tile([128, KH, D], bf16)
        sy.dma_start(out=lvlw_t[:, :, :], in_=lvlw_d[:, :, :])
        lvlb_t = attc.tile([D, 1], f32)
        sy.dma_start(out=lvlb_t[:, :], in_=lvlb_d[:])
        asd_t = attc.tile([D, H], bf16)   # att_state * decay (host)
        sy.dma_start(out=asd_t[:, :], in_=asd_d[:, :])
        e_t = attc.tile([D, S], bf16)     # softmax-normalized level weights
        zr_t = attc.tile([1, S], f32r)

        # ---- shared PSUM pool ----
        psum = tc.alloc_tile_pool(name="psum", bufs=1, space="PSUM")

        def mm_tile(p0=128, w=SC):
            return psum.tile([p0, w], f32, tag="mm", bufs=6, name="pt")

        def trp_tile():
            return psum.tile([128, 128], f32, tag="trp", bufs=2, name="tp")

        def bc_row(row_ap, dst_slice, w=SC):
            # broadcast a [1, w] f32r row across 128 partitions (K=1 matmul)
            pb = psum.tile([128, w], f32, tag="mm", bufs=6, name="pb")
            nc.tensor.matmul(pb[:, :], ones_row[:, :], row_ap,
                             start=True, stop=True)
            vec.tensor_copy(out=dst_slice, in_=pb[:, :])

        def ln_finish(s1p, s2p, tmp_pool):
            m_row = tmp_pool.tile([1, SC], f32r, name="mrow", bufs=1)
            vec.tensor_scalar_mul(out=m_row[:, :], in0=s1p[:, :],
                                  scalar1=inv_h)
            msq = tmp_pool.tile([1, SC], f32, name="msq", bufs=1)
            vec.tensor_mul(out=msq[:, :], in0=m_row[:, :], in1=m_row[:, :])
            var = tmp_pool.tile([1, SC], f32, name="var", bufs=1)
            vec.scalar_tensor_tensor(out=var[:, :], in0=s2p[:, :],
                                     scalar=inv_h, in1=msq[:, :],
                                     op0=Alu.mult, op1=Alu.subtract)
            act.activation(out=var[:, :], in_=var[:, :], func=Act.Sqrt,
                           bias=eps_t[:, 0:1])
            rs_row = tmp_pool.tile([1, SC], f32r, name="rsrow", bufs=1)
            vec.reciprocal(out=rs_row[:, :], in_=var[:, :])
            return m_row, rs_row

        # =================================================================
        # P0: load x, transpose to feature-major; LN1 stats+apply -> hT bf16
        # =================================================================
        vtmp_pool = tc.alloc_tile_pool(name="vtmp_pool", bufs=3)
        xT_pool = tc.alloc_tile_pool(name="xT_pool", bufs=1)
        xT = xT_pool.tile([128, KH, S], bf16)
        hT_pool = tc.alloc_tile_pool(name="hT_pool", bufs=1, side="right")
        hT = hT_pool.tile([128, KH, S], bf16)
        ln1_tmp = tc.alloc_tile_pool(name="ln1_tmp", bufs=3)
        tok_pool = tc.alloc_tile_pool(name="tok_pool", bufs=3)
        NTOK = S // 128
        for tt in range(NTOK):
            xtok = tok_pool.tile([128, H], bf16, name="xtok")
            sy.dma_start(out=xtok[:, :], in_=x_d[tt * 128:(tt + 1) * 128, :])
            for k in range(KH):
                tp = psum.tile([128, 128], bf16, tag="trp", bufs=2, name="tpb")
                nc.tensor.transpose(tp[:, :], xtok[:, k * 128:(k + 1) * 128],
                                    ident_bf[:, :])
                if k % 2 == 0:
                    vec.tensor_copy(out=xT[:, k, tt * 128:(tt + 1) * 128],
                                    in_=tp[:, :])
                else:
                    act.activation(out=xT[:, k, tt * 128:(tt + 1) * 128],
                                   in_=tp[:, :], func=Act.Copy)
        tok_pool.release()

        fins = []
        for sc in range(NSC):
            ssl = sc_sl(sc)
            s1p = mm_tile(1)
            s2p = mm_tile(1)
            for k in range(KH):
                sq = ln1_tmp.tile([128, SC], bf16, tag="sq", name="sq")
                act.activation(out=sq[:, :], in_=xT[:, k, ssl],
                               func=Act.Square)
                nc.tensor.matmul(s1p[:, :], ones_bf[:, :], xT[:, k, ssl],
                                 start=(k == 0), stop=(k == KH - 1))
                nc.tensor.matmul(s2p[:, :], ones_bf[:, :], sq[:, :],
                                 start=(k == 0), stop=(k == KH - 1))
            fins.append((s1p, s2p))
        rows = [ln_finish(*fins[0], ln1_tmp)]
        for sc in range(NSC):
            ssl = sc_sl(sc)
            m_row, rs_row = rows[sc]
            m1b = ln1_tmp.tile([128, SC], bf16, name="m1b", bufs=1)
            rs1b = ln1_tmp.tile([128, SC], bf16, name="rs1b", bufs=1)
            bc_row(m_row[0:1, :], m1b[:, :])
            bc_row(rs_row[0:1, :], rs1b[:, :])
            if sc + 1 < NSC:
                rows.append(ln_finish(*fins[sc + 1], ln1_tmp))
            for k in range(KH):
                t1 = ln1_tmp.tile([128, SC], bf16, tag="lt", name="t1")
                vec.tensor_sub(out=t1[:, :], in0=xT[:, k, ssl],
                               in1=m1b[:, :])
                vec.tensor_mul(out=hT[:, k, ssl], in0=t1[:, :],
                               in1=rs1b[:, :])
            # level weights: e = exp(h@lvl_w + lvl_b) with 1/z folded in
            lp = mm_tile(D)
            for k in range(KH):
                nc.tensor.matmul(lp[:, :], lvlw_t[:, k, :], hT[:, k, ssl],
                                 start=(k == 0), stop=(k == KH - 1))
            act.activation(out=e_t[:, ssl], in_=lp[:, :], func=Act.Exp,
                           bias=lvlb_t[:, 0:1])
            zp = mm_tile(1)
            nc.tensor.matmul(zp[:, :], ones_bf[0:D, :], e_t[:, ssl],
                             start=True, stop=True)
            vec.reciprocal(out=zr_t[:, ssl], in_=zp[:, :])
            z4 = psum.tile([D, SC], f32, tag="mm", bufs=6, name="z4")
            nc.tensor.matmul(z4[:, :], ones_row[0:1, 0:D], zr_t[0:1, ssl],
                             start=True, stop=True)
            vec.tensor_mul(out=e_t[:, ssl], in0=e_t[:, ssl], in1=z4[:, :])
        ln1_tmp.release()

        # =================================================================
        # P2: v/k/r projections, kv, weighted level term, r gate -> kvT bf16
        # =================================================================
        kvT_pool = tc.alloc_tile_pool(name="kvT_pool", bufs=1)
        kvT = kvT_pool.tile([128, KH, S], bf16)
        wvkr_pool = tc.alloc_tile_pool(name="wvkr_pool", bufs=2)
        for sc in range(NSC):
            ssl = sc_sl(sc)
            for hout in range(KH):
                wvc = wvkr_pool.tile([128, KH, 128], bf16, tag="wv", name="wvc")
                sy.dma_start(out=wvc[:, :, :], in_=wv_d[hout])
                wkc = wvkr_pool.tile([128, KH, 128], bf16, tag="wk", name="wkc")
                sy.dma_start(out=wkc[:, :, :], in_=wk_d[hout])
                wrc = wvkr_pool.tile([128, KH, 128], bf16, tag="wr", name="wrc")
                sy.dma_start(out=wrc[:, :, :], in_=wr_d[hout])
                pv = mm_tile()
                for k in range(KH):
                    nc.tensor.matmul(pv[:, :], wvc[:, k, :], hT[:, k, ssl],
                                     start=(k == 0), stop=(k == KH - 1))
                v_t = vtmp_pool.tile([128, SC], f32, name="v_t")
                vec.tensor_scalar_add(out=v_t[:, :], in0=pv[:, :],
                                      scalar1=bv_t[:, hout:hout + 1])
                pk = mm_tile()
                for k in range(KH):
                    nc.tensor.matmul(pk[:, :], wkc[:, k, :], hT[:, k, ssl],
                                     start=(k == 0), stop=(k == KH - 1))
                # kv = (k + bk) * v
                vec.scalar_tensor_tensor(out=kvT[:, hout, ssl], in0=pk[:, :],
                                         scalar=bk_t[:, hout:hout + 1],
                                         in1=v_t[:, :],
                                         op0=Alu.add, op1=Alu.mult)
                # + level-weighted state term
                hsl = slice(hout * 128, (hout + 1) * 128)
                pw1 = mm_tile()
                nc.tensor.matmul(pw1[:, :], asd_t[:, hsl], e_t[:, ssl],
                                 start=True, stop=True)
                vec.tensor_add(out=kvT[:, hout, ssl], in0=pw1[:, :],
                               in1=kvT[:, hout, ssl])
                # * sigmoid(r)
                pr = mm_tile()
                for k in range(KH):
                    nc.tensor.matmul(pr[:, :], wrc[:, k, :], hT[:, k, ssl],
                                     start=(k == 0), stop=(k == KH - 1))
                r_t = vtmp_pool.tile([128, SC], f32, name="r_t")
                act.activation(out=r_t[:, :], in_=pr[:, :], func=Act.Sigmoid,
                               bias=br_t[:, hout:hout + 1])
                vec.tensor_mul(out=kvT[:, hout, ssl], in0=r_t[:, :],
                               in1=kvT[:, hout, ssl])
        hT_pool.release()
        attc.release()
        wvkr_pool.release()

        # =================================================================
        # P3: att = rw @ Wo; x1 = x + att; LN2 stats (interleaved)
        # P4: LN2 apply + token shift + time-mix -> h2s bf16 (pipelined)
        # =================================================================
        # right stack: x1 (to end) under h2s (to P5 end) under ln2 (to P3 end)
        x1_pool = tc.alloc_tile_pool(name="x1_pool", bufs=1, side="right")
        x1T = x1_pool.tile([128, KH, S], f32r)
        h2_pool = tc.alloc_tile_pool(name="h2_pool", bufs=1, side="right")
        h2s = h2_pool.tile([128, KH, S + 1], bf16)
        ln2_tmp = tc.alloc_tile_pool(name="ln2_tmp", bufs=2, side="right")
        wo_pool = tc.alloc_tile_pool(name="wo_pool", bufs=3)
        # seed the token shift: h2s[:, k, 0] = shift row
        for k in range(KH):
            vec.tensor_copy(out=h2s[:, k, 0:1], in_=sh_t[:, k:k + 1])

        def p3_hout(sc, hout):
            ssl = sc_sl(sc)
            woc = wo_pool.tile([128, KH, 128], bf16, tag="wo", name="woc")
            sy.dma_start(out=woc[:, :, :], in_=wo_d[hout])
            pa = mm_tile()
            for k in range(KH):
                nc.tensor.matmul(pa[:, :], woc[:, k, :], kvT[:, k, ssl],
                                 start=(k == 0), stop=(k == KH - 1))
            vec.tensor_add(out=x1T[:, hout, ssl], in0=pa[:, :],
                           in1=xT[:, hout, ssl])

        def p3_stats(sc):
            ssl = sc_sl(sc)
            s1p = mm_tile(1)
            s2p = mm_tile(1)
            for k in range(KH):
                sq = ln2_tmp.tile([128, SC], bf16, tag="sq", name="sq")
                act.activation(out=sq[:, :], in_=x1T[:, k, ssl],
                               func=Act.Square)
                nc.tensor.matmul(s1p[:, :], ones[:, :], x1T[:, k, ssl],
                                 start=(k == 0), stop=(k == KH - 1))
                nc.tensor.matmul(s2p[:, :], ones_bf[:, :], sq[:, :],
                                 start=(k == 0), stop=(k == KH - 1))
            return ln_finish(s1p, s2p, ln2_tmp)

        def p3_bc(fin):
            m_row, rs_row = fin
            m2b = ln2_tmp.tile([128, SC], bf16, name="m2b", bufs=1)
            rs2b = ln2_tmp.tile([128, SC], bf16, name="rs2b", bufs=1)
            bc_row(m_row[0:1, :], m2b[:, :])
            bc_row(rs_row[0:1, :], rs2b[:, :])
            return m2b, rs2b

        def p4_k(sc, m2b, rs2b, k):
                t1 = ln2_tmp.tile([128, SC], bf16, tag="lt", name="t1")
                vec.tensor_sub(out=t1[:, :], in0=x1T[:, k, ssl2(sc)],
                               in1=m2b[:, :])
                vec.tensor_mul(out=h2s[:, k, 1 + sc * SC:1 + (sc + 1) * SC],
                               in0=t1[:, :], in1=rs2b[:, :])
                d_t = ln2_tmp.tile([128, SC], bf16, tag="dt", name="d_t")
                vec.tensor_sub(out=d_t[:, :],
                               in0=h2s[:, k, 1 + sc * SC:1 + (sc + 1) * SC],
                               in1=h2s[:, k, sc * SC:(sc + 1) * SC])
                vec.scalar_tensor_tensor(
                    out=h2s[:, k, sc * SC:(sc + 1) * SC], in0=d_t[:, :],
                    scalar=tmk_t[:, k:k + 1],
                    in1=h2s[:, k, sc * SC:(sc + 1) * SC],
                    op0=Alu.mult, op1=Alu.add)

        def p4(sc, m2b, rs2b):
            for k in range(KH):
                p4_k(sc, m2b, rs2b, k)

        def ssl2(sc):
            return sc_sl(sc)

        # staged emission: stats de-interleaved; broadcasts tucked behind the
        # next chunk's matmuls; P4 vector work shadowed by pa/P5 matmuls
        for hout in range(KH):
            p3_hout(0, hout)
        fin0 = p3_stats(0)
        p3_hout(1, 0)
        p3_hout(1, 1)
        mb0 = p3_bc(fin0)
        for i, hout in enumerate(range(2, KH)):
            p3_hout(1, hout)
            p4_k(0, *mb0, i)
        p4_k(0, *mb0, KH - 2)
        p4_k(0, *mb0, KH - 1)
        fin1 = p3_stats(1)
        wo_pool.release()
        kvT_pool.release()
        xT_pool.release()

        # =================================================================
        # P5: kk = relu(sqrt(s_kk)*(km @ Wkey' + bkey))^2 -> fp8 (SBUF)
        # P6: out = x1 + (kk@Wval)*sigmoid(kk@Wgate); transpose; store
        # =================================================================
        kk_pool = tc.alloc_tile_pool(name="kk_pool", bufs=1)
        kkw_pool = tc.alloc_tile_pool(name="kkw_pool", bufs=3)
        wvg_pool = fin_pool = ot_pool = None
        sqrt_skk = float(np.sqrt(S_KK)) if VG_FP8 else 1.0

        def p5_group(sc, ff, kk):
            wyc = kkw_pool.tile([128, KH, 128], bf16, name="wyc")
            sy.dma_start(out=wyc[:, :, :], in_=wkey_d[ff])
            pkk = mm_tile()
            for k in range(KH):
                nc.tensor.matmul(pkk[:, :], wyc[:, k, :],
                                 h2s[:, k, sc * SC:(sc + 1) * SC],
                                 start=(k == 0), stop=(k == KH - 1))
            u_t = vtmp_pool.tile([128, SC], bf16, name="u_t")
            act.activation(out=u_t[:, :], in_=pkk[:, :], func=Act.Relu,
                           bias=bkey_t[:, ff:ff + 1], scale=sqrt_skk)
            vec.tensor_mul(out=kk[:, ff, :], in0=u_t[:, :], in1=u_t[:, :])

        def p6(sc, kk):
            for hout in range(KH):
                wv8 = wvg_pool.tile([128, KF, 128], vg_dt, tag="wv8",
                                    name="wv8")
                sy.dma_start(out=wv8[:, :, :], in_=wval_d[hout])
                wg8 = wvg_pool.tile([128, KF, 128], vg_dt, tag="wg8",
                                    name="wg8")
                sy.dma_start(out=wg8[:, :, :], in_=wgate_d[hout])
                for dc in range(NDC):
                    dsl = slice(dc * DC, (dc + 1) * DC)
                    xsl = slice(sc * SC + dc * DC, sc * SC + (dc + 1) * DC)
                    psv = psum.tile([128, DC], f32, tag="mm", bufs=6,
                                    name="psv")
                    psg = psum.tile([128, DC], f32, tag="mm", bufs=6,
                                    name="psg")
                    if VG_FP8:
                        for f in range(KF // 2):
                            nc.tensor.matmul(psv[:, :],
                                             wv8[:, 2 * f:2 * f + 2, :],
                                             kk[:, 2 * f:2 * f + 2, dsl],
                                             start=(f == 0),
                                             stop=(f == KF // 2 - 1),
                                             perf_mode=DR)
                        for f in range(KF // 2):
                            nc.tensor.matmul(psg[:, :],
                                             wg8[:, 2 * f:2 * f + 2, :],
                                             kk[:, 2 * f:2 * f + 2, dsl],
                                             start=(f == 0),
                                             stop=(f == KF // 2 - 1),
                                             perf_mode=DR)
                    else:
                        for f in range(KF):
                            nc.tensor.matmul(psv[:, :], wv8[:, f, :],
                                             kk[:, f, dsl],
                                             start=(f == 0),
                                             stop=(f == KF - 1))
                        for f in range(KF):
                            nc.tensor.matmul(psg[:, :], wg8[:, f, :],
                                             kk[:, f, dsl],
                                             start=(f == 0),
                                             stop=(f == KF - 1))
                    sig_t = fin_pool.tile([128, DC], f32, name="sig_t")
                    act.activation(out=sig_t[:, :], in_=psg[:, :],
                                   func=Act.Sigmoid, scale=scl_t[:, 1:2])
                    glu_t = fin_pool.tile([128, DC], f32, name="glu_t")
                    vec.tensor_scalar_mul(out=glu_t[:, :], in0=psv[:, :],
                                          scalar1=scl_t[:, 0:1])
                    vec.tensor_mul(out=glu_t[:, :], in0=glu_t[:, :],
                                   in1=sig_t[:, :])
                    vec.tensor_add(out=glu_t[:, :], in0=glu_t[:, :],
                                   in1=x1T[:, hout, xsl])
                    for j in range(DC // 128):
                        tp = trp_tile()
                        nc.tensor.transpose(tp[:, :],
                                            glu_t[:, j * 128:(j + 1) * 128],
                                            ident[:, :])
                        ot = ot_pool.tile([128, 128], f32, name="ot")
                        vec.tensor_copy(out=ot[:, :], in_=tp[:, :])
                        tt = (sc * SC + dc * DC) // 128 + j
                        sy.dma_start(
                            out=out_d[tt * 128:(tt + 1) * 128,
                                      hout * 128:(hout + 1) * 128],
                            in_=ot[:, :])

        kk0 = kk_pool.tile([128, KF, SC], vg_dt, tag="kk", name="kk")
        for ff in range(4):
            p5_group(0, ff, kk0)
        mb1 = p3_bc(fin1)
        p4(1, *mb1)
        for ff in range(4, KF):
            p5_group(0, ff, kk0)
        ln2_tmp.release()
        wvg_pool = tc.alloc_tile_pool(name="wvg_pool", bufs=2)
        fin_pool = tc.alloc_tile_pool(name="fin_pool", bufs=4)
        ot_pool = tc.alloc_tile_pool(name="ot_pool", bufs=4)
        p6(0, kk0)
        kk1 = kk_pool.tile([128, KF, SC], vg_dt, tag="kk", name="kk")
        for ff in range(KF):
            p5_group(1, ff, kk1)
        p6(1, kk1)
        ot_pool.release()
        fin_pool.release()
        wvg_pool.release()
        kkw_pool.release()
        kk_pool.release()
        h2_pool.release()
        x1_pool.release()
        vtmp_pool.release()
        consts.release()
        psum.release()
    nc.finalize()
    return nc


# ---------------------------------------------------------------------------
# host side
# ---------------------------------------------------------------------------

def _ln_np(x, s, b):
    m = x.mean(-1, keepdims=True)
    vv = ((x - m) ** 2).mean(-1, keepdims=True)
    return (x - m) / np.sqrt(vv + 1e-5) * s + b


def _h2hat_row(xrow, att_state_b, ln1_s, ln1_b, ln2_s, ln2_b, td, lvl_w,
               lvl_b, Wv, Wk, Wr, Wo):
    """(x1 - m)/std for a single token row (LN2 without scale/bias)."""
    h = _ln_np(xrow[None, :], ln1_s, ln1_b)[0]
    vv = h @ Wv
    kk = h @ Wk
    rr = 1.0 / (1.0 + np.exp(-(h @ Wr)))
    lg = h @ lvl_w + lvl_b
    e = np.exp(lg - lg.max())
    lw = e / e.sum()
    decay = np.exp(-np.exp(td))
    weighted = (lw[None, :] @ (att_state_b * decay))[0] + kk * vv
    att = (rr * weighted) @ Wo
    x1 = xrow + att
    m = x1.mean()
    sd = np.sqrt(((x1 - m) ** 2).mean() + 1e-5)
    return ((x1 - m) / sd).astype(np.float32)


def _arrange_hkh(W):
    """[H, H] -> [KH, 128, KH, 128]: arr[ho, p, k, m] = W[k*128+p, ho*128+m]"""
    Wr = W.reshape(KH, 128, -1, 128)            # [k, p, ho, m]
    return np.ascontiguousarray(Wr.transpose(2, 1, 0, 3))


def _arrange_cols(v):
    """[H] -> [128, KH]: arr[p, k] = v[k*128+p]"""
    return np.ascontiguousarray(v.reshape(-1, 128).T)


_BUILT = None


def _get_built():
    global _BUILT
    if _BUILT is None:
        _BUILT = build_bass()
    return _BUILT


def make_in_maps(x, att_state, cm_state, ln1_s, ln1_b, ln2_s, ln2_b,
                 td_multi, lvl_w, lvl_b, Wv, Wk, Wr, Wo, tmk,
                 Wkey, Wval, Wgate):
    f = np.float32
    x = np.asarray(x, f)
    att_state = np.asarray(att_state, f)
    cm_state = np.asarray(cm_state, f)
    ln1_s, ln1_b = np.asarray(ln1_s, f), np.asarray(ln1_b, f)
    ln2_s, ln2_b = np.asarray(ln2_s, f), np.asarray(ln2_b, f)
    td = np.asarray(td_multi, f)
    lvl_w, lvl_b = np.asarray(lvl_w, f), np.asarray(lvl_b, f)
    Wv, Wk, Wr, Wo = (np.asarray(a, f) for a in (Wv, Wk, Wr, Wo))
    tmk = np.asarray(tmk, f)
    Wkey, Wval, Wgate = (np.asarray(a, f) for a in (Wkey, Wval, Wgate))

    # fold LN1 scale into Wv/Wk/Wr/lvl_w; LN1 bias becomes output biases
    decay = np.exp(-np.exp(td))
    sqrt_skk = np.sqrt(S_KK) if VG_FP8 else 1.0
    if VG_FP8:
        s_wv = 224.0 / max(np.abs(Wval).max(), 1e-9)
        s_wg = 224.0 / max(np.abs(Wgate).max(), 1e-9)
        wval_a = np.ascontiguousarray(
            _arrange_khf(np.clip(Wval * s_wv, -240, 240)).astype(E4NP))
        wgate_a = np.ascontiguousarray(
            _arrange_khf(np.clip(Wgate * s_wg, -240, 240)).astype(E4NP))
        scl = np.tile(np.array([1.0 / (S_KK * s_wv),
                                1.0 / (S_KK * s_wg)], f), (128, 1))
    else:
        wval_a = np.ascontiguousarray(_arrange_khf(Wval).astype(BFNP))
        wgate_a = np.ascontiguousarray(_arrange_khf(Wgate).astype(BFNP))
        scl = np.tile(np.array([1.0, 1.0], f), (128, 1))

    shared = {
        "lvl_w": np.ascontiguousarray(
            (ln1_s[:, None] * lvl_w).reshape(KH, 128, D)
            .transpose(1, 0, 2)).astype(BFNP),
        "lvl_b": lvl_b + ln1_b @ lvl_w,
        "tmk": _arrange_cols(tmk),
        "bv": _arrange_cols(ln1_b @ Wv),
        "bk": _arrange_cols(ln1_b @ Wk),
        "br": _arrange_cols(ln1_b @ Wr),
        "bkey": np.ascontiguousarray(
            ((ln2_b @ Wkey) * sqrt_skk).reshape(KF, 128).T),
        "Wv": _arrange_hkh(ln1_s[:, None] * Wv).astype(BFNP),
        "Wk": _arrange_hkh(ln1_s[:, None] * Wk).astype(BFNP),
        "Wr": _arrange_hkh(ln1_s[:, None] * Wr).astype(BFNP),
        "Wo": _arrange_hkh(Wo).astype(BFNP),
        "Wkey": _arrange_khf_key(ln2_s[:, None] * Wkey).astype(BFNP),
        "Wval": wval_a,
        "Wgate": wgate_a,
        "scl": scl,
    }
    shared = {k: np.ascontiguousarray(v) for k, v in shared.items()}

    in_maps = []
    for c in range(NCORES):
        b, piece = c // 2, c % 2
        t0 = piece * S
        if piece == 0:
            shift = (cm_state[b] - ln2_b) / ln2_s
        else:
            shift = _h2hat_row(x[b, t0 - 1], att_state[b], ln1_s, ln1_b,
                               ln2_s, ln2_b, td, lvl_w, lvl_b, Wv, Wk, Wr, Wo)
        in_maps.append({
            "xbf": np.ascontiguousarray(x[b, t0:t0 + S].astype(BFNP)),
            "shift_in": np.ascontiguousarray(
                shift.reshape(KH, 128).T.astype(BFNP)),
            "asd": np.ascontiguousarray((att_state[b] * decay).astype(BFNP)),
            **shared,
        })
    return in_maps


def _arrange_khf(W):
    """[FF, H] -> [KH, 128, KF, 128]: arr[ho, p, f, m] = W[f*128+p, ho*128+m]"""
    Wr = W.reshape(KF, 128, KH, 128)            # [f, p, ho, m]
    return np.ascontiguousarray(Wr.transpose(2, 1, 0, 3))


def _arrange_khf_key(W):
    """[H, FF] -> [KF, 128, KH, 128]: arr[fo, p, k, m] = W[k*128+p, fo*128+m]"""
    Wr = W.reshape(KH, 128, KF, 128)            # [k, p, fo, m]
    return np.ascontiguousarray(Wr.transpose(2, 1, 0, 3))


def kernel(x, att_state, cm_state, ln1_s, ln1_b, ln2_s, ln2_b,
           td_multi, lvl_w, lvl_b, Wv, Wk, Wr, Wo, tmk,
           Wkey, Wval, Wgate):
    from concourse.bass_utils import run_bass_kernel_spmd

    in_maps = make_in_maps(x, att_state, cm_state, ln1_s, ln1_b, ln2_s,
                           ln2_b, td_multi, lvl_w, lvl_b, Wv, Wk, Wr, Wo,
                           tmk, Wkey, Wval, Wgate)
    nc = _get_built()
    res = run_bass_kernel_spmd(nc, in_maps, list(range(NCORES)))
    out = np.empty((B, T, H), np.float32)
    for c in range(NCORES):
        b, piece = c // 2, c % 2
        out[b, piece * S:(piece + 1) * S] = res.results[c]["out"]
    return out


# revision 19
# speedup vs baseline: 1.0935x; 1.0346x over previous
"""EnhancedRWKVBlock Trainium2 kernel (optimized).

Sharding: 8 cores = 4 batches x 2 sequence halves (pure data parallel).
The channel-mix token shift across the halves is seeded by one host-computed
row per odd shard.

Key optimizations over the f32r baseline:
  - All big matmuls run in bf16 (same PE rate as f32r, half the LDWEIGHTS
    time, half the weight DMA); the val/gate matmuls run in fp8e4 DoubleRow
    (2x PE rate) with host-quantized weights.
  - Weights are pre-arranged on host into [tile, 128, k, 128] layouts so
    every weight DMA is contiguous per partition (the baseline was DMA
    descriptor-bound with 2048 x 256B scatters per tile).
  - LN scale/bias are folded into the following projection weights on host;
    the level-mix 1/z is folded into e_t once instead of per (hout, sc).
  - No DRAM spills: xT/x1T/kk stay SBUF-resident.
  - sc-major phase ordering pipelines the vector-only LN2/token-shift block
    behind matmul phases, removing the PE bubble.
"""

import numpy as np
import ml_dtypes

B, T, H, D, FF = 4, 2048, 2048, 4, 8192
NCORES = 8
KH = H // 128            # 16 feature tiles of H
KF = FF // 128           # 64 feature tiles of FF
S = T // 2               # tokens per core
SC = 512                 # token chunk for bf16 matmuls
NSC = S // SC
DC = 256                 # token chunk for fp8 DoubleRow matmuls
NDC = SC // DC

VG_FP8 = True            # val/gate matmuls in fp8e4 DoubleRow
S_KK = 2.0               # fp8 scale for kk = relu(.)^2  (max kk ~27 << 120)

E4NP = ml_dtypes.float8_e4m3
BFNP = ml_dtypes.bfloat16


# ---------------------------------------------------------------------------
# device kernel
# ---------------------------------------------------------------------------

def build_bass():
    import concourse.bass as bass
    from concourse import bacc
    import concourse.mybir as mybir
    import concourse.tile as tile
    from concourse.masks import make_identity

    f32 = mybir.dt.float32
    f32r = mybir.dt.float32r
    bf16 = mybir.dt.bfloat16
    fp8 = mybir.dt.float8e4
    Alu = mybir.AluOpType
    Act = mybir.ActivationFunctionType
    DR = mybir.MatmulPerfMode.DoubleRow

    inv_h = 1.0 / H
    vg_dt = fp8 if VG_FP8 else bf16

    nc = bacc.Bacc()

    # --- external I/O (per core) ---
    x_d = nc.dram_tensor("xbf", [S, H], bf16, kind="ExternalInput")
    sh_d = nc.dram_tensor("shift_in", [128, KH], bf16, kind="ExternalInput")
    asd_d = nc.dram_tensor("asd", [D, H], bf16, kind="ExternalInput")
    lvlw_d = nc.dram_tensor("lvl_w", [128, KH, D], bf16, kind="ExternalInput")
    lvlb_d = nc.dram_tensor("lvl_b", [D], f32, kind="ExternalInput")
    tmk_d = nc.dram_tensor("tmk", [128, KH], f32, kind="ExternalInput")
    bv_d = nc.dram_tensor("bv", [128, KH], f32, kind="ExternalInput")
    bk_d = nc.dram_tensor("bk", [128, KH], f32, kind="ExternalInput")
    br_d = nc.dram_tensor("br", [128, KH], f32, kind="ExternalInput")
    bkey_d = nc.dram_tensor("bkey", [128, KF], f32, kind="ExternalInput")
    wv_d = nc.dram_tensor("Wv", [KH, 128, KH, 128], bf16, kind="ExternalInput")
    wk_d = nc.dram_tensor("Wk", [KH, 128, KH, 128], bf16, kind="ExternalInput")
    wr_d = nc.dram_tensor("Wr", [KH, 128, KH, 128], fp8, kind="ExternalInput")
    wo_d = nc.dram_tensor("Wo", [KH, 128, KH, 128], bf16, kind="ExternalInput")
    wkey_d = nc.dram_tensor("Wkey", [KF, 128, KH, 128], bf16,
                            kind="ExternalInput")
    wval_d = nc.dram_tensor("Wval", [KH, 128, KF, 128], vg_dt,
                            kind="ExternalInput")
    wgate_d = nc.dram_tensor("Wgate", [KH, 128, KF, 128], vg_dt,
                             kind="ExternalInput")
    scl_d = nc.dram_tensor("scl", [128, 3], f32, kind="ExternalInput")
    out_d = nc.dram_tensor("out", [S, H], f32, kind="ExternalOutput")

    vec = nc.vector
    act = nc.scalar
    sy = nc.sync

    def sc_sl(sc):
        return slice(sc * SC, (sc + 1) * SC)

    with tile.TileContext(nc) as tc, \
            nc.allow_low_precision(reason="bf16/fp8 matmuls within rel-err budget"):
        # ---- persistent constants ----
        consts = tc.alloc_tile_pool(name="consts", bufs=1)
        ident = consts.tile([128, 128], f32)
        make_identity(nc, ident)
        ident_bf = consts.tile([128, 128], bf16)
        vec.tensor_copy(out=ident_bf[:, :], in_=ident[:, :])
        ones_f = consts.tile([128, 1], f32)
        vec.memset(ones_f[:, :], 1.0)
        ones = consts.tile([128, 1], f32r)
        vec.tensor_copy(out=ones[:, :], in_=ones_f[:, :])
        ones_bf = consts.tile([128, 1], bf16)
        vec.tensor_copy(out=ones_bf[:, :], in_=ones_f[:, :])
        ones_row_f = consts.tile([1, 128], f32)
        vec.memset(ones_row_f[:, :], 1.0)
        ones_row = consts.tile([1, 128], f32r)
        vec.tensor_copy(out=ones_row[:, :], in_=ones_row_f[:, :])
        eps_t = consts.tile([1, 1], f32)
        vec.memset(eps_t[:, :], 1e-5)
        tmk_t = consts.tile([128, KH], f32)
        sy.dma_start(out=tmk_t[:, :], in_=tmk_d[:, :])
        bv_t = consts.tile([128, KH], f32)
        sy.dma_start(out=bv_t[:, :], in_=bv_d[:, :])
        bk_t = consts.tile([128, KH], f32)
        sy.dma_start(out=bk_t[:, :], in_=bk_d[:, :])
        br_t = consts.tile([128, KH], f32)
        sy.dma_start(out=br_t[:, :], in_=br_d[:, :])
        bkey_t = consts.tile([128, KF], f32)
        sy.dma_start(out=bkey_t[:, :], in_=bkey_d[:, :])
        sh_t = consts.tile([128, KH], bf16)
        sy.dma_start(out=sh_t[:, :], in_=sh_d[:, :])
        scl_t = consts.tile([128, 3], f32)
        sy.dma_start(out=scl_t[:, :], in_=scl_d[:, :])

        # ---- attention-scoped constants ----
        attc = tc.alloc_tile_pool(name="attc", bufs=1, side="right")
        lvlw_t = attc.tile([128, KH, D], bf16)
        sy.dma_start(out=lvlw_t[:, :, :], in_=lvlw_d[:, :, :])
        lvlb_t = attc.tile([D, 1], f32)
        sy.dma_start(out=lvlb_t[:, :], in_=lvlb_d[:])
        asd_t = attc.tile([D, H], bf16)   # att_state * decay (host)
        sy.dma_start(out=asd_t[:, :], in_=asd_d[:, :])
        e_t = attc.tile([D, S], bf16)     # softmax-normalized level weights
        zr_t = attc.tile([1, S], f32r)

        # ---- shared PSUM pool ----
        psum = tc.alloc_tile_pool(name="psum", bufs=1, space="PSUM")

        def mm_tile(p0=128, w=SC):
            return psum.tile([p0, w], f32, tag="mm", bufs=6, name="pt")

        def trp_tile():
            return psum.tile([128, 128], f32, tag="trp", bufs=2, name="tp")

        def bc_row(row_ap, dst_slice, w=SC):
            # broadcast a [1, w] f32r row across 128 partitions (K=1 matmul)
            pb = psum.tile([128, w], f32, tag="mm", bufs=6, name="pb")
            nc.tensor.matmul(pb[:, :], ones_row[:, :], row_ap,
                             start=True, stop=True)
            vec.tensor_copy(out=dst_slice, in_=pb[:, :])

        def ln_finish(s1p, s2p, tmp_pool):
            m_row = tmp_pool.tile([1, SC], f32r, name="mrow", bufs=1)
            vec.tensor_scalar_mul(out=m_row[:, :], in0=s1p[:, :],
                                  scalar1=inv_h)
            msq = tmp_pool.tile([1, SC], f32, name="msq", bufs=1)
            vec.tensor_mul(out=msq[:, :], in0=m_row[:, :], in1=m_row[:, :])
            var = tmp_pool.tile([1, SC], f32, name="var", bufs=1)
            vec.scalar_tensor_tensor(out=var[:, :], in0=s2p[:, :],
                                     scalar=inv_h, in1=msq[:, :],
                                     op0=Alu.mult, op1=Alu.subtract)
            act.activation(out=var[:, :], in_=var[:, :], func=Act.Sqrt,
                           bias=eps_t[:, 0:1])
            rs_row = tmp_pool.tile([1, SC], f32r, name="rsrow", bufs=1)
            vec.reciprocal(out=rs_row[:, :], in_=var[:, :])
            return m_row, rs_row

        # =================================================================
        # P0: load x, transpose to feature-major; LN1 stats+apply -> hT bf16
        # =================================================================
        vtmp_pool = tc.alloc_tile_pool(name="vtmp_pool", bufs=3)
        xT_pool = tc.alloc_tile_pool(name="xT_pool", bufs=1)
        xT = xT_pool.tile([128, KH, S], bf16)
        hT_pool = tc.alloc_tile_pool(name="hT_pool", bufs=1, side="right")
        hT = hT_pool.tile([128, KH, S], bf16)
        h8_pool = tc.alloc_tile_pool(name="h8_pool", bufs=1, side="right")
        h8 = h8_pool.tile([128, KH, S], fp8)
        ln1_tmp = tc.alloc_tile_pool(name="ln1_tmp", bufs=3)
        tok_pool = tc.alloc_tile_pool(name="tok_pool", bufs=3)
        NTOK = S // 128
        for tt in range(NTOK):
            xtok = tok_pool.tile([128, H], bf16, name="xtok")
            sy.dma_start(out=xtok[:, :], in_=x_d[tt * 128:(tt + 1) * 128, :])
            for k in range(KH):
                tp = psum.tile([128, 128], bf16, tag="trp", bufs=2, name="tpb")
                nc.tensor.transpose(tp[:, :], xtok[:, k * 128:(k + 1) * 128],
                                    ident_bf[:, :])
                if k % 2 == 0:
                    vec.tensor_copy(out=xT[:, k, tt * 128:(tt + 1) * 128],
                                    in_=tp[:, :])
                else:
                    act.activation(out=xT[:, k, tt * 128:(tt + 1) * 128],
                                   in_=tp[:, :], func=Act.Copy)
        tok_pool.release()

        fins = []
        for sc in range(NSC):
            ssl = sc_sl(sc)
            s1p = mm_tile(1)
            s2p = mm_tile(1)
            for k in range(KH):
                sq = ln1_tmp.tile([128, SC], bf16, tag="sq", name="sq")
                act.activation(out=sq[:, :], in_=xT[:, k, ssl],
                               func=Act.Square)
                nc.tensor.matmul(s1p[:, :], ones_bf[:, :], xT[:, k, ssl],
                                 start=(k == 0), stop=(k == KH - 1))
                nc.tensor.matmul(s2p[:, :], ones_bf[:, :], sq[:, :],
                                 start=(k == 0), stop=(k == KH - 1))
            fins.append((s1p, s2p))
        rows = [ln_finish(*fins[0], ln1_tmp)]
        for sc in range(NSC):
            ssl = sc_sl(sc)
            m_row, rs_row = rows[sc]
            m1b = ln1_tmp.tile([128, SC], bf16, name="m1b", bufs=1)
            rs1b = ln1_tmp.tile([128, SC], bf16, name="rs1b", bufs=1)
            bc_row(m_row[0:1, :], m1b[:, :])
            bc_row(rs_row[0:1, :], rs1b[:, :])
            if sc + 1 < NSC:
                rows.append(ln_finish(*fins[sc + 1], ln1_tmp))
            for k in range(KH):
                t1 = ln1_tmp.tile([128, SC], bf16, tag="lt", name="t1")
                vec.tensor_sub(out=t1[:, :], in0=xT[:, k, ssl],
                               in1=m1b[:, :])
                vec.tensor_mul(out=hT[:, k, ssl], in0=t1[:, :],
                               in1=rs1b[:, :])
            for k in range(KH):
                vec.tensor_scalar_mul(out=h8[:, k, ssl], in0=hT[:, k, ssl],
                                      scalar1=16.0)
            # level weights: e = exp(h@lvl_w + lvl_b) with 1/z folded in
            lp = mm_tile(D)
            for k in range(KH):
                nc.tensor.matmul(lp[:, :], lvlw_t[:, k, :], hT[:, k, ssl],
                                 start=(k == 0), stop=(k == KH - 1))
            act.activation(out=e_t[:, ssl], in_=lp[:, :], func=Act.Exp,
                           bias=lvlb_t[:, 0:1])
            zp = mm_tile(1)
            nc.tensor.matmul(zp[:, :], ones_bf[0:D, :], e_t[:, ssl],
                             start=True, stop=True)
            vec.reciprocal(out=zr_t[:, ssl], in_=zp[:, :])
            z4 = psum.tile([D, SC], f32, tag="mm", bufs=6, name="z4")
            nc.tensor.matmul(z4[:, :], ones_row[0:1, 0:D], zr_t[0:1, ssl],
                             start=True, stop=True)
            vec.tensor_mul(out=e_t[:, ssl], in0=e_t[:, ssl], in1=z4[:, :])
        ln1_tmp.release()

        # =================================================================
        # P2: v/k/r projections, kv, weighted level term, r gate -> kvT bf16
        # =================================================================
        kvT_pool = tc.alloc_tile_pool(name="kvT_pool", bufs=1)
        kvT = kvT_pool.tile([128, KH, S], bf16)
        wvkr_pool = tc.alloc_tile_pool(name="wvkr_pool", bufs=2)
        for sc in range(NSC):
            ssl = sc_sl(sc)
            for hout in range(KH):
                wvc = wvkr_pool.tile([128, KH, 128], bf16, tag="wv", name="wvc")
                sy.dma_start(out=wvc[:, :, :], in_=wv_d[hout])
                wkc = wvkr_pool.tile([128, KH, 128], bf16, tag="wk", name="wkc")
                sy.dma_start(out=wkc[:, :, :], in_=wk_d[hout])
                wrc = wvkr_pool.tile([128, KH, 128], fp8, tag="wr", name="wrc")
                sy.dma_start(out=wrc[:, :, :], in_=wr_d[hout])
                pv = mm_tile()
                for k in range(KH):
                    nc.tensor.matmul(pv[:, :], wvc[:, k, :], hT[:, k, ssl],
                                     start=(k == 0), stop=(k == KH - 1))
                v_t = vtmp_pool.tile([128, SC], f32, name="v_t")
                vec.tensor_scalar_add(out=v_t[:, :], in0=pv[:, :],
                                      scalar1=bv_t[:, hout:hout + 1])
                pk = mm_tile()
                for k in range(KH):
                    nc.tensor.matmul(pk[:, :], wkc[:, k, :], hT[:, k, ssl],
                                     start=(k == 0), stop=(k == KH - 1))
                # kv = (k + bk) * v
                vec.scalar_tensor_tensor(out=kvT[:, hout, ssl], in0=pk[:, :],
                                         scalar=bk_t[:, hout:hout + 1],
                                         in1=v_t[:, :],
                                         op0=Alu.add, op1=Alu.mult)
                # + level-weighted state term
                hsl = slice(hout * 128, (hout + 1) * 128)
                pw1 = mm_tile()
                nc.tensor.matmul(pw1[:, :], asd_t[:, hsl], e_t[:, ssl],
                                 start=True, stop=True)
                vec.tensor_add(out=kvT[:, hout, ssl], in0=pw1[:, :],
                               in1=kvT[:, hout, ssl])
                # * sigmoid(r)  (fp8 DoubleRow)
                r_t = vtmp_pool.tile([128, SC], f32, name="r_t")
                for dc2 in range(NDC):
                    psr = psum.tile([128, DC], f32, tag="mm", bufs=6,
                                    name="psr")
                    lo = sc * SC + dc2 * DC
                    for kp in range(KH // 2):
                        nc.tensor.matmul(psr[:, :],
                                         wrc[:, 2 * kp:2 * kp + 2, :],
                                         h8[:, 2 * kp:2 * kp + 2, lo:lo + DC],
                                         start=(kp == 0),
                                         stop=(kp == KH // 2 - 1),
                                         perf_mode=DR)
                    act.activation(out=r_t[:, dc2 * DC:(dc2 + 1) * DC],
                                   in_=psr[:, :], func=Act.Sigmoid,
                                   bias=br_t[:, hout:hout + 1],
                                   scale=scl_t[:, 2:3])
                vec.tensor_mul(out=kvT[:, hout, ssl], in0=r_t[:, :],
                               in1=kvT[:, hout, ssl])
        h8_pool.release()
        hT_pool.release()
        attc.release()
        wvkr_pool.release()

        # =================================================================
        # P3: att = rw @ Wo; x1 = x + att; LN2 stats (interleaved)
        # P4: LN2 apply + token shift + time-mix -> h2s bf16 (pipelined)
        # =================================================================
        # right stack: x1 (to end) under h2s (to P5 end) under ln2 (to P3 end)
        x1_pool = tc.alloc_tile_pool(name="x1_pool", bufs=1, side="right")
        x1T = x1_pool.tile([128, KH, S], f32r)
        h2_pool = tc.alloc_tile_pool(name="h2_pool", bufs=1, side="right")
        h2s = h2_pool.tile([128, KH, S + 1], bf16)
        ln2_tmp = tc.alloc_tile_pool(name="ln2_tmp", bufs=2, side="right")
        wo_pool = tc.alloc_tile_pool(name="wo_pool", bufs=3)
        # seed the token shift: h2s[:, k, 0] = shift row
        for k in range(KH):
            vec.tensor_copy(out=h2s[:, k, 0:1], in_=sh_t[:, k:k + 1])

        def p3_hout(sc, hout):
            ssl = sc_sl(sc)
            woc = wo_pool.tile([128, KH, 128], bf16, tag="wo", name="woc")
            sy.dma_start(out=woc[:, :, :], in_=wo_d[hout])
            pa = mm_tile()
            for k in range(KH):
                nc.tensor.matmul(pa[:, :], woc[:, k, :], kvT[:, k, ssl],
                                 start=(k == 0), stop=(k == KH - 1))
            vec.tensor_add(out=x1T[:, hout, ssl], in0=pa[:, :],
                           in1=xT[:, hout, ssl])

        def p3_stats(sc):
            ssl = sc_sl(sc)
            s1p = mm_tile(1)
            s2p = mm_tile(1)
            for k in range(KH):
                sq = ln2_tmp.tile([128, SC], bf16, tag="sq", name="sq")
                act.activation(out=sq[:, :], in_=x1T[:, k, ssl],
                               func=Act.Square)
                nc.tensor.matmul(s1p[:, :], ones[:, :], x1T[:, k, ssl],
                                 start=(k == 0), stop=(k == KH - 1))
                nc.tensor.matmul(s2p[:, :], ones_bf[:, :], sq[:, :],
                                 start=(k == 0), stop=(k == KH - 1))
            return ln_finish(s1p, s2p, ln2_tmp)

        def p3_bc(fin):
            m_row, rs_row = fin
            m2b = ln2_tmp.tile([128, SC], bf16, name="m2b", bufs=1)
            rs2b = ln2_tmp.tile([128, SC], bf16, name="rs2b", bufs=1)
            bc_row(m_row[0:1, :], m2b[:, :])
            bc_row(rs_row[0:1, :], rs2b[:, :])
            return m2b, rs2b

        def p4_k(sc, m2b, rs2b, k):
                t1 = ln2_tmp.tile([128, SC], bf16, tag="lt", name="t1")
                vec.tensor_sub(out=t1[:, :], in0=x1T[:, k, ssl2(sc)],
                               in1=m2b[:, :])
                vec.tensor_mul(out=h2s[:, k, 1 + sc * SC:1 + (sc + 1) * SC],
                               in0=t1[:, :], in1=rs2b[:, :])
                d_t = ln2_tmp.tile([128, SC], bf16, tag="dt", name="d_t")
                vec.tensor_sub(out=d_t[:, :],
                               in0=h2s[:, k, 1 + sc * SC:1 + (sc + 1) * SC],
                               in1=h2s[:, k, sc * SC:(sc + 1) * SC])
                vec.scalar_tensor_tensor(
                    out=h2s[:, k, sc * SC:(sc + 1) * SC], in0=d_t[:, :],
                    scalar=tmk_t[:, k:k + 1],
                    in1=h2s[:, k, sc * SC:(sc + 1) * SC],
                    op0=Alu.mult, op1=Alu.add)

        def p4(sc, m2b, rs2b):
            for k in range(KH):
                p4_k(sc, m2b, rs2b, k)

        def ssl2(sc):
            return sc_sl(sc)

        # staged emission: stats de-interleaved; broadcasts tucked behind the
        # next chunk's matmuls; P4 vector work shadowed by pa/P5 matmuls
        for hout in range(KH):
            p3_hout(0, hout)
        fin0 = p3_stats(0)
        p3_hout(1, 0)
        p3_hout(1, 1)
        mb0 = p3_bc(fin0)
        for i, hout in enumerate(range(2, KH)):
            p3_hout(1, hout)
            p4_k(0, *mb0, i)
        p4_k(0, *mb0, KH - 2)
        p4_k(0, *mb0, KH - 1)
        fin1 = p3_stats(1)
        wo_pool.release()
        kvT_pool.release()
        xT_pool.release()

        # =================================================================
        # P5: kk = relu(sqrt(s_kk)*(km @ Wkey' + bkey))^2 -> fp8 (SBUF)
        # P6: out = x1 + (kk@Wval)*sigmoid(kk@Wgate); transpose; store
        # =================================================================
        kk_pool = tc.alloc_tile_pool(name="kk_pool", bufs=1)
        kkw_pool = tc.alloc_tile_pool(name="kkw_pool", bufs=3)
        wvg_pool = fin_pool = ot_pool = None
        sqrt_skk = float(np.sqrt(S_KK)) if VG_FP8 else 1.0

        def p5_group(sc, ff, kk):
            wyc = kkw_pool.tile([128, KH, 128], bf16, name="wyc")
            sy.dma_start(out=wyc[:, :, :], in_=wkey_d[ff])
            pkk = mm_tile()
            for k in range(KH):
                nc.tensor.matmul(pkk[:, :], wyc[:, k, :],
                                 h2s[:, k, sc * SC:(sc + 1) * SC],
                                 start=(k == 0), stop=(k == KH - 1))
            u_t = vtmp_pool.tile([128, SC], bf16, name="u_t")
            act.activation(out=u_t[:, :], in_=pkk[:, :], func=Act.Relu,
                           bias=bkey_t[:, ff:ff + 1], scale=sqrt_skk)
            vec.tensor_mul(out=kk[:, ff, :], in0=u_t[:, :], in1=u_t[:, :])

        def p6(sc, kk):
            for hout in range(KH):
                wv8 = wvg_pool.tile([128, KF, 128], vg_dt, tag="wv8",
                                    name="wv8")
                sy.dma_start(out=wv8[:, :, :], in_=wval_d[hout])
                wg8 = wvg_pool.tile([128, KF, 128], vg_dt, tag="wg8",
                                    name="wg8")
                sy.dma_start(out=wg8[:, :, :], in_=wgate_d[hout])
                for dc in range(NDC):
                    dsl = slice(dc * DC, (dc + 1) * DC)
                    xsl = slice(sc * SC + dc * DC, sc * SC + (dc + 1) * DC)
                    psv = psum.tile([128, DC], f32, tag="mm", bufs=6,
                                    name="psv")
                    psg = psum.tile([128, DC], f32, tag="mm", bufs=6,
                                    name="psg")
                    if VG_FP8:
                        for f in range(KF // 2):
                            nc.tensor.matmul(psv[:, :],
                                             wv8[:, 2 * f:2 * f + 2, :],
                                             kk[:, 2 * f:2 * f + 2, dsl],
                                             start=(f == 0),
                                             stop=(f == KF // 2 - 1),
                                             perf_mode=DR)
                        for f in range(KF // 2):
                            nc.tensor.matmul(psg[:, :],
                                             wg8[:, 2 * f:2 * f + 2, :],
                                             kk[:, 2 * f:2 * f + 2, dsl],
                                             start=(f == 0),
                                             stop=(f == KF // 2 - 1),
                                             perf_mode=DR)
                    else:
                        for f in range(KF):
                            nc.tensor.matmul(psv[:, :], wv8[:, f, :],
                                             kk[:, f, dsl],
                                             start=(f == 0),
                                             stop=(f == KF - 1))
                        for f in range(KF):
                            nc.tensor.matmul(psg[:, :], wg8[:, f, :],
                                             kk[:, f, dsl],
                                             start=(f == 0),
                                             stop=(f == KF - 1))
                    sig_t = fin_pool.tile([128, DC], f32, name="sig_t")
                    act.activation(out=sig_t[:, :], in_=psg[:, :],
                                   func=Act.Sigmoid, scale=scl_t[:, 1:2])
                    glu_t = fin_pool.tile([128, DC], f32, name="glu_t")
                    vec.tensor_scalar_mul(out=glu_t[:, :], in0=psv[:, :],
                                          scalar1=scl_t[:, 0:1])
                    vec.tensor_mul(out=glu_t[:, :], in0=glu_t[:, :],
                                   in1=sig_t[:, :])
                    vec.tensor_add(out=glu_t[:, :], in0=glu_t[:, :],
                                   in1=x1T[:, hout, xsl])
                    for j in range(DC // 128):
                        tp = trp_tile()
                        nc.tensor.transpose(tp[:, :],
                                            glu_t[:, j * 128:(j + 1) * 128],
                                            ident[:, :])
                        ot = ot_pool.tile([128, 128], f32, name="ot")
                        vec.tensor_copy(out=ot[:, :], in_=tp[:, :])
                        tt = (sc * SC + dc * DC) // 128 + j
                        sy.dma_start(
                            out=out_d[tt * 128:(tt + 1) * 128,
                                      hout * 128:(hout + 1) * 128],
                            in_=ot[:, :])

        kk0 = kk_pool.tile([128, KF, SC], vg_dt, tag="kk", name="kk")
        for ff in range(4):
            p5_group(0, ff, kk0)
        mb1 = p3_bc(fin1)
        p4(1, *mb1)
        for ff in range(4, KF):
            p5_group(0, ff, kk0)
        ln2_tmp.release()
        wvg_pool = tc.alloc_tile_pool(name="wvg_pool", bufs=2)
        fin_pool = tc.alloc_tile_pool(name="fin_pool", bufs=4)
        ot_pool = tc.alloc_tile_pool(name="ot_pool", bufs=4)
        p6(0, kk0)
        kk1 = kk_pool.tile([128, KF, SC], vg_dt, tag="kk", name="kk")
        for ff in range(KF):
            p5_group(1, ff, kk1)
        p6(1, kk1)
        ot_pool.release()
        fin_pool.release()
        wvg_pool.release()
        kkw_pool.release()
        kk_pool.release()
        h2_pool.release()
        x1_pool.release()
        vtmp_pool.release()
        consts.release()
        psum.release()
    nc.finalize()
    return nc


# ---------------------------------------------------------------------------
# host side
# ---------------------------------------------------------------------------

def _ln_np(x, s, b):
    m = x.mean(-1, keepdims=True)
    vv = ((x - m) ** 2).mean(-1, keepdims=True)
    return (x - m) / np.sqrt(vv + 1e-5) * s + b


def _h2hat_row(xrow, att_state_b, ln1_s, ln1_b, ln2_s, ln2_b, td, lvl_w,
               lvl_b, Wv, Wk, Wr, Wo):
    """(x1 - m)/std for a single token row (LN2 without scale/bias)."""
    h = _ln_np(xrow[None, :], ln1_s, ln1_b)[0]
    vv = h @ Wv
    kk = h @ Wk
    rr = 1.0 / (1.0 + np.exp(-(h @ Wr)))
    lg = h @ lvl_w + lvl_b
    e = np.exp(lg - lg.max())
    lw = e / e.sum()
    decay = np.exp(-np.exp(td))
    weighted = (lw[None, :] @ (att_state_b * decay))[0] + kk * vv
    att = (rr * weighted) @ Wo
    x1 = xrow + att
    m = x1.mean()
    sd = np.sqrt(((x1 - m) ** 2).mean() + 1e-5)
    return ((x1 - m) / sd).astype(np.float32)


def _arrange_hkh(W):
    """[H, H] -> [KH, 128, KH, 128]: arr[ho, p, k, m] = W[k*128+p, ho*128+m]"""
    Wr = W.reshape(KH, 128, -1, 128)            # [k, p, ho, m]
    return np.ascontiguousarray(Wr.transpose(2, 1, 0, 3))


def _arrange_cols(v):
    """[H] -> [128, KH]: arr[p, k] = v[k*128+p]"""
    return np.ascontiguousarray(v.reshape(-1, 128).T)


_BUILT = None


def _get_built():
    global _BUILT
    if _BUILT is None:
        _BUILT = build_bass()
    return _BUILT


def make_in_maps(x, att_state, cm_state, ln1_s, ln1_b, ln2_s, ln2_b,
                 td_multi, lvl_w, lvl_b, Wv, Wk, Wr, Wo, tmk,
                 Wkey, Wval, Wgate):
    f = np.float32
    x = np.asarray(x, f)
    att_state = np.asarray(att_state, f)
    cm_state = np.asarray(cm_state, f)
    ln1_s, ln1_b = np.asarray(ln1_s, f), np.asarray(ln1_b, f)
    ln2_s, ln2_b = np.asarray(ln2_s, f), np.asarray(ln2_b, f)
    td = np.asarray(td_multi, f)
    lvl_w, lvl_b = np.asarray(lvl_w, f), np.asarray(lvl_b, f)
    Wv, Wk, Wr, Wo = (np.asarray(a, f) for a in (Wv, Wk, Wr, Wo))
    tmk = np.asarray(tmk, f)
    Wkey, Wval, Wgate = (np.asarray(a, f) for a in (Wkey, Wval, Wgate))

    # fold LN1 scale into Wv/Wk/Wr/lvl_w; LN1 bias becomes output biases
    decay = np.exp(-np.exp(td))
    sqrt_skk = np.sqrt(S_KK) if VG_FP8 else 1.0
    Wr_f = ln1_s[:, None] * Wr
    s_wr = 224.0 / max(np.abs(Wr_f).max(), 1e-9)
    s_h = 16.0
    if VG_FP8:
        s_wv = 224.0 / max(np.abs(Wval).max(), 1e-9)
        s_wg = 224.0 / max(np.abs(Wgate).max(), 1e-9)
        wval_a = np.ascontiguousarray(
            _arrange_khf(np.clip(Wval * s_wv, -240, 240)).astype(E4NP))
        wgate_a = np.ascontiguousarray(
            _arrange_khf(np.clip(Wgate * s_wg, -240, 240)).astype(E4NP))
        scl = np.tile(np.array([1.0 / (S_KK * s_wv), 1.0 / (S_KK * s_wg),
                                1.0 / (s_h * s_wr)], f), (128, 1))
    else:
        wval_a = np.ascontiguousarray(_arrange_khf(Wval).astype(BFNP))
        wgate_a = np.ascontiguousarray(_arrange_khf(Wgate).astype(BFNP))
        scl = np.tile(np.array([1.0, 1.0, 1.0 / (s_h * s_wr)], f),
                      (128, 1))

    shared = {
        "lvl_w": np.ascontiguousarray(
            (ln1_s[:, None] * lvl_w).reshape(KH, 128, D)
            .transpose(1, 0, 2)).astype(BFNP),
        "lvl_b": lvl_b + ln1_b @ lvl_w,
        "tmk": _arrange_cols(tmk),
        "bv": _arrange_cols(ln1_b @ Wv),
        "bk": _arrange_cols(ln1_b @ Wk),
        "br": _arrange_cols(ln1_b @ Wr),
        "bkey": np.ascontiguousarray(
            ((ln2_b @ Wkey) * sqrt_skk).reshape(KF, 128).T),
        "Wv": _arrange_hkh(ln1_s[:, None] * Wv).astype(BFNP),
        "Wk": _arrange_hkh(ln1_s[:, None] * Wk).astype(BFNP),
        "Wr": _arrange_hkh(np.clip(Wr_f * s_wr, -240, 240)).astype(E4NP),
        "Wo": _arrange_hkh(Wo).astype(BFNP),
        "Wkey": _arrange_khf_key(ln2_s[:, None] * Wkey).astype(BFNP),
        "Wval": wval_a,
        "Wgate": wgate_a,
        "scl": scl,
    }
    shared = {k: np.ascontiguousarray(v) for k, v in shared.items()}

    in_maps = []
    for c in range(NCORES):
        b, piece = c // 2, c % 2
        t0 = piece * S
        if piece == 0:
            shift = (cm_state[b] - ln2_b) / ln2_s
        else:
            shift = _h2hat_row(x[b, t0 - 1], att_state[b], ln1_s, ln1_b,
                               ln2_s, ln2_b, td, lvl_w, lvl_b, Wv, Wk, Wr, Wo)
        in_maps.append({
            "xbf": np.ascontiguousarray(x[b, t0:t0 + S].astype(BFNP)),
            "shift_in": np.ascontiguousarray(
                shift.reshape(KH, 128).T.astype(BFNP)),
            "asd": np.ascontiguousarray((att_state[b] * decay).astype(BFNP)),
            **shared,
        })
    return in_maps


def _arrange_khf(W):
    """[FF, H] -> [KH, 128, KF, 128]: arr[ho, p, f, m] = W[f*128+p, ho*128+m]"""
    Wr = W.reshape(KF, 128, KH, 128)            # [f, p, ho, m]
    return np.ascontiguousarray(Wr.transpose(2, 1, 0, 3))


def _arrange_khf_key(W):
    """[H, FF] -> [KF, 128, KH, 128]: arr[fo, p, k, m] = W[k*128+p, fo*128+m]"""
    Wr = W.reshape(KH, 128, KF, 128)            # [k, p, fo, m]
    return np.ascontiguousarray(Wr.transpose(2, 1, 0, 3))


def kernel(x, att_state, cm_state, ln1_s, ln1_b, ln2_s, ln2_b,
           td_multi, lvl_w, lvl_b, Wv, Wk, Wr, Wo, tmk,
           Wkey, Wval, Wgate):
    from concourse.bass_utils import run_bass_kernel_spmd

    in_maps = make_in_maps(x, att_state, cm_state, ln1_s, ln1_b, ln2_s,
                           ln2_b, td_multi, lvl_w, lvl_b, Wv, Wk, Wr, Wo,
                           tmk, Wkey, Wval, Wgate)
    nc = _get_built()
    res = run_bass_kernel_spmd(nc, in_maps, list(range(NCORES)))
    out = np.empty((B, T, H), np.float32)
    for c in range(NCORES):
        b, piece = c // 2, c % 2
        out[b, piece * S:(piece + 1) * S] = res.results[c]["out"]
    return out
